# revision 40
# baseline (speedup 1.0000x reference)
"""ChebNet (K=4, two layers, log_softmax) on 8 Trainium2 NeuronCores.

Self-contained: takes FULL inputs, shards by destination node internally,
runs a single SPMD Bass kernel on cores 0-7 (with ncfw AllGather between
propagation hops), returns the FULL [N, 32] output.

Math (Horner re-association so propagations happen at output width):
    y = sum_k T_k(L) x @ W[k] + b,  T_k Chebyshev,  L = -D^-1/2 A D^-1/2
      = U0 + L(U1 + L(U2 + L U3)),  U_j = x @ V_j
    V0 = W0 - W2, V1 = W1 - 3 W3, V2 = 2 W2, V3 = 4 W3
    L S = -dis * A (dis * S)   with dis = where(deg>0, 1/sqrt(max(deg,1)), 0)
Scaled-space recurrence (tables hold S_hat = dis * S):
    S_hat3 = dis*U3;  S_hat_{j} = dis*U_j - dis^2 * (A S_hat_{j+1})
    final: y = U_0 - dis * (A S_hat_1) + b
Per hop, A S_hat is an edge-gather + segment-sum done as:
    indirect-DMA row gather (dest-sorted edge stream) ->
    one-hot (seg == iota) built on VectorE ->
    TensorE matmul accumulation into PSUM per 128-dest tile.
"""

import sys

if "/opt/trn_rl_repo" not in sys.path:
    sys.path.insert(0, "/opt/trn_rl_repo")

import math
from contextlib import ExitStack
from dataclasses import dataclass, field

import numpy as np

P = 128
PADIDX = 1 << 20


@dataclass
class Cfg:
    n: int            # nodes
    m: int            # cores
    c_in: int = 128
    c_hid: int = 64
    c_out: int = 32
    cw: int = 64      # gather-table row width (f32) for both layers
    gch: int = 8      # chunks per gather call (>8 overflows the SWDGE
                      # descriptor ring and hangs the device)
    ob: int = 32      # one-hot columns per build batch
    pb: int = 8       # psum tiles per writeout batch
    # filled by finish/preprocess
    b: int = 0        # dests per core
    t: int = 0        # 128-dest tiles per core
    bp: int = 0       # padded block rows = t*128
    nbuck: int = 4    # source buckets = node-block quarters
    qtiles: list = field(default_factory=list)  # tiles per quarter
    qrows: list = field(default_factory=list)   # rows per quarter (tile-aligned)
    qstart: list = field(default_factory=list)  # local row start per quarter
    nch: int = 0      # total chunks in edge stream
    nseg: int = 0     # total (tile,chunk) matmul segments
    chb: list = field(default_factory=list)     # global chunk base per bucket
    chn: list = field(default_factory=list)     # chunks per bucket
    sched: list = field(default_factory=list)   # [nb][t] = (c0,c1) local or None

    def finish(self):
        assert self.n % self.m == 0
        self.b = self.n // self.m
        self.t = (self.b + P - 1) // P
        self.bp = self.t * P
        # tile-aligned quarters of the node block; bucket q = all cores'
        # quarter-q rows (int16 gather range: 8*qrows <= 32767)
        tq = (self.t + 3) // 4
        self.qtiles = [tq, tq, tq, self.t - 3 * tq]
        self.qrows = [q * P for q in self.qtiles]
        self.qstart = [0]
        for q in self.qrows[:-1]:
            self.qstart.append(self.qstart[-1] + q)
        assert all(8 * q <= 32767 for q in self.qrows)


def preprocess(edge_index: np.ndarray, cfg: Cfg):
    """Shard edges by destination row-block; per core build a
    (bucket, tile)-major edge stream.  Cells (bucket x dest-tile) are
    padded to the cross-core max count (NOT 128-aligned), so the chunk
    schedule is shared across cores while chunks may straddle dest-tile
    boundaries; straddled chunks get one matmul segment per tile.
    Buckets are node-block quarters interleaved across cores so each
    bucket's table rows come from one AllGather piece."""
    import ml_dtypes

    row = np.asarray(edge_index[0], dtype=np.int64)
    col = np.asarray(edge_index[1], dtype=np.int64)
    n, m, b, t = cfg.n, cfg.m, cfg.b, cfg.t
    nb = cfg.nbuck
    qstart = np.array(cfg.qstart + [cfg.bp], dtype=np.int64)
    qrows = np.array(cfg.qrows, dtype=np.int64)

    deg = np.bincount(row, minlength=n).astype(np.float32)

    per_core = []
    counts = np.zeros((m, nb, t), dtype=np.int64)
    for c in range(m):
        sel = (row >= c * b) & (row < (c + 1) * b)
        d = (row[sel] - c * b).astype(np.int64)
        s = col[sel].astype(np.int64)
        sc, lr = s // b, s % b
        q = np.searchsorted(qstart, lr, side="right") - 1
        lidx = sc * qrows[q] + (lr - qstart[q])   # int16-safe bucket row
        tile_id = d >> 7
        key = q * t + tile_id
        order = np.argsort(key, kind="stable")
        d, lidx, key = d[order], lidx[order], key[order]
        np.add.at(counts[c], (q[order], tile_id[order]), 1)
        per_core.append((d, lidx, key))

    M = counts.max(axis=0)                        # [nb, t] cell sizes
    # stream layout: (tile-group, bucket)-major passes; cells back to
    # back within a pass, pass tail 128-padded.  Group g = the dest
    # tiles of node-block quarter g, so acc quarter g is final after
    # its 4 bucket passes (early blk writeout -> early next-hop AG).
    gt0 = np.concatenate([[0], np.cumsum(cfg.qtiles)]).astype(np.int64)
    cell_start = np.zeros((nb, t), dtype=np.int64)
    cell_pass = np.zeros((nb, t), dtype=np.int64)  # pass index of cell
    cfg.chb, cfg.chn, cfg.sched = {}, {}, {}
    pass_slot_base = {}
    base = 0
    nseg = 0
    seg_tile, seg_chunk = [], []                  # per segment (global chunk)
    for g in range(nb):
        for bk in range(nb):
            pos = 0
            sched_p = {}
            for ti in range(int(gt0[g]), int(gt0[g + 1])):
                cell_start[bk, ti] = pos
                cell_pass[bk, ti] = g * nb + bk
                if M[bk, ti] == 0:
                    sched_p[ti] = None
                    continue
                c0, c1 = pos // P, (pos + M[bk, ti] - 1) // P
                sched_p[ti] = (int(c0), int(c1))
                for cl in range(c0, c1 + 1):
                    seg_tile.append(ti)
                    seg_chunk.append(base // P + cl)
                nseg += c1 - c0 + 1
                pos += M[bk, ti]
            nloc = (pos + P - 1) // P
            cfg.chb[(g, bk)] = base // P
            cfg.chn[(g, bk)] = int(nloc)
            cfg.sched[(g, bk)] = sched_p
            pass_slot_base[(g, bk)] = base
            base += nloc * P
    S = base
    cfg.nch = S // P
    cfg.nseg = nseg
    seg_tile = np.array(seg_tile, dtype=np.int64)
    seg_chunk = np.array(seg_chunk, dtype=np.int64)

    pass_base = np.zeros((nb, t), dtype=np.int64)
    for g in range(nb):
        for bk in range(nb):
            pass_base[bk, int(gt0[g]) : int(gt0[g + 1])] = pass_slot_base[(g, bk)]

    idx_all, seg_all, degt_all = [], [], []
    for c in range(m):
        d, lidx, key = per_core[c]
        idx = np.zeros(S, dtype=np.int16)          # pad -> bucket row 0
        stile = np.full(S, -2, dtype=np.int64)     # slot -> dest tile
        sval = np.zeros(S, dtype=np.float32)       # slot -> dest & 127
        cnt_flat = counts[c].reshape(-1)
        pos_in_cell = np.arange(d.size) - np.concatenate(
            [[0], np.cumsum(cnt_flat)]
        )[key]
        slot = (
            pass_base.reshape(-1)[key]
            + cell_start.reshape(-1)[key]
            + pos_in_cell
        )
        idx[slot] = lidx.astype(np.int16)
        stile[slot] = key % t
        sval[slot] = (d & 127).astype(np.float32)
        # segment columns: chunk slots where the edge belongs to this
        # segment's tile; everything else -1
        ch_t = stile.reshape(cfg.nch, P)[seg_chunk]      # [nseg, P]
        ch_v = sval.reshape(cfg.nch, P)[seg_chunk]
        segm = np.where(ch_t == seg_tile[:, None], ch_v, -1.0)
        seg_all.append(
            np.ascontiguousarray(segm.T).astype(ml_dtypes.bfloat16)
        )
        # dma_gather idx layout: slot i -> partition i%16 col i//16,
        # replicated across the 8 groups of 16 partitions
        idx16 = idx.reshape(S // 16, 16).T       # [16, S/16]
        idx16 = np.tile(idx16, (8, 1))           # [128, S/16]
        idx_all.append(np.ascontiguousarray(idx16))
        degb = np.zeros(cfg.bp, dtype=np.float32)
        degb[:b] = deg[c * b : (c + 1) * b]
        degt_all.append(np.ascontiguousarray(degb.reshape(t, P).T))  # [P, t]
    return idx_all, seg_all, degt_all


def build_program(cfg: Cfg):
    import concourse.bass as bass
    import concourse.tile as tile
    from concourse import bacc, mybir

    f32 = mybir.dt.float32
    bf16 = mybir.dt.bfloat16
    n, m, b, T, bp = cfg.n, cfg.m, cfg.b, cfg.t, cfg.bp
    CIN, HID, OUT, CW = cfg.c_in, cfg.c_hid, cfg.c_out, cfg.cw
    GCH, OB, PB = cfg.gch, cfg.ob, cfg.pb
    NCH, NB = cfg.nch, cfg.nbuck
    NSEG, CHB, CHN, SCHED = cfg.nseg, cfg.chb, cfg.chn, cfg.sched
    QT, QR, QS = cfg.qtiles, cfg.qrows, cfg.qstart
    NTAB = m * bp

    nc = bacc.Bacc(
        "TRN2",
        target_bir_lowering=False,
        debug=False,
        num_devices=m,
        num_swdge_queues=4,
    )

    # ---- I/O ----
    x_t = nc.dram_tensor("x_blk", [bp, CIN], f32, kind="ExternalInput")
    w1_t = nc.dram_tensor("W1", [4, CIN, HID], f32, kind="ExternalInput")
    b1_t = nc.dram_tensor("b1", [HID], f32, kind="ExternalInput")
    w2_t = nc.dram_tensor("W2", [4, HID, OUT], f32, kind="ExternalInput")
    b2_t = nc.dram_tensor("b2", [OUT], f32, kind="ExternalInput")
    degt_t = nc.dram_tensor("deg_t", [P, T], f32, kind="ExternalInput")
    idx_t = nc.dram_tensor(
        "idx", [P, NCH * 8], mybir.dt.int16, kind="ExternalInput"
    )
    seg_t = nc.dram_tensor("seg", [P, NSEG], mybir.dt.bfloat16, kind="ExternalInput")
    y_t = nc.dram_tensor("y_blk", [bp, OUT], f32, kind="ExternalOutput")

    # ---- internal DRAM ----
    u1 = nc.dram_tensor("U1", [3, bp, HID], f32)
    u2 = nc.dram_tensor("U2", [3, bp, OUT], f32)
    # bf16 gather rows must be 256B -> 128 bf16 wide; only cols :CW are real.
    CWB = 128
    blks, tabs = {}, {}
    for l in (1, 2):
        for j in (3, 2, 1):
            blks[(l, j)] = nc.dram_tensor(f"blk_{l}_{j}", [bp, CWB], bf16)
            tabs[(l, j)] = nc.dram_tensor(f"tab_{l}_{j}", [NTAB, CWB], bf16)

    import ml_dtypes

    iota_np = np.broadcast_to(
        np.tile(np.arange(P, dtype=np.float32), OB), (P, OB * P)
    ).astype(ml_dtypes.bfloat16)
    iota_d = nc.inline_tensor(iota_np, name="iota_rep")
    ident_d = nc.inline_tensor(np.eye(P, dtype=np.float32), name="ident")

    with ExitStack() as ctx:
        tc = ctx.enter_context(tile.TileContext(nc, num_cores=m))
        const = ctx.enter_context(tc.tile_pool(name="const", bufs=1))
        xp = ctx.enter_context(tc.tile_pool(name="xp", bufs=3))
        wp = ctx.enter_context(tc.tile_pool(name="wp", bufs=3))
        up = ctx.enter_context(tc.tile_pool(name="up", bufs=3))
        gp = ctx.enter_context(tc.tile_pool(name="gp", bufs=20))
        ip = ctx.enter_context(tc.tile_pool(name="ip", bufs=20))
        op = ctx.enter_context(tc.tile_pool(name="op", bufs=3))
        accp = ctx.enter_context(tc.tile_pool(name="accp", bufs=2))
        pst = ctx.enter_context(tc.tile_pool(name="pst", bufs=2, space="PSUM"))
        psu = ctx.enter_context(tc.tile_pool(name="psu", bufs=2, space="PSUM"))
        psa = ctx.enter_context(tc.tile_pool(name="psa", bufs=4, space="PSUM"))

        # ---- constants ----
        iota_s = const.tile([P, OB * P], bf16)
        nc.sync.dma_start(iota_s[:], iota_d[:, :])
        ident_s = const.tile([P, P], f32)
        nc.sync.dma_start(ident_s[:], ident_d[:, :])

        seg_s = const.tile([P, NSEG], bf16)
        nc.sync.dma_start(seg_s[:], seg_t[:, :])

        # V1cat [CIN, 4, HID], V2cat [HID, 4, OUT]
        w1s = const.tile([CIN, 4, HID], f32)
        nc.sync.dma_start(w1s[:], w1_t[:, :, :].rearrange("k p c -> p k c"))
        v1 = const.tile([CIN, 4, HID], f32)
        nc.vector.tensor_sub(v1[:, 0, :], w1s[:, 0, :], w1s[:, 2, :])
        nc.vector.tensor_scalar(
            out=v1[:, 1, :], in0=w1s[:, 3, :], scalar1=-3.0, scalar2=None,
            op0=mybir.AluOpType.mult,
        )
        nc.vector.tensor_add(v1[:, 1, :], v1[:, 1, :], w1s[:, 1, :])
        nc.vector.tensor_scalar(
            out=v1[:, 2, :], in0=w1s[:, 2, :], scalar1=2.0, scalar2=None,
            op0=mybir.AluOpType.mult,
        )
        nc.vector.tensor_scalar(
            out=v1[:, 3, :], in0=w1s[:, 3, :], scalar1=4.0, scalar2=None,
            op0=mybir.AluOpType.mult,
        )

        w2s = const.tile([HID, 4, OUT], f32)
        nc.sync.dma_start(w2s[:], w2_t[:, :, :].rearrange("k p c -> p k c"))
        v2 = const.tile([HID, 4, OUT], f32)
        nc.vector.tensor_sub(v2[:, 0, :], w2s[:, 0, :], w2s[:, 2, :])
        nc.vector.tensor_scalar(
            out=v2[:, 1, :], in0=w2s[:, 3, :], scalar1=-3.0, scalar2=None,
            op0=mybir.AluOpType.mult,
        )
        nc.vector.tensor_add(v2[:, 1, :], v2[:, 1, :], w2s[:, 1, :])
        nc.vector.tensor_scalar(
            out=v2[:, 2, :], in0=w2s[:, 2, :], scalar1=2.0, scalar2=None,
            op0=mybir.AluOpType.mult,
        )
        nc.vector.tensor_scalar(
            out=v2[:, 3, :], in0=w2s[:, 3, :], scalar1=4.0, scalar2=None,
            op0=mybir.AluOpType.mult,
        )

        b1s = const.tile([P, HID], f32)
        nc.sync.dma_start(b1s[:1, :], b1_t[:].rearrange("(o c) -> o c", o=1))
        nc.gpsimd.partition_broadcast(b1s[:, :], b1s[:1, :])
        b2s = const.tile([P, OUT], f32)
        nc.sync.dma_start(b2s[:1, :], b2_t[:].rearrange("(o c) -> o c", o=1))
        nc.gpsimd.partition_broadcast(b2s[:, :], b2s[:1, :])

        # DMA-completion semaphores for prepare_only gathers (one per queue)
        dma_sems = [nc.alloc_semaphore(f"swdge_dma{q}") for q in range(4)]

        # ---- dis, -dis, -dis^2 in [P, T] (partition p, slot t) = dest 128t+p ----
        degs = const.tile([P, T], f32)
        nc.sync.dma_start(degs[:], degt_t[:, :])
        dis = const.tile([P, T], f32)
        ndis = const.tile([P, T], f32)
        ndis2 = const.tile([P, T], f32)
        tmp = const.tile([P, T], f32)
        nc.vector.tensor_scalar(
            out=tmp[:], in0=degs[:], scalar1=1.0, scalar2=None,
            op0=mybir.AluOpType.max,
        )
        nc.scalar.activation(tmp[:], tmp[:], mybir.ActivationFunctionType.Sqrt)
        nc.vector.reciprocal(dis[:], tmp[:])
        nc.vector.tensor_scalar(
            out=tmp[:], in0=degs[:], scalar1=0.0, scalar2=None,
            op0=mybir.AluOpType.is_gt,
        )
        nc.vector.tensor_mul(dis[:], dis[:], tmp[:])
        nc.vector.tensor_scalar(
            out=ndis[:], in0=dis[:], scalar1=-1.0, scalar2=None,
            op0=mybir.AluOpType.mult,
        )
        nc.vector.tensor_mul(ndis2[:], dis[:], ndis[:])

        # ---- prologue: U_j = src @ V_j ; U1/U2 to HBM (j=1,2 scaled by dis);
        #      j=3 scaled -> blk  ----
        def prologue(src, cl, w, vcat, u_hbm, blk_hbm, from_sbuf=False):
            for k in range(T):
                if from_sbuf:
                    xc = src[:, k, :]
                else:
                    xc = xp.tile([P, cl], f32, tag="xc")
                    nc.sync.dma_start(xc[:], src[k * P : (k + 1) * P, :])
                tp = pst.tile([cl, P], f32, space="PSUM", tag="tp")
                nc.tensor.transpose(out=tp[:, :], in_=xc[:, :], identity=ident_s[:])
                xT = wp.tile([cl, P], f32, tag="xT")
                nc.vector.tensor_copy(xT[:], tp[:, :])
                upsum = psu.tile([P, 4, w], f32, space="PSUM", tag="upsum")
                nc.tensor.matmul(
                    out=upsum[:].rearrange("p a c -> p (a c)"),
                    lhsT=xT[:, :],
                    rhs=vcat[:].rearrange("p a c -> p (a c)"),
                    start=True, stop=True,
                )
                ut = wp.tile([P, 3, w], f32, tag="ut")
                # fold bias into U_0 once (y = U0 + b - dis*(A S1))
                bb = b1s if w == HID else b2s
                nc.vector.tensor_add(ut[:, 0, :], upsum[:, 0, :], bb[:, :w])
                for j in (1, 2):
                    nc.vector.tensor_scalar(
                        out=ut[:, j, :], in0=upsum[:, j, :],
                        scalar1=dis[:, k : k + 1], scalar2=None,
                        op0=mybir.AluOpType.mult,
                    )
                nc.sync.dma_start(
                    u_hbm[:, k * P : (k + 1) * P, :].rearrange("a p c -> p a c"),
                    ut[:],
                )
                bt = wp.tile([P, w], bf16, tag="bt")
                nc.vector.tensor_scalar(
                    out=bt[:], in0=upsum[:, 3, :],
                    scalar1=dis[:, k : k + 1], scalar2=None,
                    op0=mybir.AluOpType.mult,
                )
                nc.sync.dma_start(blk_hbm[k * P : (k + 1) * P, :w], bt[:])

        # ---- one hop: table = AG(blk_in) split per bucket (overlaps the
        # gather passes); acc = scale*(A_gather) + U_j.  Edge stream is
        # (bucket, tile)-major with exact-max cells: chunks may straddle
        # tile boundaries, one matmul segment per (tile, chunk).  On the
        # last bucket pass, finished acc quarters are cast+written to
        # blk_out so the next hop's AllGather pieces can start early.
        def hop(l, j, blk_in, table, w, blk_out):
            for q in range(NB):
                nc.gpsimd.collective_compute(
                    "AllGather",
                    mybir.AluOpType.bypass,
                    replica_groups=[list(range(m))],
                    ins=[blk_in[QS[q] : QS[q] + QR[q], :].opt()],
                    outs=[table[8 * QS[q] : 8 * (QS[q] + QR[q]), :].opt()],
                )
            final = j == 0
            u_hbm = u1 if l == 1 else u2
            acc = accp.tile([P, T, w], f32, tag="acc")

            gath = None
            oneh = None
            o = 0           # segment ordinal (one-hot column)
            gcall = 0       # gather-call counter (round-robin SWDGE queues:
                            # each queue runs on its own Q7 cpu pair)
            gt0 = 0
            for g in range(NB):
                gtn = QT[g]
                for bk in range(NB):
                    tab_lo = 8 * QS[bk]
                    tab_hi = 8 * (QS[bk] + QR[bk])
                    cb = CHB[(g, bk)]
                    nloc = CHN[(g, bk)]
                    sched_p = SCHED[(g, bk)]
                    gnext = 0   # next ungathered pass-local chunk
                    gbase = 0   # current gather batch start (pass-local)
                    nbatch = (gtn + PB - 1) // PB
                    for bi in range(nbatch):
                        t0 = gt0 + bi * PB
                        tcnt = min(PB, gt0 + gtn - t0)
                        psum = psa.tile(
                            [P, PB, w], f32, space="PSUM", tag="apsum"
                        )
                        for tt in range(tcnt):
                            ti = t0 + tt
                            rng = sched_p[ti]
                            if rng is None:
                                nc.vector.memset(psum[:, tt, :], 0.0)
                                continue
                            c0l, c1l = rng
                            for k, cl in enumerate(range(c0l, c1l + 1)):
                                if cl >= gnext:
                                    glen = min(GCH, nloc - gnext)
                                    gbase = gnext
                                    cg = cb + gnext
                                    idxt = ip.tile(
                                        [P, GCH * 8], mybir.dt.int16,
                                        tag="idxt",
                                    )
                                    nc.sync.dma_start(
                                        idxt[:, : glen * 8],
                                        idx_t[:, cg * 8 : (cg + glen) * 8],
                                    )
                                    gath = gp.tile(
                                        [P, GCH, CWB], bf16, tag="gath"
                                    )
                                    nc.gpsimd.dma_gather(
                                        out_ap=gath[:, :glen, :],
                                        in_ap=table[tab_lo:tab_hi, :],
                                        idxs_ap=idxt[:, : glen * 8],
                                        num_idxs=glen * P,
                                        num_idxs_reg=glen * P,
                                        elem_size=CWB,
                                        queue_num=gcall % 4,
                                    )
                                    gcall += 1
                                    gnext += glen
                                gpos = cl - gbase
                                oc, opos = divmod(o, OB)
                                if opos == 0:
                                    olen = min(OB, NSEG - oc * OB)
                                    oneh = op.tile(
                                        [P, OB, P], bf16, tag="oneh"
                                    )
                                    nc.vector.tensor_tensor(
                                        out=oneh[:, :olen, :],
                                        in0=iota_s[:].rearrange(
                                            "p (a q) -> p a q", q=P
                                        )[:, :olen, :],
                                        in1=seg_s[:, oc * OB : oc * OB + olen]
                                        .to_broadcast([P, olen, P]),
                                        op=mybir.AluOpType.is_equal,
                                    )
                                nc.tensor.matmul(
                                    out=psum[:, tt, :],
                                    lhsT=oneh[:, opos, :],
                                    rhs=gath[:, gpos, :w],
                                    start=(k == 0),
                                    stop=(k == c1l - c0l),
                                )
                                o += 1
                        # fold this pass's aggregation into acc
                        sl = slice(t0, t0 + tcnt)
                        if bk == 0:
                            nc.vector.tensor_copy(
                                acc[:, sl, :], psum[:, :tcnt, :]
                            )
                        else:
                            nc.vector.tensor_add(
                                acc[:, sl, :], acc[:, sl, :],
                                psum[:, :tcnt, :],
                            )
                        if bk == NB - 1:
                            scale = ndis if final else ndis2
                            nc.vector.tensor_tensor(
                                out=acc[:, sl, :],
                                in0=acc[:, sl, :],
                                in1=scale[:, sl].to_broadcast([P, tcnt, w]),
                                op=mybir.AluOpType.mult,
                            )
                            uti = up.tile([P, PB, w], f32, tag="uti")
                            nc.sync.dma_start(
                                uti[:, :tcnt, :],
                                u_hbm[
                                    j if not final else 0,
                                    t0 * P : (t0 + tcnt) * P, :,
                                ].rearrange("(a p) c -> p a c", p=P),
                            )
                            nc.vector.tensor_add(
                                acc[:, sl, :], acc[:, sl, :],
                                uti[:, :tcnt, :],
                            )
                            if final and l == 1:
                                nc.scalar.activation(
                                    acc[:, sl, :],
                                    acc[:, sl, :],
                                    mybir.ActivationFunctionType.Relu,
                                )
                # acc quarter g is final: cast + write to blk for the next
                # hop's AllGather piece g
                if blk_out is not None:
                    ab = wp.tile([P, gtn, w], bf16, tag="accb")
                    nc.vector.tensor_copy(ab[:], acc[:, gt0 : gt0 + gtn, :])
                    nc.sync.dma_start(
                        blk_out[QS[g] : QS[g] + QR[g], :w].rearrange(
                            "(a p) c -> p a c", p=P
                        ),
                        ab[:],
                    )
                gt0 += gtn
            return acc

        def acc_to_hbm(acc, dst, w):
            nc.sync.dma_start(
                dst[:, :].rearrange("(a p) c -> p a c", p=P), acc[:, :, :w]
            )

        # ================= layer 1 =================
        prologue(x_t, CIN, HID, v1, u1, blks[(1, 3)])
        acc = hop(1, 2, blks[(1, 3)], tabs[(1, 3)], HID, blks[(1, 2)])
        acc = hop(1, 1, blks[(1, 2)], tabs[(1, 2)], HID, blks[(1, 1)])
        acc = hop(1, 0, blks[(1, 1)], tabs[(1, 1)], HID, None)

        # ================= layer 2 (prologue reads h=acc from SBUF) ======
        prologue(acc, HID, OUT, v2, u2, blks[(2, 3)], from_sbuf=True)
        acc = hop(2, 2, blks[(2, 3)], tabs[(2, 3)], OUT, blks[(2, 2)])
        acc = hop(2, 1, blks[(2, 2)], tabs[(2, 2)], OUT, blks[(2, 1)])
        acc = hop(2, 0, blks[(2, 1)], tabs[(2, 1)], OUT, None)

        # ---- log_softmax over acc [P, T, OUT] ----
        red = wp.tile([P, T], f32, tag="red")
        nc.vector.tensor_reduce(
            out=red[:], in_=acc[:, :, :], axis=mybir.AxisListType.X,
            op=mybir.AluOpType.max,
        )
        xm = accp.tile([P, T, OUT], f32, tag="xm")
        nc.vector.tensor_tensor(
            out=xm[:], in0=acc[:, :, :], in1=red[:].to_broadcast([P, T, OUT]),
            op=mybir.AluOpType.subtract,
        )
        ex = accp.tile([P, T, OUT], f32, tag="ex")
        nc.scalar.activation(ex[:], xm[:], mybir.ActivationFunctionType.Exp)
        nc.vector.tensor_reduce(
            out=red[:], in_=ex[:, :, :], axis=mybir.AxisListType.X,
            op=mybir.AluOpType.add,
        )
        nc.scalar.activation(red[:], red[:], mybir.ActivationFunctionType.Ln)
        nc.vector.tensor_tensor(
            out=xm[:], in0=xm[:, :, :], in1=red[:].to_broadcast([P, T, OUT]),
            op=mybir.AluOpType.subtract,
        )
        nc.sync.dma_start(
            y_t[:, :].rearrange("(a p) c -> p a c", p=P), xm[:, :, :]
        )

    nc.compile()
    return nc


def make_in_maps(cfg: Cfg, inputs: dict, idx_all, seg_all, degt_all):
    x = np.asarray(inputs["x"], dtype=np.float32)
    maps = []
    for c in range(cfg.m):
        xb = np.zeros((cfg.bp, cfg.c_in), dtype=np.float32)
        xb[: cfg.b] = x[c * cfg.b : (c + 1) * cfg.b]
        maps.append(
            {
                "x_blk": xb,
                "W1": np.asarray(inputs["W1"], dtype=np.float32),
                "b1": np.asarray(inputs["b1"], dtype=np.float32),
                "W2": np.asarray(inputs["W2"], dtype=np.float32),
                "b2": np.asarray(inputs["b2"], dtype=np.float32),
                "deg_t": degt_all[c],
                "idx": idx_all[c],
                "seg": seg_all[c],
            }
        )
    return maps


def kernel(**inputs) -> np.ndarray:
    from concourse import bass_utils

    cfg = Cfg(n=100000, m=8)
    cfg.finish()
    edge_index = np.asarray(inputs["edge_index"])
    idx_all, seg_all, degt_all = preprocess(edge_index, cfg)
    nc = build_program(cfg)
    in_maps = make_in_maps(cfg, inputs, idx_all, seg_all, degt_all)
    res = bass_utils.run_bass_kernel_spmd(nc, in_maps, core_ids=list(range(cfg.m)))
    out = np.concatenate(
        [res.results[c]["y_blk"][: cfg.b] for c in range(cfg.m)], axis=0
    )
    return out.astype(np.float32)



# revision 43
# speedup vs baseline: 1.0614x; 1.0614x over previous
"""ChebNet (K=4, two layers, log_softmax) on 8 Trainium2 NeuronCores.

Self-contained: takes FULL inputs, shards by destination node internally,
runs a single SPMD Bass kernel on cores 0-7 (with ncfw AllGather between
propagation hops), returns the FULL [N, 32] output.

Math (Horner re-association so propagations happen at output width):
    y = sum_k T_k(L) x @ W[k] + b,  T_k Chebyshev,  L = -D^-1/2 A D^-1/2
      = U0 + L(U1 + L(U2 + L U3)),  U_j = x @ V_j
    V0 = W0 - W2, V1 = W1 - 3 W3, V2 = 2 W2, V3 = 4 W3
    L S = -dis * A (dis * S)   with dis = where(deg>0, 1/sqrt(max(deg,1)), 0)
Scaled-space recurrence (tables hold S_hat = dis * S):
    S_hat3 = dis*U3;  S_hat_{j} = dis*U_j - dis^2 * (A S_hat_{j+1})
    final: y = U_0 - dis * (A S_hat_1) + b
Per hop, A S_hat is an edge-gather + segment-sum done as:
    indirect-DMA row gather (dest-sorted edge stream) ->
    one-hot (seg == iota) built on VectorE ->
    TensorE matmul accumulation into PSUM per 128-dest tile.
"""

import sys

if "/opt/trn_rl_repo" not in sys.path:
    sys.path.insert(0, "/opt/trn_rl_repo")

import math
from contextlib import ExitStack
from dataclasses import dataclass, field

import numpy as np

P = 128
PADIDX = 1 << 20


@dataclass
class Cfg:
    n: int            # nodes
    m: int            # cores
    c_in: int = 128
    c_hid: int = 64
    c_out: int = 32
    cw: int = 64      # gather-table row width (f32) for both layers
    gch: int = 8      # chunks per gather call (>8 overflows the SWDGE
                      # descriptor ring and hangs the device)
    ob: int = 32      # one-hot columns per build batch
    pb: int = 8       # psum tiles per writeout batch
    # filled by finish/preprocess
    b: int = 0        # dests per core
    t: int = 0        # 128-dest tiles per core
    bp: int = 0       # padded block rows = t*128
    nbuck: int = 4    # source buckets = node-block quarters
    qtiles: list = field(default_factory=list)  # tiles per quarter
    qrows: list = field(default_factory=list)   # rows per quarter (tile-aligned)
    qstart: list = field(default_factory=list)  # local row start per quarter
    nch: int = 0      # total chunks in edge stream
    nseg: int = 0     # total (tile,chunk) matmul segments
    chb: list = field(default_factory=list)     # global chunk base per bucket
    chn: list = field(default_factory=list)     # chunks per bucket
    sched: list = field(default_factory=list)   # [nb][t] = (c0,c1) local or None

    def finish(self):
        assert self.n % self.m == 0
        self.b = self.n // self.m
        self.t = (self.b + P - 1) // P
        self.bp = self.t * P
        # tile-aligned quarters of the node block; bucket q = all cores'
        # quarter-q rows (int16 gather range: 8*qrows <= 32767)
        tq = (self.t + 3) // 4
        self.qtiles = [tq, tq, tq, self.t - 3 * tq]
        self.qrows = [q * P for q in self.qtiles]
        self.qstart = [0]
        for q in self.qrows[:-1]:
            self.qstart.append(self.qstart[-1] + q)
        assert all(8 * q <= 32767 for q in self.qrows)


def preprocess(edge_index: np.ndarray, cfg: Cfg):
    """Shard edges by destination row-block; per core build a
    (bucket, tile)-major edge stream.  Cells (bucket x dest-tile) are
    padded to the cross-core max count (NOT 128-aligned), so the chunk
    schedule is shared across cores while chunks may straddle dest-tile
    boundaries; straddled chunks get one matmul segment per tile.
    Buckets are node-block quarters interleaved across cores so each
    bucket's table rows come from one AllGather piece."""
    import ml_dtypes

    row = np.asarray(edge_index[0], dtype=np.int64)
    col = np.asarray(edge_index[1], dtype=np.int64)
    n, m, b, t = cfg.n, cfg.m, cfg.b, cfg.t
    nb = cfg.nbuck
    qstart = np.array(cfg.qstart + [cfg.bp], dtype=np.int64)
    qrows = np.array(cfg.qrows, dtype=np.int64)

    deg = np.bincount(row, minlength=n).astype(np.float32)

    per_core = []
    counts = np.zeros((m, nb, t), dtype=np.int64)
    for c in range(m):
        sel = (row >= c * b) & (row < (c + 1) * b)
        d = (row[sel] - c * b).astype(np.int64)
        s = col[sel].astype(np.int64)
        sc, lr = s // b, s % b
        q = np.searchsorted(qstart, lr, side="right") - 1
        lidx = sc * qrows[q] + (lr - qstart[q])   # int16-safe bucket row
        tile_id = d >> 7
        key = q * t + tile_id
        order = np.argsort(key, kind="stable")
        d, lidx, key = d[order], lidx[order], key[order]
        np.add.at(counts[c], (q[order], tile_id[order]), 1)
        per_core.append((d, lidx, key))

    M = counts.max(axis=0)                        # [nb, t] cell sizes
    # stream layout: (tile-group, bucket)-major passes; cells back to
    # back within a pass, pass tail 128-padded.  Group g = the dest
    # tiles of node-block quarter g, so acc quarter g is final after
    # its 4 bucket passes (early blk writeout -> early next-hop AG).
    gt0 = np.concatenate([[0], np.cumsum(cfg.qtiles)]).astype(np.int64)
    cell_start = np.zeros((nb, t), dtype=np.int64)
    cfg.chb, cfg.chn, cfg.sched = {}, {}, {}
    pass_slot_base = {}
    base = 0
    for g in range(nb):
        for bk in range(nb):
            pos = 0
            sched_p = {}
            for ti in range(int(gt0[g]), int(gt0[g + 1])):
                cell_start[bk, ti] = pos
                if M[bk, ti] == 0:
                    sched_p[ti] = None
                    continue
                c0, c1 = pos // P, (pos + M[bk, ti] - 1) // P
                sched_p[ti] = (int(c0), int(c1))
                pos += M[bk, ti]
            nloc = (pos + P - 1) // P
            cfg.chb[(g, bk)] = base // P
            cfg.chn[(g, bk)] = int(nloc)
            cfg.sched[(g, bk)] = sched_p
            pass_slot_base[(g, bk)] = base
            base += nloc * P
    S = base
    cfg.nch = S // P
    # one-hot segment order must match kernel consumption: bucket-major
    nseg = 0
    seg_tile, seg_chunk = [], []                  # per segment (global chunk)
    for bk in range(nb):
        for g in range(nb):
            cb = cfg.chb[(g, bk)]
            for ti in range(int(gt0[g]), int(gt0[g + 1])):
                rng = cfg.sched[(g, bk)][ti]
                if rng is None:
                    continue
                c0, c1 = rng
                for cl in range(c0, c1 + 1):
                    seg_tile.append(ti)
                    seg_chunk.append(cb + cl)
                nseg += c1 - c0 + 1
    cfg.nseg = nseg
    seg_tile = np.array(seg_tile, dtype=np.int64)
    seg_chunk = np.array(seg_chunk, dtype=np.int64)

    pass_base = np.zeros((nb, t), dtype=np.int64)
    for g in range(nb):
        for bk in range(nb):
            pass_base[bk, int(gt0[g]) : int(gt0[g + 1])] = pass_slot_base[(g, bk)]

    idx_all, seg_all, degt_all = [], [], []
    for c in range(m):
        d, lidx, key = per_core[c]
        idx = np.zeros(S, dtype=np.int16)          # pad -> bucket row 0
        stile = np.full(S, -2, dtype=np.int64)     # slot -> dest tile
        sval = np.zeros(S, dtype=np.float32)       # slot -> dest & 127
        cnt_flat = counts[c].reshape(-1)
        pos_in_cell = np.arange(d.size) - np.concatenate(
            [[0], np.cumsum(cnt_flat)]
        )[key]
        slot = (
            pass_base.reshape(-1)[key]
            + cell_start.reshape(-1)[key]
            + pos_in_cell
        )
        idx[slot] = lidx.astype(np.int16)
        stile[slot] = key % t
        sval[slot] = (d & 127).astype(np.float32)
        # segment columns: chunk slots where the edge belongs to this
        # segment's tile; everything else -1
        ch_t = stile.reshape(cfg.nch, P)[seg_chunk]      # [nseg, P]
        ch_v = sval.reshape(cfg.nch, P)[seg_chunk]
        segm = np.where(ch_t == seg_tile[:, None], ch_v, -1.0)
        seg_all.append(
            np.ascontiguousarray(segm.T).astype(ml_dtypes.bfloat16)
        )
        # dma_gather idx layout: slot i -> partition i%16 col i//16,
        # replicated across the 8 groups of 16 partitions
        idx16 = idx.reshape(S // 16, 16).T       # [16, S/16]
        idx16 = np.tile(idx16, (8, 1))           # [128, S/16]
        idx_all.append(np.ascontiguousarray(idx16))
        degb = np.zeros(cfg.bp, dtype=np.float32)
        degb[:b] = deg[c * b : (c + 1) * b]
        degt_all.append(np.ascontiguousarray(degb.reshape(t, P).T))  # [P, t]
    return idx_all, seg_all, degt_all


def build_program(cfg: Cfg):
    import concourse.bass as bass
    import concourse.tile as tile
    from concourse import bacc, mybir

    f32 = mybir.dt.float32
    bf16 = mybir.dt.bfloat16
    n, m, b, T, bp = cfg.n, cfg.m, cfg.b, cfg.t, cfg.bp
    CIN, HID, OUT, CW = cfg.c_in, cfg.c_hid, cfg.c_out, cfg.cw
    GCH, OB, PB = cfg.gch, cfg.ob, cfg.pb
    NCH, NB = cfg.nch, cfg.nbuck
    NSEG, CHB, CHN, SCHED = cfg.nseg, cfg.chb, cfg.chn, cfg.sched
    QT, QR, QS = cfg.qtiles, cfg.qrows, cfg.qstart
    NTAB = m * bp

    nc = bacc.Bacc(
        "TRN2",
        target_bir_lowering=False,
        debug=False,
        num_devices=m,
        num_swdge_queues=4,
    )

    # ---- I/O ----
    x_t = nc.dram_tensor("x_blk", [bp, CIN], f32, kind="ExternalInput")
    w1_t = nc.dram_tensor("W1", [4, CIN, HID], f32, kind="ExternalInput")
    b1_t = nc.dram_tensor("b1", [HID], f32, kind="ExternalInput")
    w2_t = nc.dram_tensor("W2", [4, HID, OUT], f32, kind="ExternalInput")
    b2_t = nc.dram_tensor("b2", [OUT], f32, kind="ExternalInput")
    degt_t = nc.dram_tensor("deg_t", [P, T], f32, kind="ExternalInput")
    idx_t = nc.dram_tensor(
        "idx", [P, NCH * 8], mybir.dt.int16, kind="ExternalInput"
    )
    seg_t = nc.dram_tensor("seg", [P, NSEG], mybir.dt.bfloat16, kind="ExternalInput")
    y_t = nc.dram_tensor("y_blk", [bp, OUT], f32, kind="ExternalOutput")

    # ---- internal DRAM ----
    u1 = nc.dram_tensor("U1", [3, bp, HID], f32)
    u2 = nc.dram_tensor("U2", [3, bp, OUT], f32)
    # bf16 gather rows must be 256B -> 128 bf16 wide; only cols :CW are real.
    CWB = 128
    blks, tabs = {}, {}
    for l in (1, 2):
        for j in (3, 2, 1):
            blks[(l, j)] = nc.dram_tensor(f"blk_{l}_{j}", [bp, CWB], bf16)
            tabs[(l, j)] = nc.dram_tensor(f"tab_{l}_{j}", [NTAB, CWB], bf16)

    import ml_dtypes

    iota_np = np.broadcast_to(
        np.tile(np.arange(P, dtype=np.float32), OB), (P, OB * P)
    ).astype(ml_dtypes.bfloat16)
    iota_d = nc.inline_tensor(iota_np, name="iota_rep")
    ident_d = nc.inline_tensor(np.eye(P, dtype=np.float32), name="ident")

    with ExitStack() as ctx:
        tc = ctx.enter_context(tile.TileContext(nc, num_cores=m))
        const = ctx.enter_context(tc.tile_pool(name="const", bufs=1))
        xp = ctx.enter_context(tc.tile_pool(name="xp", bufs=3))
        wp = ctx.enter_context(tc.tile_pool(name="wp", bufs=3))
        up = ctx.enter_context(tc.tile_pool(name="up", bufs=3))
        gp = ctx.enter_context(tc.tile_pool(name="gp", bufs=20))
        ip = ctx.enter_context(tc.tile_pool(name="ip", bufs=20))
        op = ctx.enter_context(tc.tile_pool(name="op", bufs=3))
        accp = ctx.enter_context(tc.tile_pool(name="accp", bufs=2))
        pst = ctx.enter_context(tc.tile_pool(name="pst", bufs=2, space="PSUM"))
        psu = ctx.enter_context(tc.tile_pool(name="psu", bufs=2, space="PSUM"))
        psa = ctx.enter_context(tc.tile_pool(name="psa", bufs=4, space="PSUM"))

        # ---- constants ----
        iota_s = const.tile([P, OB * P], bf16)
        nc.sync.dma_start(iota_s[:], iota_d[:, :])
        ident_s = const.tile([P, P], f32)
        nc.sync.dma_start(ident_s[:], ident_d[:, :])

        seg_s = const.tile([P, NSEG], bf16)
        nc.sync.dma_start(seg_s[:], seg_t[:, :])

        # V1cat [CIN, 4, HID], V2cat [HID, 4, OUT]
        w1s = const.tile([CIN, 4, HID], f32)
        nc.sync.dma_start(w1s[:], w1_t[:, :, :].rearrange("k p c -> p k c"))
        v1 = const.tile([CIN, 4, HID], f32)
        nc.vector.tensor_sub(v1[:, 0, :], w1s[:, 0, :], w1s[:, 2, :])
        nc.vector.tensor_scalar(
            out=v1[:, 1, :], in0=w1s[:, 3, :], scalar1=-3.0, scalar2=None,
            op0=mybir.AluOpType.mult,
        )
        nc.vector.tensor_add(v1[:, 1, :], v1[:, 1, :], w1s[:, 1, :])
        nc.vector.tensor_scalar(
            out=v1[:, 2, :], in0=w1s[:, 2, :], scalar1=2.0, scalar2=None,
            op0=mybir.AluOpType.mult,
        )
        nc.vector.tensor_scalar(
            out=v1[:, 3, :], in0=w1s[:, 3, :], scalar1=4.0, scalar2=None,
            op0=mybir.AluOpType.mult,
        )

        w2s = const.tile([HID, 4, OUT], f32)
        nc.sync.dma_start(w2s[:], w2_t[:, :, :].rearrange("k p c -> p k c"))
        v2 = const.tile([HID, 4, OUT], f32)
        nc.vector.tensor_sub(v2[:, 0, :], w2s[:, 0, :], w2s[:, 2, :])
        nc.vector.tensor_scalar(
            out=v2[:, 1, :], in0=w2s[:, 3, :], scalar1=-3.0, scalar2=None,
            op0=mybir.AluOpType.mult,
        )
        nc.vector.tensor_add(v2[:, 1, :], v2[:, 1, :], w2s[:, 1, :])
        nc.vector.tensor_scalar(
            out=v2[:, 2, :], in0=w2s[:, 2, :], scalar1=2.0, scalar2=None,
            op0=mybir.AluOpType.mult,
        )
        nc.vector.tensor_scalar(
            out=v2[:, 3, :], in0=w2s[:, 3, :], scalar1=4.0, scalar2=None,
            op0=mybir.AluOpType.mult,
        )

        b1s = const.tile([P, HID], f32)
        nc.sync.dma_start(b1s[:1, :], b1_t[:].rearrange("(o c) -> o c", o=1))
        nc.gpsimd.partition_broadcast(b1s[:, :], b1s[:1, :])
        b2s = const.tile([P, OUT], f32)
        nc.sync.dma_start(b2s[:1, :], b2_t[:].rearrange("(o c) -> o c", o=1))
        nc.gpsimd.partition_broadcast(b2s[:, :], b2s[:1, :])

        # DMA-completion semaphores for prepare_only gathers (one per queue)
        dma_sems = [nc.alloc_semaphore(f"swdge_dma{q}") for q in range(4)]

        # ---- dis, -dis, -dis^2 in [P, T] (partition p, slot t) = dest 128t+p ----
        degs = const.tile([P, T], f32)
        nc.sync.dma_start(degs[:], degt_t[:, :])
        dis = const.tile([P, T], f32)
        ndis = const.tile([P, T], f32)
        ndis2 = const.tile([P, T], f32)
        tmp = const.tile([P, T], f32)
        nc.vector.tensor_scalar(
            out=tmp[:], in0=degs[:], scalar1=1.0, scalar2=None,
            op0=mybir.AluOpType.max,
        )
        nc.scalar.activation(tmp[:], tmp[:], mybir.ActivationFunctionType.Sqrt)
        nc.vector.reciprocal(dis[:], tmp[:])
        nc.vector.tensor_scalar(
            out=tmp[:], in0=degs[:], scalar1=0.0, scalar2=None,
            op0=mybir.AluOpType.is_gt,
        )
        nc.vector.tensor_mul(dis[:], dis[:], tmp[:])
        nc.vector.tensor_scalar(
            out=ndis[:], in0=dis[:], scalar1=-1.0, scalar2=None,
            op0=mybir.AluOpType.mult,
        )
        nc.vector.tensor_mul(ndis2[:], dis[:], ndis[:])

        # ---- prologue: U_j = src @ V_j ; U1/U2 to HBM (j=1,2 scaled by dis);
        #      j=3 scaled -> blk  ----
        def prologue(src, cl, w, vcat, u_hbm, blk_hbm, from_sbuf=False):
            for k in range(T):
                if from_sbuf:
                    xc = src[:, k, :]
                else:
                    xc = xp.tile([P, cl], f32, tag="xc")
                    nc.sync.dma_start(xc[:], src[k * P : (k + 1) * P, :])
                tp = pst.tile([cl, P], f32, space="PSUM", tag="tp")
                nc.tensor.transpose(out=tp[:, :], in_=xc[:, :], identity=ident_s[:])
                xT = wp.tile([cl, P], f32, tag="xT")
                nc.vector.tensor_copy(xT[:], tp[:, :])
                upsum = psu.tile([P, 4, w], f32, space="PSUM", tag="upsum")
                nc.tensor.matmul(
                    out=upsum[:].rearrange("p a c -> p (a c)"),
                    lhsT=xT[:, :],
                    rhs=vcat[:].rearrange("p a c -> p (a c)"),
                    start=True, stop=True,
                )
                ut = wp.tile([P, 3, w], f32, tag="ut")
                # fold bias into U_0 once (y = U0 + b - dis*(A S1))
                bb = b1s if w == HID else b2s
                nc.vector.tensor_add(ut[:, 0, :], upsum[:, 0, :], bb[:, :w])
                for j in (1, 2):
                    nc.vector.tensor_scalar(
                        out=ut[:, j, :], in0=upsum[:, j, :],
                        scalar1=dis[:, k : k + 1], scalar2=None,
                        op0=mybir.AluOpType.mult,
                    )
                nc.sync.dma_start(
                    u_hbm[:, k * P : (k + 1) * P, :].rearrange("a p c -> p a c"),
                    ut[:],
                )
                bt = wp.tile([P, w], bf16, tag="bt")
                nc.vector.tensor_scalar(
                    out=bt[:], in0=upsum[:, 3, :],
                    scalar1=dis[:, k : k + 1], scalar2=None,
                    op0=mybir.AluOpType.mult,
                )
                nc.sync.dma_start(blk_hbm[k * P : (k + 1) * P, :w], bt[:])

        # ---- one hop: table = AG(blk_in) split per bucket (overlaps the
        # gather passes); acc = scale*(A_gather) + U_j.  Edge stream is
        # (bucket, tile)-major with exact-max cells: chunks may straddle
        # tile boundaries, one matmul segment per (tile, chunk).  On the
        # last bucket pass, finished acc quarters are cast+written to
        # blk_out so the next hop's AllGather pieces can start early.
        def hop(l, j, blk_in, table, w, blk_out):
            for q in range(NB):
                nc.gpsimd.collective_compute(
                    "AllGather",
                    mybir.AluOpType.bypass,
                    replica_groups=[list(range(m))],
                    ins=[blk_in[QS[q] : QS[q] + QR[q], :].opt()],
                    outs=[table[8 * QS[q] : 8 * (QS[q] + QR[q]), :].opt()],
                )
            final = j == 0
            u_hbm = u1 if l == 1 else u2
            acc = accp.tile([P, T, w], f32, tag="acc")

            gath = None
            oneh = None
            o = 0           # segment ordinal (one-hot column)
            gcall = 0       # gather-call counter (round-robin SWDGE queues:
                            # each queue runs on its own Q7 cpu pair)
            # pass order: bucket-major (all of bucket bk across every tile
            # group) so CC AllGathers land in the gather lulls at hop
            # boundaries; quarter g of acc is written out as soon as its
            # bucket-3 batches finish.
            for bk in range(NB):
                gt0 = 0
                for g in range(NB):
                    gtn = QT[g]
                    tab_lo = 8 * QS[bk]
                    tab_hi = 8 * (QS[bk] + QR[bk])
                    cb = CHB[(g, bk)]
                    nloc = CHN[(g, bk)]
                    sched_p = SCHED[(g, bk)]
                    gnext = 0   # next ungathered pass-local chunk
                    gbase = 0   # current gather batch start (pass-local)
                    nbatch = (gtn + PB - 1) // PB
                    for bi in range(nbatch):
                        t0 = gt0 + bi * PB
                        tcnt = min(PB, gt0 + gtn - t0)
                        psum = psa.tile(
                            [P, PB, w], f32, space="PSUM", tag="apsum"
                        )
                        for tt in range(tcnt):
                            ti = t0 + tt
                            rng = sched_p[ti]
                            if rng is None:
                                nc.vector.memset(psum[:, tt, :], 0.0)
                                continue
                            c0l, c1l = rng
                            for k, cl in enumerate(range(c0l, c1l + 1)):
                                if cl >= gnext:
                                    glen = min(GCH, nloc - gnext)
                                    gbase = gnext
                                    cg = cb + gnext
                                    idxt = ip.tile(
                                        [P, GCH * 8], mybir.dt.int16,
                                        tag="idxt",
                                    )
                                    nc.sync.dma_start(
                                        idxt[:, : glen * 8],
                                        idx_t[:, cg * 8 : (cg + glen) * 8],
                                    )
                                    gath = gp.tile(
                                        [P, GCH, CWB], bf16, tag="gath"
                                    )
                                    nc.gpsimd.dma_gather(
                                        out_ap=gath[:, :glen, :],
                                        in_ap=table[tab_lo:tab_hi, :],
                                        idxs_ap=idxt[:, : glen * 8],
                                        num_idxs=glen * P,
                                        num_idxs_reg=glen * P,
                                        elem_size=CWB,
                                        queue_num=gcall % 4,
                                    )
                                    gcall += 1
                                    gnext += glen
                                gpos = cl - gbase
                                oc, opos = divmod(o, OB)
                                if opos == 0:
                                    olen = min(OB, NSEG - oc * OB)
                                    oneh = op.tile(
                                        [P, OB, P], bf16, tag="oneh"
                                    )
                                    nc.vector.tensor_tensor(
                                        out=oneh[:, :olen, :],
                                        in0=iota_s[:].rearrange(
                                            "p (a q) -> p a q", q=P
                                        )[:, :olen, :],
                                        in1=seg_s[:, oc * OB : oc * OB + olen]
                                        .to_broadcast([P, olen, P]),
                                        op=mybir.AluOpType.is_equal,
                                    )
                                nc.tensor.matmul(
                                    out=psum[:, tt, :],
                                    lhsT=oneh[:, opos, :],
                                    rhs=gath[:, gpos, :w],
                                    start=(k == 0),
                                    stop=(k == c1l - c0l),
                                )
                                o += 1
                        # fold this pass's aggregation into acc
                        sl = slice(t0, t0 + tcnt)
                        if bk == 0:
                            nc.vector.tensor_copy(
                                acc[:, sl, :], psum[:, :tcnt, :]
                            )
                        else:
                            nc.vector.tensor_add(
                                acc[:, sl, :], acc[:, sl, :],
                                psum[:, :tcnt, :],
                            )
                        if bk == NB - 1:
                            scale = ndis if final else ndis2
                            nc.vector.tensor_tensor(
                                out=acc[:, sl, :],
                                in0=acc[:, sl, :],
                                in1=scale[:, sl].to_broadcast([P, tcnt, w]),
                                op=mybir.AluOpType.mult,
                            )
                            uti = up.tile([P, PB, w], f32, tag="uti")
                            nc.sync.dma_start(
                                uti[:, :tcnt, :],
                                u_hbm[
                                    j if not final else 0,
                                    t0 * P : (t0 + tcnt) * P, :,
                                ].rearrange("(a p) c -> p a c", p=P),
                            )
                            nc.vector.tensor_add(
                                acc[:, sl, :], acc[:, sl, :],
                                uti[:, :tcnt, :],
                            )
                            if final and l == 1:
                                nc.scalar.activation(
                                    acc[:, sl, :],
                                    acc[:, sl, :],
                                    mybir.ActivationFunctionType.Relu,
                                )
                    # acc quarter g final on the last bucket pass: cast +
                    # write to blk so the next hop's AG piece g starts early
                    if bk == NB - 1 and blk_out is not None:
                        ab = wp.tile([P, gtn, w], bf16, tag="accb")
                        nc.vector.tensor_copy(
                            ab[:], acc[:, gt0 : gt0 + gtn, :]
                        )
                        nc.sync.dma_start(
                            blk_out[QS[g] : QS[g] + QR[g], :w].rearrange(
                                "(a p) c -> p a c", p=P
                            ),
                            ab[:],
                        )
                    gt0 += gtn
            return acc

        def acc_to_hbm(acc, dst, w):
            nc.sync.dma_start(
                dst[:, :].rearrange("(a p) c -> p a c", p=P), acc[:, :, :w]
            )

        # ================= layer 1 =================
        prologue(x_t, CIN, HID, v1, u1, blks[(1, 3)])
        acc = hop(1, 2, blks[(1, 3)], tabs[(1, 3)], HID, blks[(1, 2)])
        acc = hop(1, 1, blks[(1, 2)], tabs[(1, 2)], HID, blks[(1, 1)])
        acc = hop(1, 0, blks[(1, 1)], tabs[(1, 1)], HID, None)

        # ================= layer 2 (prologue reads h=acc from SBUF) ======
        prologue(acc, HID, OUT, v2, u2, blks[(2, 3)], from_sbuf=True)
        acc = hop(2, 2, blks[(2, 3)], tabs[(2, 3)], OUT, blks[(2, 2)])
        acc = hop(2, 1, blks[(2, 2)], tabs[(2, 2)], OUT, blks[(2, 1)])
        acc = hop(2, 0, blks[(2, 1)], tabs[(2, 1)], OUT, None)

        # ---- log_softmax over acc [P, T, OUT] ----
        red = wp.tile([P, T], f32, tag="red")
        nc.vector.tensor_reduce(
            out=red[:], in_=acc[:, :, :], axis=mybir.AxisListType.X,
            op=mybir.AluOpType.max,
        )
        xm = accp.tile([P, T, OUT], f32, tag="xm")
        nc.vector.tensor_tensor(
            out=xm[:], in0=acc[:, :, :], in1=red[:].to_broadcast([P, T, OUT]),
            op=mybir.AluOpType.subtract,
        )
        ex = accp.tile([P, T, OUT], f32, tag="ex")
        nc.scalar.activation(ex[:], xm[:], mybir.ActivationFunctionType.Exp)
        nc.vector.tensor_reduce(
            out=red[:], in_=ex[:, :, :], axis=mybir.AxisListType.X,
            op=mybir.AluOpType.add,
        )
        nc.scalar.activation(red[:], red[:], mybir.ActivationFunctionType.Ln)
        nc.vector.tensor_tensor(
            out=xm[:], in0=xm[:, :, :], in1=red[:].to_broadcast([P, T, OUT]),
            op=mybir.AluOpType.subtract,
        )
        nc.sync.dma_start(
            y_t[:, :].rearrange("(a p) c -> p a c", p=P), xm[:, :, :]
        )

    nc.compile()
    return nc


def make_in_maps(cfg: Cfg, inputs: dict, idx_all, seg_all, degt_all):
    x = np.asarray(inputs["x"], dtype=np.float32)
    maps = []
    for c in range(cfg.m):
        xb = np.zeros((cfg.bp, cfg.c_in), dtype=np.float32)
        xb[: cfg.b] = x[c * cfg.b : (c + 1) * cfg.b]
        maps.append(
            {
                "x_blk": xb,
                "W1": np.asarray(inputs["W1"], dtype=np.float32),
                "b1": np.asarray(inputs["b1"], dtype=np.float32),
                "W2": np.asarray(inputs["W2"], dtype=np.float32),
                "b2": np.asarray(inputs["b2"], dtype=np.float32),
                "deg_t": degt_all[c],
                "idx": idx_all[c],
                "seg": seg_all[c],
            }
        )
    return maps


def kernel(**inputs) -> np.ndarray:
    from concourse import bass_utils

    cfg = Cfg(n=100000, m=8)
    cfg.finish()
    edge_index = np.asarray(inputs["edge_index"])
    idx_all, seg_all, degt_all = preprocess(edge_index, cfg)
    nc = build_program(cfg)
    in_maps = make_in_maps(cfg, inputs, idx_all, seg_all, degt_all)
    res = bass_utils.run_bass_kernel_spmd(nc, in_maps, core_ids=list(range(cfg.m)))
    out = np.concatenate(
        [res.results[c]["y_blk"][: cfg.b] for c in range(cfg.m)], axis=0
    )
    return out.astype(np.float32)



# revision 48
# speedup vs baseline: 1.2371x; 1.1656x over previous
"""ChebNet (K=4, two layers, log_softmax) on 8 Trainium2 NeuronCores.

Self-contained: takes FULL inputs, shards by destination node internally,
runs a single SPMD Bass kernel on cores 0-7 (with ncfw AllGather between
propagation hops), returns the FULL [N, 32] output.

Math (Horner re-association so propagations happen at output width):
    y = sum_k T_k(L) x @ W[k] + b,  T_k Chebyshev,  L = -D^-1/2 A D^-1/2
      = U0 + L(U1 + L(U2 + L U3)),  U_j = x @ V_j
    V0 = W0 - W2, V1 = W1 - 3 W3, V2 = 2 W2, V3 = 4 W3
    L S = -dis * A (dis * S)   with dis = where(deg>0, 1/sqrt(max(deg,1)), 0)
Scaled-space recurrence (tables hold S_hat = dis * S):
    S_hat3 = dis*U3;  S_hat_{j} = dis*U_j - dis^2 * (A S_hat_{j+1})
    final: y = U_0 - dis * (A S_hat_1) + b
Per hop, A S_hat is an edge-gather + segment-sum done as:
    indirect-DMA row gather (dest-sorted edge stream) ->
    one-hot (seg == iota) built on VectorE ->
    TensorE matmul accumulation into PSUM per 128-dest tile.
"""

import sys

if "/opt/trn_rl_repo" not in sys.path:
    sys.path.insert(0, "/opt/trn_rl_repo")

import math
from contextlib import ExitStack
from dataclasses import dataclass, field

import numpy as np

P = 128
PADIDX = 1 << 20


@dataclass
class Cfg:
    n: int            # nodes
    m: int            # cores
    c_in: int = 128
    c_hid: int = 64
    c_out: int = 32
    cw: int = 64      # gather-table row width (f32) for both layers
    gch: int = 8      # chunks per gather call (>8 overflows the SWDGE
                      # descriptor ring and hangs the device)
    ob: int = 32      # one-hot columns per build batch
    pb: int = 8       # psum tiles per writeout batch
    # filled by finish/preprocess
    b: int = 0        # dests per core
    t: int = 0        # 128-dest tiles per core
    bp: int = 0       # padded block rows = t*128
    nbuck: int = 4    # source buckets = node-block quarters
    qtiles: list = field(default_factory=list)  # tiles per quarter
    qrows: list = field(default_factory=list)   # rows per quarter (tile-aligned)
    qstart: list = field(default_factory=list)  # local row start per quarter
    nch: int = 0      # total chunks in edge stream
    nseg: int = 0     # total (tile,chunk) matmul segments
    chb: list = field(default_factory=list)     # global chunk base per bucket
    chn: list = field(default_factory=list)     # chunks per bucket
    sched: list = field(default_factory=list)   # [nb][t] = (c0,c1) local or None

    def finish(self):
        assert self.n % self.m == 0
        self.b = self.n // self.m
        self.t = (self.b + P - 1) // P
        self.bp = self.t * P
        # tile-aligned quarters of the node block; bucket q = all cores'
        # quarter-q rows (int16 gather range: 8*qrows <= 32767)
        tq = (self.t + 3) // 4
        self.qtiles = [tq, tq, tq, self.t - 3 * tq]
        self.qrows = [q * P for q in self.qtiles]
        self.qstart = [0]
        for q in self.qrows[:-1]:
            self.qstart.append(self.qstart[-1] + q)
        assert all(8 * q <= 32767 for q in self.qrows)


def preprocess(edge_index: np.ndarray, cfg: Cfg):
    """Shard edges by destination row-block; per core build a
    (bucket, tile)-major edge stream.  Cells (bucket x dest-tile) are
    padded to the cross-core max count (NOT 128-aligned), so the chunk
    schedule is shared across cores while chunks may straddle dest-tile
    boundaries; straddled chunks get one matmul segment per tile.
    Buckets are node-block quarters interleaved across cores so each
    bucket's table rows come from one AllGather piece."""
    import ml_dtypes

    row = np.asarray(edge_index[0], dtype=np.int64)
    col = np.asarray(edge_index[1], dtype=np.int64)
    n, m, b, t = cfg.n, cfg.m, cfg.b, cfg.t
    nb = cfg.nbuck
    qstart = np.array(cfg.qstart + [cfg.bp], dtype=np.int64)
    qrows = np.array(cfg.qrows, dtype=np.int64)

    deg = np.bincount(row, minlength=n).astype(np.float32)
    gt0 = np.concatenate([[0], np.cumsum(cfg.qtiles)]).astype(np.int64)

    # --- balance permutation: per core, permute dests WITHIN their node
    # quarter so each (src-quarter, dest-tile) cell count flattens to the
    # mean; the cross-core max then carries far less padding.  A node's
    # quarter is invariant, so other cores' source-row quarters are
    # unaffected (no cross-core coupling).
    raw = []
    pos_of = []
    for c in range(m):
        sel = (row >= c * b) & (row < (c + 1) * b)
        d = (row[sel] - c * b).astype(np.int64)
        s = col[sel].astype(np.int64)
        raw.append((d, s))
        srcq = np.searchsorted(qstart, s % b, side="right") - 1
        v = np.zeros((cfg.bp, nb), dtype=np.int64)
        np.add.at(v, (d, srcq), 1)
        pos = np.empty(cfg.bp, dtype=np.int64)
        for g in range(nb):
            lo = int(qstart[g])
            hi = min(int(qstart[g + 1]), b)
            nt = cfg.qtiles[g]
            cap = 128
            dq = np.arange(lo, int(qstart[g + 1]))      # slots incl pad
            real = dq[dq < hi]
            vq = v[real]                                 # [nreal, nb]
            orderq = np.argsort(-vq.sum(1), kind="stable")
            load = np.zeros((nt, nb), dtype=np.int64)
            cnt = np.zeros(nt, dtype=np.int64)
            slot_in_tile = np.zeros(len(real), dtype=np.int64)
            tile_pick = np.zeros(len(real), dtype=np.int64)
            for di in orderq:
                sc_ = np.where(
                    cnt < cap, (load + vq[di]).max(1), 1 << 40
                )
                tt = int(np.argmin(sc_))
                tile_pick[di] = tt
                slot_in_tile[di] = cnt[tt]
                load[tt] += vq[di]
                cnt[tt] += 1
            # pad dests fill remaining slots (any order)
            pos[real] = lo + tile_pick * cap + slot_in_tile
        pos_of.append(pos)
    cfg.pos_of = pos_of

    per_core = []
    counts = np.zeros((m, nb, t), dtype=np.int64)
    for c in range(m):
        d_orig, s = raw[c]
        d = pos_of[c][d_orig]
        sc, lr = s // b, s % b
        # source permuted position within its owner core
        spos = np.stack(pos_of)[sc, lr]
        q = np.searchsorted(qstart, spos, side="right") - 1
        lidx = sc * qrows[q] + (spos - qstart[q])   # int16-safe bucket row
        tile_id = d >> 7
        key = q * t + tile_id
        order = np.argsort(key, kind="stable")
        d, lidx, key = d[order], lidx[order], key[order]
        np.add.at(counts[c], (q[order], tile_id[order]), 1)
        per_core.append((d, lidx, key))

    M = counts.max(axis=0)                        # [nb, t] cell sizes
    # stream layout: (tile-group, bucket)-major passes; cells back to
    # back within a pass, pass tail 128-padded.  Group g = the dest
    # tiles of node-block quarter g, so acc quarter g is final after
    # its 4 bucket passes (early blk writeout -> early next-hop AG).
    gt0 = np.concatenate([[0], np.cumsum(cfg.qtiles)]).astype(np.int64)
    cell_start = np.zeros((nb, t), dtype=np.int64)
    cfg.chb, cfg.chn, cfg.sched = {}, {}, {}
    pass_slot_base = {}
    base = 0
    for g in range(nb):
        for bk in range(nb):
            pos = 0
            sched_p = {}
            for ti in range(int(gt0[g]), int(gt0[g + 1])):
                cell_start[bk, ti] = pos
                if M[bk, ti] == 0:
                    sched_p[ti] = None
                    continue
                c0, c1 = pos // P, (pos + M[bk, ti] - 1) // P
                sched_p[ti] = (int(c0), int(c1))
                pos += M[bk, ti]
            nloc = (pos + P - 1) // P
            cfg.chb[(g, bk)] = base // P
            cfg.chn[(g, bk)] = int(nloc)
            cfg.sched[(g, bk)] = sched_p
            pass_slot_base[(g, bk)] = base
            base += nloc * P
    S = base
    cfg.nch = S // P
    # one-hot segment order must match kernel consumption: bucket-major
    nseg = 0
    seg_tile, seg_chunk = [], []                  # per segment (global chunk)
    for bk in range(nb):
        for g in range(nb):
            cb = cfg.chb[(g, bk)]
            for ti in range(int(gt0[g]), int(gt0[g + 1])):
                rng = cfg.sched[(g, bk)][ti]
                if rng is None:
                    continue
                c0, c1 = rng
                for cl in range(c0, c1 + 1):
                    seg_tile.append(ti)
                    seg_chunk.append(cb + cl)
                nseg += c1 - c0 + 1
    cfg.nseg = nseg
    seg_tile = np.array(seg_tile, dtype=np.int64)
    seg_chunk = np.array(seg_chunk, dtype=np.int64)

    pass_base = np.zeros((nb, t), dtype=np.int64)
    for g in range(nb):
        for bk in range(nb):
            pass_base[bk, int(gt0[g]) : int(gt0[g + 1])] = pass_slot_base[(g, bk)]

    idx_all, seg_all, degt_all = [], [], []
    for c in range(m):
        d, lidx, key = per_core[c]
        idx = np.zeros(S, dtype=np.int16)          # pad -> bucket row 0
        stile = np.full(S, -2, dtype=np.int64)     # slot -> dest tile
        sval = np.zeros(S, dtype=np.float32)       # slot -> dest & 127
        cnt_flat = counts[c].reshape(-1)
        pos_in_cell = np.arange(d.size) - np.concatenate(
            [[0], np.cumsum(cnt_flat)]
        )[key]
        slot = (
            pass_base.reshape(-1)[key]
            + cell_start.reshape(-1)[key]
            + pos_in_cell
        )
        idx[slot] = lidx.astype(np.int16)
        stile[slot] = key % t
        sval[slot] = (d & 127).astype(np.float32)
        # segment columns: chunk slots where the edge belongs to this
        # segment's tile; everything else -1
        ch_t = stile.reshape(cfg.nch, P)[seg_chunk]      # [nseg, P]
        ch_v = sval.reshape(cfg.nch, P)[seg_chunk]
        segm = np.where(ch_t == seg_tile[:, None], ch_v, -1.0)
        seg_all.append(
            np.ascontiguousarray(segm.T).astype(ml_dtypes.bfloat16)
        )
        # dma_gather idx layout: slot i -> partition i%16 col i//16,
        # replicated across the 8 groups of 16 partitions
        idx16 = idx.reshape(S // 16, 16).T       # [16, S/16]
        idx16 = np.tile(idx16, (8, 1))           # [128, S/16]
        idx_all.append(np.ascontiguousarray(idx16))
        degb = np.zeros(cfg.bp, dtype=np.float32)
        degb[pos_of[c][:b]] = deg[c * b : (c + 1) * b]
        degt_all.append(np.ascontiguousarray(degb.reshape(t, P).T))  # [P, t]
    return idx_all, seg_all, degt_all


def build_program(cfg: Cfg):
    import concourse.bass as bass
    import concourse.tile as tile
    from concourse import bacc, mybir

    f32 = mybir.dt.float32
    bf16 = mybir.dt.bfloat16
    n, m, b, T, bp = cfg.n, cfg.m, cfg.b, cfg.t, cfg.bp
    CIN, HID, OUT, CW = cfg.c_in, cfg.c_hid, cfg.c_out, cfg.cw
    GCH, OB, PB = cfg.gch, cfg.ob, cfg.pb
    NCH, NB = cfg.nch, cfg.nbuck
    NSEG, CHB, CHN, SCHED = cfg.nseg, cfg.chb, cfg.chn, cfg.sched
    QT, QR, QS = cfg.qtiles, cfg.qrows, cfg.qstart
    NTAB = m * bp

    nc = bacc.Bacc(
        "TRN2",
        target_bir_lowering=False,
        debug=False,
        num_devices=m,
        num_swdge_queues=4,
    )

    # ---- I/O ----
    x_t = nc.dram_tensor("x_blk", [bp, CIN], f32, kind="ExternalInput")
    w1_t = nc.dram_tensor("W1", [4, CIN, HID], f32, kind="ExternalInput")
    b1_t = nc.dram_tensor("b1", [HID], f32, kind="ExternalInput")
    w2_t = nc.dram_tensor("W2", [4, HID, OUT], f32, kind="ExternalInput")
    b2_t = nc.dram_tensor("b2", [OUT], f32, kind="ExternalInput")
    degt_t = nc.dram_tensor("deg_t", [P, T], f32, kind="ExternalInput")
    idx_t = nc.dram_tensor(
        "idx", [P, NCH * 8], mybir.dt.int16, kind="ExternalInput"
    )
    seg_t = nc.dram_tensor("seg", [P, NSEG], mybir.dt.bfloat16, kind="ExternalInput")
    y_t = nc.dram_tensor("y_blk", [bp, OUT], f32, kind="ExternalOutput")

    # ---- internal DRAM ----
    u1 = nc.dram_tensor("U1", [3, bp, HID], f32)
    u2 = nc.dram_tensor("U2", [3, bp, OUT], f32)
    # bf16 gather rows must be 256B -> 128 bf16 wide; only cols :CW are real.
    CWB = 128
    blks, tabs = {}, {}
    for l in (1, 2):
        for j in (3, 2, 1):
            blks[(l, j)] = nc.dram_tensor(f"blk_{l}_{j}", [bp, CWB], bf16)
            tabs[(l, j)] = nc.dram_tensor(f"tab_{l}_{j}", [NTAB, CWB], bf16)

    import ml_dtypes

    iota_np = np.broadcast_to(
        np.tile(np.arange(P, dtype=np.float32), OB), (P, OB * P)
    ).astype(ml_dtypes.bfloat16)
    iota_d = nc.inline_tensor(iota_np, name="iota_rep")
    ident_d = nc.inline_tensor(np.eye(P, dtype=np.float32), name="ident")

    with ExitStack() as ctx:
        tc = ctx.enter_context(tile.TileContext(nc, num_cores=m))
        const = ctx.enter_context(tc.tile_pool(name="const", bufs=1))
        xp = ctx.enter_context(tc.tile_pool(name="xp", bufs=3))
        wp = ctx.enter_context(tc.tile_pool(name="wp", bufs=3))
        up = ctx.enter_context(tc.tile_pool(name="up", bufs=3))
        gp = ctx.enter_context(tc.tile_pool(name="gp", bufs=20))
        ip = ctx.enter_context(tc.tile_pool(name="ip", bufs=20))
        op = ctx.enter_context(tc.tile_pool(name="op", bufs=3))
        accp = ctx.enter_context(tc.tile_pool(name="accp", bufs=2))
        pst = ctx.enter_context(tc.tile_pool(name="pst", bufs=2, space="PSUM"))
        psu = ctx.enter_context(tc.tile_pool(name="psu", bufs=2, space="PSUM"))
        psa = ctx.enter_context(tc.tile_pool(name="psa", bufs=4, space="PSUM"))

        # ---- constants ----
        iota_s = const.tile([P, OB * P], bf16)
        nc.sync.dma_start(iota_s[:], iota_d[:, :])
        ident_s = const.tile([P, P], f32)
        nc.sync.dma_start(ident_s[:], ident_d[:, :])

        seg_s = const.tile([P, NSEG], bf16)
        nc.sync.dma_start(seg_s[:], seg_t[:, :])

        # V1cat [CIN, 4, HID], V2cat [HID, 4, OUT]
        w1s = const.tile([CIN, 4, HID], f32)
        nc.sync.dma_start(w1s[:], w1_t[:, :, :].rearrange("k p c -> p k c"))
        v1 = const.tile([CIN, 4, HID], f32)
        nc.vector.tensor_sub(v1[:, 0, :], w1s[:, 0, :], w1s[:, 2, :])
        nc.vector.tensor_scalar(
            out=v1[:, 1, :], in0=w1s[:, 3, :], scalar1=-3.0, scalar2=None,
            op0=mybir.AluOpType.mult,
        )
        nc.vector.tensor_add(v1[:, 1, :], v1[:, 1, :], w1s[:, 1, :])
        nc.vector.tensor_scalar(
            out=v1[:, 2, :], in0=w1s[:, 2, :], scalar1=2.0, scalar2=None,
            op0=mybir.AluOpType.mult,
        )
        nc.vector.tensor_scalar(
            out=v1[:, 3, :], in0=w1s[:, 3, :], scalar1=4.0, scalar2=None,
            op0=mybir.AluOpType.mult,
        )

        w2s = const.tile([HID, 4, OUT], f32)
        nc.sync.dma_start(w2s[:], w2_t[:, :, :].rearrange("k p c -> p k c"))
        v2 = const.tile([HID, 4, OUT], f32)
        nc.vector.tensor_sub(v2[:, 0, :], w2s[:, 0, :], w2s[:, 2, :])
        nc.vector.tensor_scalar(
            out=v2[:, 1, :], in0=w2s[:, 3, :], scalar1=-3.0, scalar2=None,
            op0=mybir.AluOpType.mult,
        )
        nc.vector.tensor_add(v2[:, 1, :], v2[:, 1, :], w2s[:, 1, :])
        nc.vector.tensor_scalar(
            out=v2[:, 2, :], in0=w2s[:, 2, :], scalar1=2.0, scalar2=None,
            op0=mybir.AluOpType.mult,
        )
        nc.vector.tensor_scalar(
            out=v2[:, 3, :], in0=w2s[:, 3, :], scalar1=4.0, scalar2=None,
            op0=mybir.AluOpType.mult,
        )

        b1s = const.tile([P, HID], f32)
        nc.sync.dma_start(b1s[:1, :], b1_t[:].rearrange("(o c) -> o c", o=1))
        nc.gpsimd.partition_broadcast(b1s[:, :], b1s[:1, :])
        b2s = const.tile([P, OUT], f32)
        nc.sync.dma_start(b2s[:1, :], b2_t[:].rearrange("(o c) -> o c", o=1))
        nc.gpsimd.partition_broadcast(b2s[:, :], b2s[:1, :])

        # DMA-completion semaphores for prepare_only gathers (one per queue)
        dma_sems = [nc.alloc_semaphore(f"swdge_dma{q}") for q in range(4)]

        # ---- dis, -dis, -dis^2 in [P, T] (partition p, slot t) = dest 128t+p ----
        degs = const.tile([P, T], f32)
        nc.sync.dma_start(degs[:], degt_t[:, :])
        dis = const.tile([P, T], f32)
        ndis = const.tile([P, T], f32)
        ndis2 = const.tile([P, T], f32)
        tmp = const.tile([P, T], f32)
        nc.vector.tensor_scalar(
            out=tmp[:], in0=degs[:], scalar1=1.0, scalar2=None,
            op0=mybir.AluOpType.max,
        )
        nc.scalar.activation(tmp[:], tmp[:], mybir.ActivationFunctionType.Sqrt)
        nc.vector.reciprocal(dis[:], tmp[:])
        nc.vector.tensor_scalar(
            out=tmp[:], in0=degs[:], scalar1=0.0, scalar2=None,
            op0=mybir.AluOpType.is_gt,
        )
        nc.vector.tensor_mul(dis[:], dis[:], tmp[:])
        nc.vector.tensor_scalar(
            out=ndis[:], in0=dis[:], scalar1=-1.0, scalar2=None,
            op0=mybir.AluOpType.mult,
        )
        nc.vector.tensor_mul(ndis2[:], dis[:], ndis[:])

        # ---- prologue: U_j = src @ V_j ; U1/U2 to HBM (j=1,2 scaled by dis);
        #      j=3 scaled -> blk  ----
        def prologue(src, cl, w, vcat, u_hbm, blk_hbm, from_sbuf=False):
            for k in range(T):
                if from_sbuf:
                    xc = src[:, k, :]
                else:
                    xc = xp.tile([P, cl], f32, tag="xc")
                    nc.sync.dma_start(xc[:], src[k * P : (k + 1) * P, :])
                tp = pst.tile([cl, P], f32, space="PSUM", tag="tp")
                nc.tensor.transpose(out=tp[:, :], in_=xc[:, :], identity=ident_s[:])
                xT = wp.tile([cl, P], f32, tag="xT")
                nc.vector.tensor_copy(xT[:], tp[:, :])
                upsum = psu.tile([P, 4, w], f32, space="PSUM", tag="upsum")
                nc.tensor.matmul(
                    out=upsum[:].rearrange("p a c -> p (a c)"),
                    lhsT=xT[:, :],
                    rhs=vcat[:].rearrange("p a c -> p (a c)"),
                    start=True, stop=True,
                )
                ut = wp.tile([P, 3, w], f32, tag="ut")
                # fold bias into U_0 once (y = U0 + b - dis*(A S1))
                bb = b1s if w == HID else b2s
                nc.vector.tensor_add(ut[:, 0, :], upsum[:, 0, :], bb[:, :w])
                for j in (1, 2):
                    nc.vector.tensor_scalar(
                        out=ut[:, j, :], in0=upsum[:, j, :],
                        scalar1=dis[:, k : k + 1], scalar2=None,
                        op0=mybir.AluOpType.mult,
                    )
                nc.sync.dma_start(
                    u_hbm[:, k * P : (k + 1) * P, :].rearrange("a p c -> p a c"),
                    ut[:],
                )
                bt = wp.tile([P, w], bf16, tag="bt")
                nc.vector.tensor_scalar(
                    out=bt[:], in0=upsum[:, 3, :],
                    scalar1=dis[:, k : k + 1], scalar2=None,
                    op0=mybir.AluOpType.mult,
                )
                nc.sync.dma_start(blk_hbm[k * P : (k + 1) * P, :w], bt[:])

        # ---- one hop: table = AG(blk_in) split per bucket (overlaps the
        # gather passes); acc = scale*(A_gather) + U_j.  Edge stream is
        # (bucket, tile)-major with exact-max cells: chunks may straddle
        # tile boundaries, one matmul segment per (tile, chunk).  On the
        # last bucket pass, finished acc quarters are cast+written to
        # blk_out so the next hop's AllGather pieces can start early.
        def hop(l, j, blk_in, table, w, blk_out):
            for q in range(NB):
                nc.gpsimd.collective_compute(
                    "AllGather",
                    mybir.AluOpType.bypass,
                    replica_groups=[list(range(m))],
                    ins=[blk_in[QS[q] : QS[q] + QR[q], :].opt()],
                    outs=[table[8 * QS[q] : 8 * (QS[q] + QR[q]), :].opt()],
                )
            final = j == 0
            u_hbm = u1 if l == 1 else u2
            acc = accp.tile([P, T, w], f32, tag="acc")

            gath = None
            oneh = None
            o = 0           # segment ordinal (one-hot column)
            gcall = 0       # gather-call counter (round-robin SWDGE queues:
                            # each queue runs on its own Q7 cpu pair)
            # pass order: bucket-major (all of bucket bk across every tile
            # group) so CC AllGathers land in the gather lulls at hop
            # boundaries; quarter g of acc is written out as soon as its
            # bucket-3 batches finish.
            for bk in range(NB):
                gt0 = 0
                for g in range(NB):
                    gtn = QT[g]
                    tab_lo = 8 * QS[bk]
                    tab_hi = 8 * (QS[bk] + QR[bk])
                    cb = CHB[(g, bk)]
                    nloc = CHN[(g, bk)]
                    sched_p = SCHED[(g, bk)]
                    gnext = 0   # next ungathered pass-local chunk
                    gbase = 0   # current gather batch start (pass-local)
                    nbatch = (gtn + PB - 1) // PB
                    for bi in range(nbatch):
                        t0 = gt0 + bi * PB
                        tcnt = min(PB, gt0 + gtn - t0)
                        psum = psa.tile(
                            [P, PB, w], f32, space="PSUM", tag="apsum"
                        )
                        for tt in range(tcnt):
                            ti = t0 + tt
                            rng = sched_p[ti]
                            if rng is None:
                                nc.vector.memset(psum[:, tt, :], 0.0)
                                continue
                            c0l, c1l = rng
                            for k, cl in enumerate(range(c0l, c1l + 1)):
                                if cl >= gnext:
                                    glen = min(GCH, nloc - gnext)
                                    gbase = gnext
                                    cg = cb + gnext
                                    idxt = ip.tile(
                                        [P, GCH * 8], mybir.dt.int16,
                                        tag="idxt",
                                    )
                                    nc.sync.dma_start(
                                        idxt[:, : glen * 8],
                                        idx_t[:, cg * 8 : (cg + glen) * 8],
                                    )
                                    gath = gp.tile(
                                        [P, GCH, CWB], bf16, tag="gath"
                                    )
                                    nc.gpsimd.dma_gather(
                                        out_ap=gath[:, :glen, :],
                                        in_ap=table[tab_lo:tab_hi, :],
                                        idxs_ap=idxt[:, : glen * 8],
                                        num_idxs=glen * P,
                                        num_idxs_reg=glen * P,
                                        elem_size=CWB,
                                        queue_num=gcall % 4,
                                    )
                                    gcall += 1
                                    gnext += glen
                                gpos = cl - gbase
                                oc, opos = divmod(o, OB)
                                if opos == 0:
                                    olen = min(OB, NSEG - oc * OB)
                                    oneh = op.tile(
                                        [P, OB, P], bf16, tag="oneh"
                                    )
                                    nc.vector.tensor_tensor(
                                        out=oneh[:, :olen, :],
                                        in0=iota_s[:].rearrange(
                                            "p (a q) -> p a q", q=P
                                        )[:, :olen, :],
                                        in1=seg_s[:, oc * OB : oc * OB + olen]
                                        .to_broadcast([P, olen, P]),
                                        op=mybir.AluOpType.is_equal,
                                    )
                                nc.tensor.matmul(
                                    out=psum[:, tt, :],
                                    lhsT=oneh[:, opos, :],
                                    rhs=gath[:, gpos, :w],
                                    start=(k == 0),
                                    stop=(k == c1l - c0l),
                                )
                                o += 1
                        # fold this pass's aggregation into acc
                        sl = slice(t0, t0 + tcnt)
                        if bk == 0:
                            nc.vector.tensor_copy(
                                acc[:, sl, :], psum[:, :tcnt, :]
                            )
                        else:
                            nc.vector.tensor_add(
                                acc[:, sl, :], acc[:, sl, :],
                                psum[:, :tcnt, :],
                            )
                        if bk == NB - 1:
                            scale = ndis if final else ndis2
                            nc.vector.tensor_tensor(
                                out=acc[:, sl, :],
                                in0=acc[:, sl, :],
                                in1=scale[:, sl].to_broadcast([P, tcnt, w]),
                                op=mybir.AluOpType.mult,
                            )
                            uti = up.tile([P, PB, w], f32, tag="uti")
                            nc.sync.dma_start(
                                uti[:, :tcnt, :],
                                u_hbm[
                                    j if not final else 0,
                                    t0 * P : (t0 + tcnt) * P, :,
                                ].rearrange("(a p) c -> p a c", p=P),
                            )
                            nc.vector.tensor_add(
                                acc[:, sl, :], acc[:, sl, :],
                                uti[:, :tcnt, :],
                            )
                            if final and l == 1:
                                nc.scalar.activation(
                                    acc[:, sl, :],
                                    acc[:, sl, :],
                                    mybir.ActivationFunctionType.Relu,
                                )
                    # acc quarter g final on the last bucket pass: cast +
                    # write to blk so the next hop's AG piece g starts early
                    if bk == NB - 1 and blk_out is not None:
                        ab = wp.tile([P, gtn, w], bf16, tag="accb")
                        nc.vector.tensor_copy(
                            ab[:], acc[:, gt0 : gt0 + gtn, :]
                        )
                        nc.sync.dma_start(
                            blk_out[QS[g] : QS[g] + QR[g], :w].rearrange(
                                "(a p) c -> p a c", p=P
                            ),
                            ab[:],
                        )
                    gt0 += gtn
            return acc

        def acc_to_hbm(acc, dst, w):
            nc.sync.dma_start(
                dst[:, :].rearrange("(a p) c -> p a c", p=P), acc[:, :, :w]
            )

        # ================= layer 1 =================
        prologue(x_t, CIN, HID, v1, u1, blks[(1, 3)])
        acc = hop(1, 2, blks[(1, 3)], tabs[(1, 3)], HID, blks[(1, 2)])
        acc = hop(1, 1, blks[(1, 2)], tabs[(1, 2)], HID, blks[(1, 1)])
        acc = hop(1, 0, blks[(1, 1)], tabs[(1, 1)], HID, None)

        # ================= layer 2 (prologue reads h=acc from SBUF) ======
        prologue(acc, HID, OUT, v2, u2, blks[(2, 3)], from_sbuf=True)
        acc = hop(2, 2, blks[(2, 3)], tabs[(2, 3)], OUT, blks[(2, 2)])
        acc = hop(2, 1, blks[(2, 2)], tabs[(2, 2)], OUT, blks[(2, 1)])
        acc = hop(2, 0, blks[(2, 1)], tabs[(2, 1)], OUT, None)

        # ---- log_softmax over acc [P, T, OUT] ----
        red = wp.tile([P, T], f32, tag="red")
        nc.vector.tensor_reduce(
            out=red[:], in_=acc[:, :, :], axis=mybir.AxisListType.X,
            op=mybir.AluOpType.max,
        )
        xm = accp.tile([P, T, OUT], f32, tag="xm")
        nc.vector.tensor_tensor(
            out=xm[:], in0=acc[:, :, :], in1=red[:].to_broadcast([P, T, OUT]),
            op=mybir.AluOpType.subtract,
        )
        ex = accp.tile([P, T, OUT], f32, tag="ex")
        nc.scalar.activation(ex[:], xm[:], mybir.ActivationFunctionType.Exp)
        nc.vector.tensor_reduce(
            out=red[:], in_=ex[:, :, :], axis=mybir.AxisListType.X,
            op=mybir.AluOpType.add,
        )
        nc.scalar.activation(red[:], red[:], mybir.ActivationFunctionType.Ln)
        nc.vector.tensor_tensor(
            out=xm[:], in0=xm[:, :, :], in1=red[:].to_broadcast([P, T, OUT]),
            op=mybir.AluOpType.subtract,
        )
        nc.sync.dma_start(
            y_t[:, :].rearrange("(a p) c -> p a c", p=P), xm[:, :, :]
        )

    nc.compile()
    return nc


def make_in_maps(cfg: Cfg, inputs: dict, idx_all, seg_all, degt_all):
    x = np.asarray(inputs["x"], dtype=np.float32)
    maps = []
    for c in range(cfg.m):
        xb = np.zeros((cfg.bp, cfg.c_in), dtype=np.float32)
        xb[cfg.pos_of[c][: cfg.b]] = x[c * cfg.b : (c + 1) * cfg.b]
        maps.append(
            {
                "x_blk": xb,
                "W1": np.asarray(inputs["W1"], dtype=np.float32),
                "b1": np.asarray(inputs["b1"], dtype=np.float32),
                "W2": np.asarray(inputs["W2"], dtype=np.float32),
                "b2": np.asarray(inputs["b2"], dtype=np.float32),
                "deg_t": degt_all[c],
                "idx": idx_all[c],
                "seg": seg_all[c],
            }
        )
    return maps


def kernel(**inputs) -> np.ndarray:
    from concourse import bass_utils

    cfg = Cfg(n=100000, m=8)
    cfg.finish()
    edge_index = np.asarray(inputs["edge_index"])
    idx_all, seg_all, degt_all = preprocess(edge_index, cfg)
    nc = build_program(cfg)
    in_maps = make_in_maps(cfg, inputs, idx_all, seg_all, degt_all)
    res = bass_utils.run_bass_kernel_spmd(nc, in_maps, core_ids=list(range(cfg.m)))
    out = np.concatenate(
        [res.results[c]["y_blk"][cfg.pos_of[c][: cfg.b]] for c in range(cfg.m)],
        axis=0,
    )
    return out.astype(np.float32)



# revision 53
# speedup vs baseline: 1.3317x; 1.0764x over previous
"""ChebNet (K=4, two layers, log_softmax) on 8 Trainium2 NeuronCores.

Self-contained: takes FULL inputs, shards by destination node internally,
runs a single SPMD Bass kernel on cores 0-7 (with ncfw AllGather between
propagation hops), returns the FULL [N, 32] output.

Math (Horner re-association so propagations happen at output width):
    y = sum_k T_k(L) x @ W[k] + b,  T_k Chebyshev,  L = -D^-1/2 A D^-1/2
      = U0 + L(U1 + L(U2 + L U3)),  U_j = x @ V_j
    V0 = W0 - W2, V1 = W1 - 3 W3, V2 = 2 W2, V3 = 4 W3
    L S = -dis * A (dis * S)   with dis = where(deg>0, 1/sqrt(max(deg,1)), 0)
Scaled-space recurrence (tables hold S_hat = dis * S):
    S_hat3 = dis*U3;  S_hat_{j} = dis*U_j - dis^2 * (A S_hat_{j+1})
    final: y = U_0 - dis * (A S_hat_1) + b
Per hop, A S_hat is an edge-gather + segment-sum done as:
    indirect-DMA row gather (dest-sorted edge stream) ->
    one-hot (seg == iota) built on VectorE ->
    TensorE matmul accumulation into PSUM per 128-dest tile.
"""

import sys

if "/opt/trn_rl_repo" not in sys.path:
    sys.path.insert(0, "/opt/trn_rl_repo")

import math
from contextlib import ExitStack
from dataclasses import dataclass, field

import numpy as np

P = 128
PADIDX = 1 << 20


@dataclass
class Cfg:
    n: int            # nodes
    m: int            # cores
    c_in: int = 128
    c_hid: int = 64
    c_out: int = 32
    cw: int = 64      # gather-table row width (f32) for both layers
    gch: int = 8      # chunks per gather call (>8 overflows the SWDGE
                      # descriptor ring and hangs the device)
    ob: int = 32      # one-hot columns per build batch
    pb: int = 8       # psum tiles per writeout batch
    # filled by finish/preprocess
    b: int = 0        # dests per core
    t: int = 0        # 128-dest tiles per core
    bp: int = 0       # padded block rows = t*128
    nbuck: int = 4    # source buckets = node-block quarters
    qtiles: list = field(default_factory=list)  # tiles per quarter
    qrows: list = field(default_factory=list)   # rows per quarter (tile-aligned)
    qstart: list = field(default_factory=list)  # local row start per quarter
    nch: int = 0      # total chunks in edge stream
    nseg: int = 0     # total (tile,chunk) matmul segments
    chb: list = field(default_factory=list)     # global chunk base per bucket
    chn: list = field(default_factory=list)     # chunks per bucket
    sched: list = field(default_factory=list)   # [nb][t] = (c0,c1) local or None

    def finish(self):
        assert self.n % self.m == 0
        self.b = self.n // self.m
        self.t = (self.b + P - 1) // P
        self.bp = self.t * P
        # tile-aligned quarters of the node block; bucket q = all cores'
        # quarter-q rows (int16 gather range: 8*qrows <= 32767)
        tq = (self.t + 3) // 4
        self.qtiles = [tq, tq, tq, self.t - 3 * tq]
        self.qrows = [q * P for q in self.qtiles]
        self.qstart = [0]
        for q in self.qrows[:-1]:
            self.qstart.append(self.qstart[-1] + q)
        assert all(8 * q <= 32767 for q in self.qrows)


def preprocess(edge_index: np.ndarray, cfg: Cfg):
    """Shard edges by destination row-block; per core build a
    (bucket, tile)-major edge stream.  Cells (bucket x dest-tile) are
    padded to the cross-core max count (NOT 128-aligned), so the chunk
    schedule is shared across cores while chunks may straddle dest-tile
    boundaries; straddled chunks get one matmul segment per tile.
    Buckets are node-block quarters interleaved across cores so each
    bucket's table rows come from one AllGather piece."""
    import ml_dtypes

    row = np.asarray(edge_index[0], dtype=np.int64)
    col = np.asarray(edge_index[1], dtype=np.int64)
    n, m, b, t = cfg.n, cfg.m, cfg.b, cfg.t
    nb = cfg.nbuck
    qstart = np.array(cfg.qstart + [cfg.bp], dtype=np.int64)
    qrows = np.array(cfg.qrows, dtype=np.int64)

    deg = np.bincount(row, minlength=n).astype(np.float32)
    gt0 = np.concatenate([[0], np.cumsum(cfg.qtiles)]).astype(np.int64)

    # --- balance permutation: per core, permute dests WITHIN their node
    # quarter so each (src-quarter, dest-tile) cell count flattens to the
    # mean; the cross-core max then carries far less padding.  A node's
    # quarter is invariant, so other cores' source-row quarters are
    # unaffected (no cross-core coupling).
    raw = []
    pos_of = []
    for c in range(m):
        sel = (row >= c * b) & (row < (c + 1) * b)
        d = (row[sel] - c * b).astype(np.int64)
        s = col[sel].astype(np.int64)
        raw.append((d, s))
        srcq = np.searchsorted(qstart, s % b, side="right") - 1
        v = np.zeros((cfg.bp, nb), dtype=np.int64)
        np.add.at(v, (d, srcq), 1)
        pos = np.empty(cfg.bp, dtype=np.int64)
        for g in range(nb):
            lo = int(qstart[g])
            hi = min(int(qstart[g + 1]), b)
            nt = cfg.qtiles[g]
            cap = 128
            dq = np.arange(lo, int(qstart[g + 1]))      # slots incl pad
            real = dq[dq < hi]
            vq = v[real]                                 # [nreal, nb]
            orderq = np.argsort(-vq.sum(1), kind="stable")
            load = np.zeros((nt, nb), dtype=np.int64)
            cnt = np.zeros(nt, dtype=np.int64)
            slot_in_tile = np.zeros(len(real), dtype=np.int64)
            tile_pick = np.zeros(len(real), dtype=np.int64)
            for di in orderq:
                sc_ = np.where(
                    cnt < cap, (load + vq[di]).max(1), 1 << 40
                )
                tt = int(np.argmin(sc_))
                tile_pick[di] = tt
                slot_in_tile[di] = cnt[tt]
                load[tt] += vq[di]
                cnt[tt] += 1
            # pad dests fill remaining slots (any order)
            pos[real] = lo + tile_pick * cap + slot_in_tile
        pos_of.append(pos)
    cfg.pos_of = pos_of

    per_core = []
    counts = np.zeros((m, nb, t), dtype=np.int64)
    for c in range(m):
        d_orig, s = raw[c]
        d = pos_of[c][d_orig]
        sc, lr = s // b, s % b
        # source permuted position within its owner core
        spos = np.stack(pos_of)[sc, lr]
        q = np.searchsorted(qstart, spos, side="right") - 1
        lidx = sc * qrows[q] + (spos - qstart[q])   # int16-safe bucket row
        tile_id = d >> 7
        key = q * t + tile_id
        order = np.argsort(key, kind="stable")
        d, lidx, key = d[order], lidx[order], key[order]
        np.add.at(counts[c], (q[order], tile_id[order]), 1)
        per_core.append((d, lidx, key))

    M = counts.max(axis=0)                        # [nb, t] cell sizes
    # stream layout: (tile-group, bucket)-major passes; cells back to
    # back within a pass, pass tail 128-padded.  Group g = the dest
    # tiles of node-block quarter g, so acc quarter g is final after
    # its 4 bucket passes (early blk writeout -> early next-hop AG).
    gt0 = np.concatenate([[0], np.cumsum(cfg.qtiles)]).astype(np.int64)
    cell_start = np.zeros((nb, t), dtype=np.int64)
    cfg.chb, cfg.chn, cfg.sched = {}, {}, {}
    pass_slot_base = {}
    base = 0
    for g in range(nb):
        for bk in range(nb):
            pos = 0
            sched_p = {}
            for ti in range(int(gt0[g]), int(gt0[g + 1])):
                cell_start[bk, ti] = pos
                if M[bk, ti] == 0:
                    sched_p[ti] = None
                    continue
                c0, c1 = pos // P, (pos + M[bk, ti] - 1) // P
                sched_p[ti] = (int(c0), int(c1))
                pos += M[bk, ti]
            nloc = (pos + P - 1) // P
            cfg.chb[(g, bk)] = base // P
            cfg.chn[(g, bk)] = int(nloc)
            cfg.sched[(g, bk)] = sched_p
            pass_slot_base[(g, bk)] = base
            base += nloc * P
    S = base
    cfg.nch = S // P
    # one-hot segment order must match kernel consumption: bucket-major
    nseg = 0
    seg_tile, seg_chunk = [], []                  # per segment (global chunk)
    for bk in range(nb):
        for g in range(nb):
            cb = cfg.chb[(g, bk)]
            for ti in range(int(gt0[g]), int(gt0[g + 1])):
                rng = cfg.sched[(g, bk)][ti]
                if rng is None:
                    continue
                c0, c1 = rng
                for cl in range(c0, c1 + 1):
                    seg_tile.append(ti)
                    seg_chunk.append(cb + cl)
                nseg += c1 - c0 + 1
    cfg.nseg = nseg
    seg_tile = np.array(seg_tile, dtype=np.int64)
    seg_chunk = np.array(seg_chunk, dtype=np.int64)

    pass_base = np.zeros((nb, t), dtype=np.int64)
    for g in range(nb):
        for bk in range(nb):
            pass_base[bk, int(gt0[g]) : int(gt0[g + 1])] = pass_slot_base[(g, bk)]

    idx_all, seg_all, degt_all = [], [], []
    for c in range(m):
        d, lidx, key = per_core[c]
        idx = np.zeros(S, dtype=np.int16)          # pad -> bucket row 0
        stile = np.full(S, -2, dtype=np.int64)     # slot -> dest tile
        sval = np.zeros(S, dtype=np.float32)       # slot -> dest & 127
        cnt_flat = counts[c].reshape(-1)
        pos_in_cell = np.arange(d.size) - np.concatenate(
            [[0], np.cumsum(cnt_flat)]
        )[key]
        slot = (
            pass_base.reshape(-1)[key]
            + cell_start.reshape(-1)[key]
            + pos_in_cell
        )
        idx[slot] = lidx.astype(np.int16)
        stile[slot] = key % t
        sval[slot] = (d & 127).astype(np.float32)
        # segment columns: chunk slots where the edge belongs to this
        # segment's tile; everything else -1
        ch_t = stile.reshape(cfg.nch, P)[seg_chunk]      # [nseg, P]
        ch_v = sval.reshape(cfg.nch, P)[seg_chunk]
        segm = np.where(ch_t == seg_tile[:, None], ch_v, -1.0)
        seg_all.append(
            np.ascontiguousarray(segm.T).astype(ml_dtypes.bfloat16)
        )
        # dma_gather idx layout: slot i -> partition i%16 col i//16,
        # replicated across the 8 groups of 16 partitions
        idx16 = idx.reshape(S // 16, 16).T       # [16, S/16]
        idx16 = np.tile(idx16, (8, 1))           # [128, S/16]
        idx_all.append(np.ascontiguousarray(idx16))
        degb = np.zeros(cfg.bp, dtype=np.float32)
        degb[pos_of[c][:b]] = deg[c * b : (c + 1) * b]
        degt_all.append(np.ascontiguousarray(degb.reshape(t, P).T))  # [P, t]
    return idx_all, seg_all, degt_all


def build_program(cfg: Cfg):
    import concourse.bass as bass
    import concourse.tile as tile
    from concourse import bacc, mybir

    f32 = mybir.dt.float32
    bf16 = mybir.dt.bfloat16
    n, m, b, T, bp = cfg.n, cfg.m, cfg.b, cfg.t, cfg.bp
    CIN, HID, OUT, CW = cfg.c_in, cfg.c_hid, cfg.c_out, cfg.cw
    GCH, OB, PB = cfg.gch, cfg.ob, cfg.pb
    NCH, NB = cfg.nch, cfg.nbuck
    NSEG, CHB, CHN, SCHED = cfg.nseg, cfg.chb, cfg.chn, cfg.sched
    QT, QR, QS = cfg.qtiles, cfg.qrows, cfg.qstart
    NTAB = m * bp

    nc = bacc.Bacc(
        "TRN2",
        target_bir_lowering=False,
        debug=False,
        num_devices=m,
        num_swdge_queues=4,
    )

    # ---- I/O ----
    x_t = nc.dram_tensor("x_blk", [bp, CIN], f32, kind="ExternalInput")
    w1_t = nc.dram_tensor("W1", [4, CIN, HID], f32, kind="ExternalInput")
    b1_t = nc.dram_tensor("b1", [HID], f32, kind="ExternalInput")
    w2_t = nc.dram_tensor("W2", [4, HID, OUT], f32, kind="ExternalInput")
    b2_t = nc.dram_tensor("b2", [OUT], f32, kind="ExternalInput")
    degt_t = nc.dram_tensor("deg_t", [P, T], f32, kind="ExternalInput")
    idx_t = nc.dram_tensor(
        "idx", [P, NCH * 8], mybir.dt.int16, kind="ExternalInput"
    )
    seg_t = nc.dram_tensor("seg", [P, NSEG], mybir.dt.bfloat16, kind="ExternalInput")
    y_t = nc.dram_tensor("y_blk", [bp, OUT], f32, kind="ExternalOutput")

    # ---- internal DRAM ----
    u1 = nc.dram_tensor("U1", [3, bp, HID], f32)
    u2 = nc.dram_tensor("U2", [3, bp, OUT], f32)
    # bf16 gather rows must be 256B -> 128 bf16 wide; only cols :CW are real.
    CWB = 128
    blks, tabs = {}, {}
    for l in (1, 2):
        for j in (3, 2, 1):
            blks[(l, j)] = nc.dram_tensor(f"blk_{l}_{j}", [bp, CWB], bf16)
            tabs[(l, j)] = nc.dram_tensor(f"tab_{l}_{j}", [NTAB, CWB], bf16)

    import ml_dtypes

    iota_np = np.broadcast_to(
        np.tile(np.arange(P, dtype=np.float32), OB), (P, OB * P)
    ).astype(ml_dtypes.bfloat16)
    iota_d = nc.inline_tensor(iota_np, name="iota_rep")
    ident_d = nc.inline_tensor(np.eye(P, dtype=np.float32), name="ident")

    with ExitStack() as ctx:
        tc = ctx.enter_context(tile.TileContext(nc, num_cores=m))
        const = ctx.enter_context(tc.tile_pool(name="const", bufs=1))
        xp = ctx.enter_context(tc.tile_pool(name="xp", bufs=3))
        wp = ctx.enter_context(tc.tile_pool(name="wp", bufs=3))
        up = ctx.enter_context(tc.tile_pool(name="up", bufs=3))
        gp = ctx.enter_context(tc.tile_pool(name="gp", bufs=20))
        op = ctx.enter_context(tc.tile_pool(name="op", bufs=3))
        accp = ctx.enter_context(tc.tile_pool(name="accp", bufs=2))
        ep = ctx.enter_context(tc.tile_pool(name="ep", bufs=1))
        pst = ctx.enter_context(tc.tile_pool(name="pst", bufs=2, space="PSUM"))
        psu = ctx.enter_context(tc.tile_pool(name="psu", bufs=2, space="PSUM"))
        psa = ctx.enter_context(tc.tile_pool(name="psa", bufs=4, space="PSUM"))

        # ---- constants ----
        iota_s = const.tile([P, OB * P], bf16)
        nc.sync.dma_start(iota_s[:], iota_d[:, :])
        ident_s = const.tile([P, P], f32)
        nc.sync.dma_start(ident_s[:], ident_d[:, :])

        seg_s = const.tile([P, NSEG], bf16)
        nc.sync.dma_start(seg_s[:], seg_t[:, :])

        # whole gather-index stream resident in SBUF (no per-call DMAs)
        idx_s = const.tile([P, NCH * 8], mybir.dt.int16)
        nc.sync.dma_start(idx_s[:], idx_t[:, :])

        # V1cat [CIN, 4, HID], V2cat [HID, 4, OUT]
        w1s = const.tile([CIN, 4, HID], f32)
        nc.sync.dma_start(w1s[:], w1_t[:, :, :].rearrange("k p c -> p k c"))
        v1 = const.tile([CIN, 4, HID], f32)
        nc.vector.tensor_sub(v1[:, 0, :], w1s[:, 0, :], w1s[:, 2, :])
        nc.vector.tensor_scalar(
            out=v1[:, 1, :], in0=w1s[:, 3, :], scalar1=-3.0, scalar2=None,
            op0=mybir.AluOpType.mult,
        )
        nc.vector.tensor_add(v1[:, 1, :], v1[:, 1, :], w1s[:, 1, :])
        nc.vector.tensor_scalar(
            out=v1[:, 2, :], in0=w1s[:, 2, :], scalar1=2.0, scalar2=None,
            op0=mybir.AluOpType.mult,
        )
        nc.vector.tensor_scalar(
            out=v1[:, 3, :], in0=w1s[:, 3, :], scalar1=4.0, scalar2=None,
            op0=mybir.AluOpType.mult,
        )

        w2s = const.tile([HID, 4, OUT], f32)
        nc.sync.dma_start(w2s[:], w2_t[:, :, :].rearrange("k p c -> p k c"))
        v2 = const.tile([HID, 4, OUT], f32)
        nc.vector.tensor_sub(v2[:, 0, :], w2s[:, 0, :], w2s[:, 2, :])
        nc.vector.tensor_scalar(
            out=v2[:, 1, :], in0=w2s[:, 3, :], scalar1=-3.0, scalar2=None,
            op0=mybir.AluOpType.mult,
        )
        nc.vector.tensor_add(v2[:, 1, :], v2[:, 1, :], w2s[:, 1, :])
        nc.vector.tensor_scalar(
            out=v2[:, 2, :], in0=w2s[:, 2, :], scalar1=2.0, scalar2=None,
            op0=mybir.AluOpType.mult,
        )
        nc.vector.tensor_scalar(
            out=v2[:, 3, :], in0=w2s[:, 3, :], scalar1=4.0, scalar2=None,
            op0=mybir.AluOpType.mult,
        )

        b1s = const.tile([P, HID], f32)
        nc.sync.dma_start(b1s[:1, :], b1_t[:].rearrange("(o c) -> o c", o=1))
        nc.gpsimd.partition_broadcast(b1s[:, :], b1s[:1, :])
        b2s = const.tile([P, OUT], f32)
        nc.sync.dma_start(b2s[:1, :], b2_t[:].rearrange("(o c) -> o c", o=1))
        nc.gpsimd.partition_broadcast(b2s[:, :], b2s[:1, :])

        # DMA-completion semaphores for prepare_only gathers (one per queue)
        dma_sems = [nc.alloc_semaphore(f"swdge_dma{q}") for q in range(4)]

        # ---- dis, -dis, -dis^2 in [P, T] (partition p, slot t) = dest 128t+p ----
        degs = const.tile([P, T], f32)
        nc.sync.dma_start(degs[:], degt_t[:, :])
        dis = const.tile([P, T], f32)
        ndis = const.tile([P, T], f32)
        ndis2 = const.tile([P, T], f32)
        tmp = const.tile([P, T], f32)
        nc.vector.tensor_scalar(
            out=tmp[:], in0=degs[:], scalar1=1.0, scalar2=None,
            op0=mybir.AluOpType.max,
        )
        nc.scalar.activation(tmp[:], tmp[:], mybir.ActivationFunctionType.Sqrt)
        nc.vector.reciprocal(dis[:], tmp[:])
        nc.vector.tensor_scalar(
            out=tmp[:], in0=degs[:], scalar1=0.0, scalar2=None,
            op0=mybir.AluOpType.is_gt,
        )
        nc.vector.tensor_mul(dis[:], dis[:], tmp[:])
        nc.vector.tensor_scalar(
            out=ndis[:], in0=dis[:], scalar1=-1.0, scalar2=None,
            op0=mybir.AluOpType.mult,
        )
        nc.vector.tensor_mul(ndis2[:], dis[:], ndis[:])

        # ---- prologue: U_j = src @ V_j ; U1/U2 to HBM (j=1,2 scaled by dis);
        #      j=3 scaled -> blk  ----
        def prologue(src, cl, w, vcat, u_hbm, blk_hbm, from_sbuf=False):
            for k in range(T):
                if from_sbuf:
                    xc = src[:, k, :]
                else:
                    xc = xp.tile([P, cl], f32, tag="xc")
                    nc.sync.dma_start(xc[:], src[k * P : (k + 1) * P, :])
                tp = pst.tile([cl, P], f32, space="PSUM", tag="tp")
                nc.tensor.transpose(out=tp[:, :], in_=xc[:, :], identity=ident_s[:])
                xT = wp.tile([cl, P], f32, tag="xT")
                nc.vector.tensor_copy(xT[:], tp[:, :])
                upsum = psu.tile([P, 4, w], f32, space="PSUM", tag="upsum")
                nc.tensor.matmul(
                    out=upsum[:].rearrange("p a c -> p (a c)"),
                    lhsT=xT[:, :],
                    rhs=vcat[:].rearrange("p a c -> p (a c)"),
                    start=True, stop=True,
                )
                ut = wp.tile([P, 3, w], f32, tag="ut")
                # fold bias into U_0 once (y = U0 + b - dis*(A S1))
                bb = b1s if w == HID else b2s
                nc.vector.tensor_add(ut[:, 0, :], upsum[:, 0, :], bb[:, :w])
                for j in (1, 2):
                    nc.vector.tensor_scalar(
                        out=ut[:, j, :], in0=upsum[:, j, :],
                        scalar1=dis[:, k : k + 1], scalar2=None,
                        op0=mybir.AluOpType.mult,
                    )
                nc.sync.dma_start(
                    u_hbm[:, k * P : (k + 1) * P, :].rearrange("a p c -> p a c"),
                    ut[:],
                )
                bt = wp.tile([P, w], bf16, tag="bt")
                nc.vector.tensor_scalar(
                    out=bt[:], in0=upsum[:, 3, :],
                    scalar1=dis[:, k : k + 1], scalar2=None,
                    op0=mybir.AluOpType.mult,
                )
                nc.sync.dma_start(blk_hbm[k * P : (k + 1) * P, :w], bt[:])

        # ---- one hop: table = AG(blk_in) split per bucket (overlaps the
        # gather passes); acc = scale*(A_gather) + U_j.  Edge stream is
        # (bucket, tile)-major with exact-max cells: chunks may straddle
        # tile boundaries, one matmul segment per (tile, chunk).  On the
        # last bucket pass, finished acc quarters are cast+written to
        # blk_out so the next hop's AllGather pieces can start early.
        def hop(l, j, blk_in, table, w, blk_out):
            for q in range(NB):
                nc.gpsimd.collective_compute(
                    "AllGather",
                    mybir.AluOpType.bypass,
                    replica_groups=[list(range(m))],
                    ins=[blk_in[QS[q] : QS[q] + QR[q], :].opt()],
                    outs=[table[8 * QS[q] : 8 * (QS[q] + QR[q]), :].opt()],
                )
            final = j == 0
            u_hbm = u1 if l == 1 else u2
            acc = accp.tile([P, T, w], f32, tag="acc")

            gath = None
            oneh = None
            o = 0           # segment ordinal (one-hot column)
            gcall = 0       # gather-call counter (round-robin SWDGE queues:
                            # each queue runs on its own Q7 cpu pair)
            # pass order: bucket-major (all of bucket bk across every tile
            # group) so CC AllGathers land in the gather lulls at hop
            # boundaries; quarter g of acc is written out as soon as its
            # bucket-3 batches finish.
            for bk in range(NB):
                gt0 = 0
                for g in range(NB):
                    gtn = QT[g]
                    tab_lo = 8 * QS[bk]
                    tab_hi = 8 * (QS[bk] + QR[bk])
                    cb = CHB[(g, bk)]
                    nloc = CHN[(g, bk)]
                    sched_p = SCHED[(g, bk)]
                    gnext = 0   # next ungathered pass-local chunk
                    gbase = 0   # current gather batch start (pass-local)
                    nbatch = (gtn + PB - 1) // PB
                    for bi in range(nbatch):
                        t0 = gt0 + bi * PB
                        tcnt = min(PB, gt0 + gtn - t0)
                        psum = psa.tile(
                            [P, PB, w], f32, space="PSUM", tag="apsum"
                        )
                        for tt in range(tcnt):
                            ti = t0 + tt
                            rng = sched_p[ti]
                            if rng is None:
                                nc.vector.memset(psum[:, tt, :], 0.0)
                                continue
                            c0l, c1l = rng
                            for k, cl in enumerate(range(c0l, c1l + 1)):
                                if cl >= gnext:
                                    glen = min(GCH, nloc - gnext)
                                    gbase = gnext
                                    cg = cb + gnext
                                    gath = gp.tile(
                                        [P, GCH, CWB], bf16, tag="gath"
                                    )
                                    nc.gpsimd.dma_gather(
                                        out_ap=gath[:, :glen, :],
                                        in_ap=table[tab_lo:tab_hi, :],
                                        idxs_ap=idx_s[
                                            :, cg * 8 : (cg + glen) * 8
                                        ],
                                        num_idxs=glen * P,
                                        num_idxs_reg=glen * P,
                                        elem_size=CWB,
                                        queue_num=gcall % 4,
                                    )
                                    gcall += 1
                                    gnext += glen
                                gpos = cl - gbase
                                oc, opos = divmod(o, OB)
                                if opos == 0:
                                    olen = min(OB, NSEG - oc * OB)
                                    oneh = op.tile(
                                        [P, OB, P], bf16, tag="oneh"
                                    )
                                    nc.vector.tensor_tensor(
                                        out=oneh[:, :olen, :],
                                        in0=iota_s[:].rearrange(
                                            "p (a q) -> p a q", q=P
                                        )[:, :olen, :],
                                        in1=seg_s[:, oc * OB : oc * OB + olen]
                                        .to_broadcast([P, olen, P]),
                                        op=mybir.AluOpType.is_equal,
                                    )
                                nc.tensor.matmul(
                                    out=psum[:, tt, :],
                                    lhsT=oneh[:, opos, :],
                                    rhs=gath[:, gpos, :w],
                                    start=(k == 0),
                                    stop=(k == c1l - c0l),
                                )
                                o += 1
                        # fold this pass's aggregation into acc
                        sl = slice(t0, t0 + tcnt)
                        if bk == 0:
                            nc.vector.tensor_copy(
                                acc[:, sl, :], psum[:, :tcnt, :]
                            )
                        else:
                            nc.vector.tensor_add(
                                acc[:, sl, :], acc[:, sl, :],
                                psum[:, :tcnt, :],
                            )
                        if bk == NB - 1:
                            scale = ndis if final else ndis2
                            nc.vector.tensor_tensor(
                                out=acc[:, sl, :],
                                in0=acc[:, sl, :],
                                in1=scale[:, sl].to_broadcast([P, tcnt, w]),
                                op=mybir.AluOpType.mult,
                            )
                            uti = up.tile([P, PB, w], f32, tag="uti")
                            nc.sync.dma_start(
                                uti[:, :tcnt, :],
                                u_hbm[
                                    j if not final else 0,
                                    t0 * P : (t0 + tcnt) * P, :,
                                ].rearrange("(a p) c -> p a c", p=P),
                            )
                            nc.vector.tensor_add(
                                acc[:, sl, :], acc[:, sl, :],
                                uti[:, :tcnt, :],
                            )
                            if final and l == 1:
                                nc.scalar.activation(
                                    acc[:, sl, :],
                                    acc[:, sl, :],
                                    mybir.ActivationFunctionType.Relu,
                                )
                    # acc quarter g final on the last bucket pass: cast +
                    # write to blk so the next hop's AG piece g starts early
                    if bk == NB - 1 and blk_out is not None:
                        ab = wp.tile([P, gtn, w], bf16, tag="accb")
                        nc.vector.tensor_copy(
                            ab[:], acc[:, gt0 : gt0 + gtn, :]
                        )
                        nc.sync.dma_start(
                            blk_out[QS[g] : QS[g] + QR[g], :w].rearrange(
                                "(a p) c -> p a c", p=P
                            ),
                            ab[:],
                        )
                    gt0 += gtn
            return acc

        def acc_to_hbm(acc, dst, w):
            nc.sync.dma_start(
                dst[:, :].rearrange("(a p) c -> p a c", p=P), acc[:, :, :w]
            )

        # ================= layer 1 =================
        prologue(x_t, CIN, HID, v1, u1, blks[(1, 3)])
        acc = hop(1, 2, blks[(1, 3)], tabs[(1, 3)], HID, blks[(1, 2)])
        acc = hop(1, 1, blks[(1, 2)], tabs[(1, 2)], HID, blks[(1, 1)])
        acc = hop(1, 0, blks[(1, 1)], tabs[(1, 1)], HID, None)

        # ================= layer 2 (prologue reads h=acc from SBUF) ======
        prologue(acc, HID, OUT, v2, u2, blks[(2, 3)], from_sbuf=True)
        acc = hop(2, 2, blks[(2, 3)], tabs[(2, 3)], OUT, blks[(2, 2)])
        acc = hop(2, 1, blks[(2, 2)], tabs[(2, 2)], OUT, blks[(2, 1)])
        acc = hop(2, 0, blks[(2, 1)], tabs[(2, 1)], OUT, None)

        # ---- log_softmax over acc [P, T, OUT] (in place) ----
        red = wp.tile([P, T], f32, tag="red")
        nc.vector.tensor_reduce(
            out=red[:], in_=acc[:, :, :], axis=mybir.AxisListType.X,
            op=mybir.AluOpType.max,
        )
        nc.vector.tensor_tensor(
            out=acc[:], in0=acc[:, :, :], in1=red[:].to_broadcast([P, T, OUT]),
            op=mybir.AluOpType.subtract,
        )
        ex = ep.tile([P, T, OUT], f32, tag="ex")
        nc.scalar.activation(ex[:], acc[:], mybir.ActivationFunctionType.Exp)
        nc.vector.tensor_reduce(
            out=red[:], in_=ex[:, :, :], axis=mybir.AxisListType.X,
            op=mybir.AluOpType.add,
        )
        nc.scalar.activation(red[:], red[:], mybir.ActivationFunctionType.Ln)
        nc.vector.tensor_tensor(
            out=acc[:], in0=acc[:, :, :], in1=red[:].to_broadcast([P, T, OUT]),
            op=mybir.AluOpType.subtract,
        )
        nc.sync.dma_start(
            y_t[:, :].rearrange("(a p) c -> p a c", p=P), acc[:, :, :]
        )

    nc.compile()
    return nc


def make_in_maps(cfg: Cfg, inputs: dict, idx_all, seg_all, degt_all):
    x = np.asarray(inputs["x"], dtype=np.float32)
    maps = []
    for c in range(cfg.m):
        xb = np.zeros((cfg.bp, cfg.c_in), dtype=np.float32)
        xb[cfg.pos_of[c][: cfg.b]] = x[c * cfg.b : (c + 1) * cfg.b]
        maps.append(
            {
                "x_blk": xb,
                "W1": np.asarray(inputs["W1"], dtype=np.float32),
                "b1": np.asarray(inputs["b1"], dtype=np.float32),
                "W2": np.asarray(inputs["W2"], dtype=np.float32),
                "b2": np.asarray(inputs["b2"], dtype=np.float32),
                "deg_t": degt_all[c],
                "idx": idx_all[c],
                "seg": seg_all[c],
            }
        )
    return maps


def kernel(**inputs) -> np.ndarray:
    from concourse import bass_utils

    cfg = Cfg(n=100000, m=8)
    cfg.finish()
    edge_index = np.asarray(inputs["edge_index"])
    idx_all, seg_all, degt_all = preprocess(edge_index, cfg)
    nc = build_program(cfg)
    in_maps = make_in_maps(cfg, inputs, idx_all, seg_all, degt_all)
    res = bass_utils.run_bass_kernel_spmd(nc, in_maps, core_ids=list(range(cfg.m)))
    out = np.concatenate(
        [res.results[c]["y_blk"][cfg.pos_of[c][: cfg.b]] for c in range(cfg.m)],
        axis=0,
    )
    return out.astype(np.float32)



# revision 54
# speedup vs baseline: 1.3370x; 1.0040x over previous
"""ChebNet (K=4, two layers, log_softmax) on 8 Trainium2 NeuronCores.

Self-contained: takes FULL inputs, shards by destination node internally,
runs a single SPMD Bass kernel on cores 0-7 (with ncfw AllGather between
propagation hops), returns the FULL [N, 32] output.

Math (Horner re-association so propagations happen at output width):
    y = sum_k T_k(L) x @ W[k] + b,  T_k Chebyshev,  L = -D^-1/2 A D^-1/2
      = U0 + L(U1 + L(U2 + L U3)),  U_j = x @ V_j
    V0 = W0 - W2, V1 = W1 - 3 W3, V2 = 2 W2, V3 = 4 W3
    L S = -dis * A (dis * S)   with dis = where(deg>0, 1/sqrt(max(deg,1)), 0)
Scaled-space recurrence (tables hold S_hat = dis * S):
    S_hat3 = dis*U3;  S_hat_{j} = dis*U_j - dis^2 * (A S_hat_{j+1})
    final: y = U_0 - dis * (A S_hat_1) + b
Per hop, A S_hat is an edge-gather + segment-sum done as:
    indirect-DMA row gather (dest-sorted edge stream) ->
    one-hot (seg == iota) built on VectorE ->
    TensorE matmul accumulation into PSUM per 128-dest tile.
"""

import sys

if "/opt/trn_rl_repo" not in sys.path:
    sys.path.insert(0, "/opt/trn_rl_repo")

import math
from contextlib import ExitStack
from dataclasses import dataclass, field

import numpy as np

P = 128
PADIDX = 1 << 20


@dataclass
class Cfg:
    n: int            # nodes
    m: int            # cores
    c_in: int = 128
    c_hid: int = 64
    c_out: int = 32
    cw: int = 64      # gather-table row width (f32) for both layers
    gch: int = 8      # chunks per gather call (>8 overflows the SWDGE
                      # descriptor ring and hangs the device)
    ob: int = 16      # one-hot columns per build batch
    pb: int = 8       # psum tiles per writeout batch
    # filled by finish/preprocess
    b: int = 0        # dests per core
    t: int = 0        # 128-dest tiles per core
    bp: int = 0       # padded block rows = t*128
    nbuck: int = 4    # source buckets = node-block quarters
    qtiles: list = field(default_factory=list)  # tiles per quarter
    qrows: list = field(default_factory=list)   # rows per quarter (tile-aligned)
    qstart: list = field(default_factory=list)  # local row start per quarter
    nch: int = 0      # total chunks in edge stream
    nseg: int = 0     # total (tile,chunk) matmul segments
    chb: list = field(default_factory=list)     # global chunk base per bucket
    chn: list = field(default_factory=list)     # chunks per bucket
    sched: list = field(default_factory=list)   # [nb][t] = (c0,c1) local or None

    def finish(self):
        assert self.n % self.m == 0
        self.b = self.n // self.m
        self.t = (self.b + P - 1) // P
        self.bp = self.t * P
        # tile-aligned quarters of the node block; bucket q = all cores'
        # quarter-q rows (int16 gather range: 8*qrows <= 32767)
        tq = (self.t + 3) // 4
        self.qtiles = [tq, tq, tq, self.t - 3 * tq]
        self.qrows = [q * P for q in self.qtiles]
        self.qstart = [0]
        for q in self.qrows[:-1]:
            self.qstart.append(self.qstart[-1] + q)
        assert all(8 * q <= 32767 for q in self.qrows)


def preprocess(edge_index: np.ndarray, cfg: Cfg):
    """Shard edges by destination row-block; per core build a
    (bucket, tile)-major edge stream.  Cells (bucket x dest-tile) are
    padded to the cross-core max count (NOT 128-aligned), so the chunk
    schedule is shared across cores while chunks may straddle dest-tile
    boundaries; straddled chunks get one matmul segment per tile.
    Buckets are node-block quarters interleaved across cores so each
    bucket's table rows come from one AllGather piece."""
    import ml_dtypes

    row = np.asarray(edge_index[0], dtype=np.int64)
    col = np.asarray(edge_index[1], dtype=np.int64)
    n, m, b, t = cfg.n, cfg.m, cfg.b, cfg.t
    nb = cfg.nbuck
    qstart = np.array(cfg.qstart + [cfg.bp], dtype=np.int64)
    qrows = np.array(cfg.qrows, dtype=np.int64)

    deg = np.bincount(row, minlength=n).astype(np.float32)
    gt0 = np.concatenate([[0], np.cumsum(cfg.qtiles)]).astype(np.int64)

    # --- balance permutation: per core, permute dests WITHIN their node
    # quarter so each (src-quarter, dest-tile) cell count flattens to the
    # mean; the cross-core max then carries far less padding.  A node's
    # quarter is invariant, so other cores' source-row quarters are
    # unaffected (no cross-core coupling).
    raw = []
    pos_of = []
    for c in range(m):
        sel = (row >= c * b) & (row < (c + 1) * b)
        d = (row[sel] - c * b).astype(np.int64)
        s = col[sel].astype(np.int64)
        raw.append((d, s))
        srcq = np.searchsorted(qstart, s % b, side="right") - 1
        v = np.zeros((cfg.bp, nb), dtype=np.int64)
        np.add.at(v, (d, srcq), 1)
        pos = np.empty(cfg.bp, dtype=np.int64)
        for g in range(nb):
            lo = int(qstart[g])
            hi = min(int(qstart[g + 1]), b)
            nt = cfg.qtiles[g]
            cap = 128
            dq = np.arange(lo, int(qstart[g + 1]))      # slots incl pad
            real = dq[dq < hi]
            vq = v[real]                                 # [nreal, nb]
            orderq = np.argsort(-vq.sum(1), kind="stable")
            load = np.zeros((nt, nb), dtype=np.int64)
            cnt = np.zeros(nt, dtype=np.int64)
            slot_in_tile = np.zeros(len(real), dtype=np.int64)
            tile_pick = np.zeros(len(real), dtype=np.int64)
            for di in orderq:
                sc_ = np.where(
                    cnt < cap, (load + vq[di]).max(1), 1 << 40
                )
                tt = int(np.argmin(sc_))
                tile_pick[di] = tt
                slot_in_tile[di] = cnt[tt]
                load[tt] += vq[di]
                cnt[tt] += 1
            # pad dests fill remaining slots (any order)
            pos[real] = lo + tile_pick * cap + slot_in_tile
        pos_of.append(pos)
    cfg.pos_of = pos_of

    per_core = []
    counts = np.zeros((m, nb, t), dtype=np.int64)
    for c in range(m):
        d_orig, s = raw[c]
        d = pos_of[c][d_orig]
        sc, lr = s // b, s % b
        # source permuted position within its owner core
        spos = np.stack(pos_of)[sc, lr]
        q = np.searchsorted(qstart, spos, side="right") - 1
        lidx = sc * qrows[q] + (spos - qstart[q])   # int16-safe bucket row
        tile_id = d >> 7
        key = q * t + tile_id
        order = np.argsort(key, kind="stable")
        d, lidx, key = d[order], lidx[order], key[order]
        np.add.at(counts[c], (q[order], tile_id[order]), 1)
        per_core.append((d, lidx, key))

    M = counts.max(axis=0)                        # [nb, t] cell sizes
    # stream layout: (tile-group, bucket)-major passes; cells back to
    # back within a pass, pass tail 128-padded.  Group g = the dest
    # tiles of node-block quarter g, so acc quarter g is final after
    # its 4 bucket passes (early blk writeout -> early next-hop AG).
    gt0 = np.concatenate([[0], np.cumsum(cfg.qtiles)]).astype(np.int64)
    cell_start = np.zeros((nb, t), dtype=np.int64)
    cfg.chb, cfg.chn, cfg.sched = {}, {}, {}
    pass_slot_base = {}
    base = 0
    for g in range(nb):
        for bk in range(nb):
            pos = 0
            sched_p = {}
            for ti in range(int(gt0[g]), int(gt0[g + 1])):
                cell_start[bk, ti] = pos
                if M[bk, ti] == 0:
                    sched_p[ti] = None
                    continue
                c0, c1 = pos // P, (pos + M[bk, ti] - 1) // P
                sched_p[ti] = (int(c0), int(c1))
                pos += M[bk, ti]
            nloc = (pos + P - 1) // P
            cfg.chb[(g, bk)] = base // P
            cfg.chn[(g, bk)] = int(nloc)
            cfg.sched[(g, bk)] = sched_p
            pass_slot_base[(g, bk)] = base
            base += nloc * P
    S = base
    cfg.nch = S // P
    # one-hot segment order must match kernel consumption: bucket-major
    nseg = 0
    seg_tile, seg_chunk = [], []                  # per segment (global chunk)
    for bk in range(nb):
        for g in range(nb):
            cb = cfg.chb[(g, bk)]
            for ti in range(int(gt0[g]), int(gt0[g + 1])):
                rng = cfg.sched[(g, bk)][ti]
                if rng is None:
                    continue
                c0, c1 = rng
                for cl in range(c0, c1 + 1):
                    seg_tile.append(ti)
                    seg_chunk.append(cb + cl)
                nseg += c1 - c0 + 1
    cfg.nseg = nseg
    seg_tile = np.array(seg_tile, dtype=np.int64)
    seg_chunk = np.array(seg_chunk, dtype=np.int64)

    pass_base = np.zeros((nb, t), dtype=np.int64)
    for g in range(nb):
        for bk in range(nb):
            pass_base[bk, int(gt0[g]) : int(gt0[g + 1])] = pass_slot_base[(g, bk)]

    idx_all, seg_all, degt_all = [], [], []
    for c in range(m):
        d, lidx, key = per_core[c]
        idx = np.zeros(S, dtype=np.int16)          # pad -> bucket row 0
        stile = np.full(S, -2, dtype=np.int64)     # slot -> dest tile
        sval = np.zeros(S, dtype=np.float32)       # slot -> dest & 127
        cnt_flat = counts[c].reshape(-1)
        pos_in_cell = np.arange(d.size) - np.concatenate(
            [[0], np.cumsum(cnt_flat)]
        )[key]
        slot = (
            pass_base.reshape(-1)[key]
            + cell_start.reshape(-1)[key]
            + pos_in_cell
        )
        idx[slot] = lidx.astype(np.int16)
        stile[slot] = key % t
        sval[slot] = (d & 127).astype(np.float32)
        # segment columns: chunk slots where the edge belongs to this
        # segment's tile; everything else -1
        ch_t = stile.reshape(cfg.nch, P)[seg_chunk]      # [nseg, P]
        ch_v = sval.reshape(cfg.nch, P)[seg_chunk]
        segm = np.where(ch_t == seg_tile[:, None], ch_v, -1.0)
        seg_all.append(
            np.ascontiguousarray(segm.T).astype(ml_dtypes.bfloat16)
        )
        # dma_gather idx layout: slot i -> partition i%16 col i//16,
        # replicated across the 8 groups of 16 partitions
        idx16 = idx.reshape(S // 16, 16).T       # [16, S/16]
        idx16 = np.tile(idx16, (8, 1))           # [128, S/16]
        idx_all.append(np.ascontiguousarray(idx16))
        degb = np.zeros(cfg.bp, dtype=np.float32)
        degb[pos_of[c][:b]] = deg[c * b : (c + 1) * b]
        degt_all.append(np.ascontiguousarray(degb.reshape(t, P).T))  # [P, t]
    return idx_all, seg_all, degt_all


def build_program(cfg: Cfg):
    import concourse.bass as bass
    import concourse.tile as tile
    from concourse import bacc, mybir

    f32 = mybir.dt.float32
    bf16 = mybir.dt.bfloat16
    n, m, b, T, bp = cfg.n, cfg.m, cfg.b, cfg.t, cfg.bp
    CIN, HID, OUT, CW = cfg.c_in, cfg.c_hid, cfg.c_out, cfg.cw
    GCH, OB, PB = cfg.gch, cfg.ob, cfg.pb
    NCH, NB = cfg.nch, cfg.nbuck
    NSEG, CHB, CHN, SCHED = cfg.nseg, cfg.chb, cfg.chn, cfg.sched
    QT, QR, QS = cfg.qtiles, cfg.qrows, cfg.qstart
    NTAB = m * bp

    nc = bacc.Bacc(
        "TRN2",
        target_bir_lowering=False,
        debug=False,
        num_devices=m,
        num_swdge_queues=4,
    )

    # ---- I/O ----
    x_t = nc.dram_tensor("x_blk", [bp, CIN], f32, kind="ExternalInput")
    w1_t = nc.dram_tensor("W1", [4, CIN, HID], f32, kind="ExternalInput")
    b1_t = nc.dram_tensor("b1", [HID], f32, kind="ExternalInput")
    w2_t = nc.dram_tensor("W2", [4, HID, OUT], f32, kind="ExternalInput")
    b2_t = nc.dram_tensor("b2", [OUT], f32, kind="ExternalInput")
    degt_t = nc.dram_tensor("deg_t", [P, T], f32, kind="ExternalInput")
    idx_t = nc.dram_tensor(
        "idx", [P, NCH * 8], mybir.dt.int16, kind="ExternalInput"
    )
    seg_t = nc.dram_tensor("seg", [P, NSEG], mybir.dt.bfloat16, kind="ExternalInput")
    y_t = nc.dram_tensor("y_blk", [bp, OUT], f32, kind="ExternalOutput")

    # ---- internal DRAM ----
    u1 = nc.dram_tensor("U1", [3, bp, HID], f32)
    u2 = nc.dram_tensor("U2", [3, bp, OUT], f32)
    # bf16 gather rows must be 256B -> 128 bf16 wide; only cols :CW are real.
    CWB = 128
    blks, tabs = {}, {}
    for l in (1, 2):
        for j in (3, 2, 1):
            blks[(l, j)] = nc.dram_tensor(f"blk_{l}_{j}", [bp, CWB], bf16)
            tabs[(l, j)] = nc.dram_tensor(f"tab_{l}_{j}", [NTAB, CWB], bf16)

    import ml_dtypes

    iota_np = np.broadcast_to(
        np.tile(np.arange(P, dtype=np.float32), OB), (P, OB * P)
    ).astype(ml_dtypes.bfloat16)
    iota_d = nc.inline_tensor(iota_np, name="iota_rep")
    ident_d = nc.inline_tensor(np.eye(P, dtype=np.float32), name="ident")

    with ExitStack() as ctx:
        tc = ctx.enter_context(tile.TileContext(nc, num_cores=m))
        const = ctx.enter_context(tc.tile_pool(name="const", bufs=1))
        xp = ctx.enter_context(tc.tile_pool(name="xp", bufs=3))
        wp = ctx.enter_context(tc.tile_pool(name="wp", bufs=3))
        up = ctx.enter_context(tc.tile_pool(name="up", bufs=3))
        gp = ctx.enter_context(tc.tile_pool(name="gp", bufs=24))
        op = ctx.enter_context(tc.tile_pool(name="op", bufs=4))
        accp = ctx.enter_context(tc.tile_pool(name="accp", bufs=2))
        ep = ctx.enter_context(tc.tile_pool(name="ep", bufs=1))
        pst = ctx.enter_context(tc.tile_pool(name="pst", bufs=2, space="PSUM"))
        psu = ctx.enter_context(tc.tile_pool(name="psu", bufs=2, space="PSUM"))
        psa = ctx.enter_context(tc.tile_pool(name="psa", bufs=4, space="PSUM"))

        # ---- constants ----
        iota_s = const.tile([P, OB * P], bf16)
        nc.sync.dma_start(iota_s[:], iota_d[:, :])
        ident_s = const.tile([P, P], f32)
        nc.sync.dma_start(ident_s[:], ident_d[:, :])

        seg_s = const.tile([P, NSEG], bf16)
        nc.sync.dma_start(seg_s[:], seg_t[:, :])

        # whole gather-index stream resident in SBUF (no per-call DMAs)
        idx_s = const.tile([P, NCH * 8], mybir.dt.int16)
        nc.sync.dma_start(idx_s[:], idx_t[:, :])

        # V1cat [CIN, 4, HID], V2cat [HID, 4, OUT]
        w1s = const.tile([CIN, 4, HID], f32)
        nc.sync.dma_start(w1s[:], w1_t[:, :, :].rearrange("k p c -> p k c"))
        v1 = const.tile([CIN, 4, HID], f32)
        nc.vector.tensor_sub(v1[:, 0, :], w1s[:, 0, :], w1s[:, 2, :])
        nc.vector.tensor_scalar(
            out=v1[:, 1, :], in0=w1s[:, 3, :], scalar1=-3.0, scalar2=None,
            op0=mybir.AluOpType.mult,
        )
        nc.vector.tensor_add(v1[:, 1, :], v1[:, 1, :], w1s[:, 1, :])
        nc.vector.tensor_scalar(
            out=v1[:, 2, :], in0=w1s[:, 2, :], scalar1=2.0, scalar2=None,
            op0=mybir.AluOpType.mult,
        )
        nc.vector.tensor_scalar(
            out=v1[:, 3, :], in0=w1s[:, 3, :], scalar1=4.0, scalar2=None,
            op0=mybir.AluOpType.mult,
        )

        w2s = const.tile([HID, 4, OUT], f32)
        nc.sync.dma_start(w2s[:], w2_t[:, :, :].rearrange("k p c -> p k c"))
        v2 = const.tile([HID, 4, OUT], f32)
        nc.vector.tensor_sub(v2[:, 0, :], w2s[:, 0, :], w2s[:, 2, :])
        nc.vector.tensor_scalar(
            out=v2[:, 1, :], in0=w2s[:, 3, :], scalar1=-3.0, scalar2=None,
            op0=mybir.AluOpType.mult,
        )
        nc.vector.tensor_add(v2[:, 1, :], v2[:, 1, :], w2s[:, 1, :])
        nc.vector.tensor_scalar(
            out=v2[:, 2, :], in0=w2s[:, 2, :], scalar1=2.0, scalar2=None,
            op0=mybir.AluOpType.mult,
        )
        nc.vector.tensor_scalar(
            out=v2[:, 3, :], in0=w2s[:, 3, :], scalar1=4.0, scalar2=None,
            op0=mybir.AluOpType.mult,
        )

        b1s = const.tile([P, HID], f32)
        nc.sync.dma_start(b1s[:1, :], b1_t[:].rearrange("(o c) -> o c", o=1))
        nc.gpsimd.partition_broadcast(b1s[:, :], b1s[:1, :])
        b2s = const.tile([P, OUT], f32)
        nc.sync.dma_start(b2s[:1, :], b2_t[:].rearrange("(o c) -> o c", o=1))
        nc.gpsimd.partition_broadcast(b2s[:, :], b2s[:1, :])

        # DMA-completion semaphores for prepare_only gathers (one per queue)
        dma_sems = [nc.alloc_semaphore(f"swdge_dma{q}") for q in range(4)]

        # ---- dis, -dis, -dis^2 in [P, T] (partition p, slot t) = dest 128t+p ----
        degs = const.tile([P, T], f32)
        nc.sync.dma_start(degs[:], degt_t[:, :])
        dis = const.tile([P, T], f32)
        ndis = const.tile([P, T], f32)
        ndis2 = const.tile([P, T], f32)
        tmp = const.tile([P, T], f32)
        nc.vector.tensor_scalar(
            out=tmp[:], in0=degs[:], scalar1=1.0, scalar2=None,
            op0=mybir.AluOpType.max,
        )
        nc.scalar.activation(tmp[:], tmp[:], mybir.ActivationFunctionType.Sqrt)
        nc.vector.reciprocal(dis[:], tmp[:])
        nc.vector.tensor_scalar(
            out=tmp[:], in0=degs[:], scalar1=0.0, scalar2=None,
            op0=mybir.AluOpType.is_gt,
        )
        nc.vector.tensor_mul(dis[:], dis[:], tmp[:])
        nc.vector.tensor_scalar(
            out=ndis[:], in0=dis[:], scalar1=-1.0, scalar2=None,
            op0=mybir.AluOpType.mult,
        )
        nc.vector.tensor_mul(ndis2[:], dis[:], ndis[:])

        # ---- prologue: U_j = src @ V_j ; U1/U2 to HBM (j=1,2 scaled by dis);
        #      j=3 scaled -> blk  ----
        def prologue(src, cl, w, vcat, u_hbm, blk_hbm, from_sbuf=False):
            for k in range(T):
                if from_sbuf:
                    xc = src[:, k, :]
                else:
                    xc = xp.tile([P, cl], f32, tag="xc")
                    nc.sync.dma_start(xc[:], src[k * P : (k + 1) * P, :])
                tp = pst.tile([cl, P], f32, space="PSUM", tag="tp")
                nc.tensor.transpose(out=tp[:, :], in_=xc[:, :], identity=ident_s[:])
                xT = wp.tile([cl, P], f32, tag="xT")
                nc.vector.tensor_copy(xT[:], tp[:, :])
                upsum = psu.tile([P, 4, w], f32, space="PSUM", tag="upsum")
                nc.tensor.matmul(
                    out=upsum[:].rearrange("p a c -> p (a c)"),
                    lhsT=xT[:, :],
                    rhs=vcat[:].rearrange("p a c -> p (a c)"),
                    start=True, stop=True,
                )
                ut = wp.tile([P, 3, w], f32, tag="ut")
                # fold bias into U_0 once (y = U0 + b - dis*(A S1))
                bb = b1s if w == HID else b2s
                nc.vector.tensor_add(ut[:, 0, :], upsum[:, 0, :], bb[:, :w])
                for j in (1, 2):
                    nc.vector.tensor_scalar(
                        out=ut[:, j, :], in0=upsum[:, j, :],
                        scalar1=dis[:, k : k + 1], scalar2=None,
                        op0=mybir.AluOpType.mult,
                    )
                nc.sync.dma_start(
                    u_hbm[:, k * P : (k + 1) * P, :].rearrange("a p c -> p a c"),
                    ut[:],
                )
                bt = wp.tile([P, w], bf16, tag="bt")
                nc.vector.tensor_scalar(
                    out=bt[:], in0=upsum[:, 3, :],
                    scalar1=dis[:, k : k + 1], scalar2=None,
                    op0=mybir.AluOpType.mult,
                )
                nc.sync.dma_start(blk_hbm[k * P : (k + 1) * P, :w], bt[:])

        # ---- one hop: table = AG(blk_in) split per bucket (overlaps the
        # gather passes); acc = scale*(A_gather) + U_j.  Edge stream is
        # (bucket, tile)-major with exact-max cells: chunks may straddle
        # tile boundaries, one matmul segment per (tile, chunk).  On the
        # last bucket pass, finished acc quarters are cast+written to
        # blk_out so the next hop's AllGather pieces can start early.
        def hop(l, j, blk_in, table, w, blk_out):
            for q in range(NB):
                nc.gpsimd.collective_compute(
                    "AllGather",
                    mybir.AluOpType.bypass,
                    replica_groups=[list(range(m))],
                    ins=[blk_in[QS[q] : QS[q] + QR[q], :].opt()],
                    outs=[table[8 * QS[q] : 8 * (QS[q] + QR[q]), :].opt()],
                )
            final = j == 0
            u_hbm = u1 if l == 1 else u2
            acc = accp.tile([P, T, w], f32, tag="acc")

            gath = None
            oneh = None
            o = 0           # segment ordinal (one-hot column)
            gcall = 0       # gather-call counter (round-robin SWDGE queues:
                            # each queue runs on its own Q7 cpu pair)
            # pass order: bucket-major (all of bucket bk across every tile
            # group) so CC AllGathers land in the gather lulls at hop
            # boundaries; quarter g of acc is written out as soon as its
            # bucket-3 batches finish.
            for bk in range(NB):
                gt0 = 0
                for g in range(NB):
                    gtn = QT[g]
                    tab_lo = 8 * QS[bk]
                    tab_hi = 8 * (QS[bk] + QR[bk])
                    cb = CHB[(g, bk)]
                    nloc = CHN[(g, bk)]
                    sched_p = SCHED[(g, bk)]
                    gnext = 0   # next ungathered pass-local chunk
                    gbase = 0   # current gather batch start (pass-local)
                    nbatch = (gtn + PB - 1) // PB
                    for bi in range(nbatch):
                        t0 = gt0 + bi * PB
                        tcnt = min(PB, gt0 + gtn - t0)
                        psum = psa.tile(
                            [P, PB, w], f32, space="PSUM", tag="apsum"
                        )
                        for tt in range(tcnt):
                            ti = t0 + tt
                            rng = sched_p[ti]
                            if rng is None:
                                nc.vector.memset(psum[:, tt, :], 0.0)
                                continue
                            c0l, c1l = rng
                            for k, cl in enumerate(range(c0l, c1l + 1)):
                                if cl >= gnext:
                                    glen = min(GCH, nloc - gnext)
                                    gbase = gnext
                                    cg = cb + gnext
                                    gath = gp.tile(
                                        [P, GCH, CWB], bf16, tag="gath"
                                    )
                                    nc.gpsimd.dma_gather(
                                        out_ap=gath[:, :glen, :],
                                        in_ap=table[tab_lo:tab_hi, :],
                                        idxs_ap=idx_s[
                                            :, cg * 8 : (cg + glen) * 8
                                        ],
                                        num_idxs=glen * P,
                                        num_idxs_reg=glen * P,
                                        elem_size=CWB,
                                        queue_num=gcall % 4,
                                    )
                                    gcall += 1
                                    gnext += glen
                                gpos = cl - gbase
                                oc, opos = divmod(o, OB)
                                if opos == 0:
                                    olen = min(OB, NSEG - oc * OB)
                                    oneh = op.tile(
                                        [P, OB, P], bf16, tag="oneh"
                                    )
                                    nc.vector.tensor_tensor(
                                        out=oneh[:, :olen, :],
                                        in0=iota_s[:].rearrange(
                                            "p (a q) -> p a q", q=P
                                        )[:, :olen, :],
                                        in1=seg_s[:, oc * OB : oc * OB + olen]
                                        .to_broadcast([P, olen, P]),
                                        op=mybir.AluOpType.is_equal,
                                    )
                                nc.tensor.matmul(
                                    out=psum[:, tt, :],
                                    lhsT=oneh[:, opos, :],
                                    rhs=gath[:, gpos, :w],
                                    start=(k == 0),
                                    stop=(k == c1l - c0l),
                                )
                                o += 1
                        # fold this pass's aggregation into acc
                        sl = slice(t0, t0 + tcnt)
                        if bk == 0:
                            nc.vector.tensor_copy(
                                acc[:, sl, :], psum[:, :tcnt, :]
                            )
                        else:
                            nc.vector.tensor_add(
                                acc[:, sl, :], acc[:, sl, :],
                                psum[:, :tcnt, :],
                            )
                        if bk == NB - 1:
                            scale = ndis if final else ndis2
                            nc.vector.tensor_tensor(
                                out=acc[:, sl, :],
                                in0=acc[:, sl, :],
                                in1=scale[:, sl].to_broadcast([P, tcnt, w]),
                                op=mybir.AluOpType.mult,
                            )
                            uti = up.tile([P, PB, w], f32, tag="uti")
                            nc.sync.dma_start(
                                uti[:, :tcnt, :],
                                u_hbm[
                                    j if not final else 0,
                                    t0 * P : (t0 + tcnt) * P, :,
                                ].rearrange("(a p) c -> p a c", p=P),
                            )
                            nc.vector.tensor_add(
                                acc[:, sl, :], acc[:, sl, :],
                                uti[:, :tcnt, :],
                            )
                            if final and l == 1:
                                nc.scalar.activation(
                                    acc[:, sl, :],
                                    acc[:, sl, :],
                                    mybir.ActivationFunctionType.Relu,
                                )
                    # acc quarter g final on the last bucket pass: cast +
                    # write to blk so the next hop's AG piece g starts early
                    if bk == NB - 1 and blk_out is not None:
                        ab = wp.tile([P, gtn, w], bf16, tag="accb")
                        nc.vector.tensor_copy(
                            ab[:], acc[:, gt0 : gt0 + gtn, :]
                        )
                        nc.sync.dma_start(
                            blk_out[QS[g] : QS[g] + QR[g], :w].rearrange(
                                "(a p) c -> p a c", p=P
                            ),
                            ab[:],
                        )
                    gt0 += gtn
            return acc

        def acc_to_hbm(acc, dst, w):
            nc.sync.dma_start(
                dst[:, :].rearrange("(a p) c -> p a c", p=P), acc[:, :, :w]
            )

        # ================= layer 1 =================
        prologue(x_t, CIN, HID, v1, u1, blks[(1, 3)])
        acc = hop(1, 2, blks[(1, 3)], tabs[(1, 3)], HID, blks[(1, 2)])
        acc = hop(1, 1, blks[(1, 2)], tabs[(1, 2)], HID, blks[(1, 1)])
        acc = hop(1, 0, blks[(1, 1)], tabs[(1, 1)], HID, None)

        # ================= layer 2 (prologue reads h=acc from SBUF) ======
        prologue(acc, HID, OUT, v2, u2, blks[(2, 3)], from_sbuf=True)
        acc = hop(2, 2, blks[(2, 3)], tabs[(2, 3)], OUT, blks[(2, 2)])
        acc = hop(2, 1, blks[(2, 2)], tabs[(2, 2)], OUT, blks[(2, 1)])
        acc = hop(2, 0, blks[(2, 1)], tabs[(2, 1)], OUT, None)

        # ---- log_softmax over acc [P, T, OUT] (in place) ----
        red = wp.tile([P, T], f32, tag="red")
        nc.vector.tensor_reduce(
            out=red[:], in_=acc[:, :, :], axis=mybir.AxisListType.X,
            op=mybir.AluOpType.max,
        )
        nc.vector.tensor_tensor(
            out=acc[:], in0=acc[:, :, :], in1=red[:].to_broadcast([P, T, OUT]),
            op=mybir.AluOpType.subtract,
        )
        ex = ep.tile([P, T, OUT], f32, tag="ex")
        nc.scalar.activation(ex[:], acc[:], mybir.ActivationFunctionType.Exp)
        nc.vector.tensor_reduce(
            out=red[:], in_=ex[:, :, :], axis=mybir.AxisListType.X,
            op=mybir.AluOpType.add,
        )
        nc.scalar.activation(red[:], red[:], mybir.ActivationFunctionType.Ln)
        nc.vector.tensor_tensor(
            out=acc[:], in0=acc[:, :, :], in1=red[:].to_broadcast([P, T, OUT]),
            op=mybir.AluOpType.subtract,
        )
        nc.sync.dma_start(
            y_t[:, :].rearrange("(a p) c -> p a c", p=P), acc[:, :, :]
        )

    nc.compile()
    return nc


def make_in_maps(cfg: Cfg, inputs: dict, idx_all, seg_all, degt_all):
    x = np.asarray(inputs["x"], dtype=np.float32)
    maps = []
    for c in range(cfg.m):
        xb = np.zeros((cfg.bp, cfg.c_in), dtype=np.float32)
        xb[cfg.pos_of[c][: cfg.b]] = x[c * cfg.b : (c + 1) * cfg.b]
        maps.append(
            {
                "x_blk": xb,
                "W1": np.asarray(inputs["W1"], dtype=np.float32),
                "b1": np.asarray(inputs["b1"], dtype=np.float32),
                "W2": np.asarray(inputs["W2"], dtype=np.float32),
                "b2": np.asarray(inputs["b2"], dtype=np.float32),
                "deg_t": degt_all[c],
                "idx": idx_all[c],
                "seg": seg_all[c],
            }
        )
    return maps


def kernel(**inputs) -> np.ndarray:
    from concourse import bass_utils

    cfg = Cfg(n=100000, m=8)
    cfg.finish()
    edge_index = np.asarray(inputs["edge_index"])
    idx_all, seg_all, degt_all = preprocess(edge_index, cfg)
    nc = build_program(cfg)
    in_maps = make_in_maps(cfg, inputs, idx_all, seg_all, degt_all)
    res = bass_utils.run_bass_kernel_spmd(nc, in_maps, core_ids=list(range(cfg.m)))
    out = np.concatenate(
        [res.results[c]["y_blk"][cfg.pos_of[c][: cfg.b]] for c in range(cfg.m)],
        axis=0,
    )
    return out.astype(np.float32)



# revision 55
# speedup vs baseline: 1.3389x; 1.0014x over previous
"""ChebNet (K=4, two layers, log_softmax) on 8 Trainium2 NeuronCores.

Self-contained: takes FULL inputs, shards by destination node internally,
runs a single SPMD Bass kernel on cores 0-7 (with ncfw AllGather between
propagation hops), returns the FULL [N, 32] output.

Math (Horner re-association so propagations happen at output width):
    y = sum_k T_k(L) x @ W[k] + b,  T_k Chebyshev,  L = -D^-1/2 A D^-1/2
      = U0 + L(U1 + L(U2 + L U3)),  U_j = x @ V_j
    V0 = W0 - W2, V1 = W1 - 3 W3, V2 = 2 W2, V3 = 4 W3
    L S = -dis * A (dis * S)   with dis = where(deg>0, 1/sqrt(max(deg,1)), 0)
Scaled-space recurrence (tables hold S_hat = dis * S):
    S_hat3 = dis*U3;  S_hat_{j} = dis*U_j - dis^2 * (A S_hat_{j+1})
    final: y = U_0 - dis * (A S_hat_1) + b
Per hop, A S_hat is an edge-gather + segment-sum done as:
    indirect-DMA row gather (dest-sorted edge stream) ->
    one-hot (seg == iota) built on VectorE ->
    TensorE matmul accumulation into PSUM per 128-dest tile.
"""

import sys

if "/opt/trn_rl_repo" not in sys.path:
    sys.path.insert(0, "/opt/trn_rl_repo")

import math
from contextlib import ExitStack
from dataclasses import dataclass, field

import numpy as np

P = 128
PADIDX = 1 << 20


@dataclass
class Cfg:
    n: int            # nodes
    m: int            # cores
    c_in: int = 128
    c_hid: int = 64
    c_out: int = 32
    cw: int = 64      # gather-table row width (f32) for both layers
    gch: int = 8      # chunks per gather call (>8 overflows the SWDGE
                      # descriptor ring and hangs the device)
    ob: int = 16      # one-hot columns per build batch
    pb: int = 8       # psum tiles per writeout batch
    # filled by finish/preprocess
    b: int = 0        # dests per core
    t: int = 0        # 128-dest tiles per core
    bp: int = 0       # padded block rows = t*128
    nbuck: int = 4    # source buckets = node-block quarters
    qtiles: list = field(default_factory=list)  # tiles per quarter
    qrows: list = field(default_factory=list)   # rows per quarter (tile-aligned)
    qstart: list = field(default_factory=list)  # local row start per quarter
    nch: int = 0      # total chunks in edge stream
    nseg: int = 0     # total (tile,chunk) matmul segments
    chb: list = field(default_factory=list)     # global chunk base per bucket
    chn: list = field(default_factory=list)     # chunks per bucket
    sched: list = field(default_factory=list)   # [nb][t] = (c0,c1) local or None

    def finish(self):
        assert self.n % self.m == 0
        self.b = self.n // self.m
        self.t = (self.b + P - 1) // P
        self.bp = self.t * P
        # tile-aligned quarters of the node block; bucket q = all cores'
        # quarter-q rows (int16 gather range: 8*qrows <= 32767)
        tq = (self.t + 3) // 4
        self.qtiles = [tq, tq, tq, self.t - 3 * tq]
        self.qrows = [q * P for q in self.qtiles]
        self.qstart = [0]
        for q in self.qrows[:-1]:
            self.qstart.append(self.qstart[-1] + q)
        assert all(8 * q <= 32767 for q in self.qrows)


def preprocess(edge_index: np.ndarray, cfg: Cfg):
    """Shard edges by destination row-block; per core build a
    (bucket, tile)-major edge stream.  Cells (bucket x dest-tile) are
    padded to the cross-core max count (NOT 128-aligned), so the chunk
    schedule is shared across cores while chunks may straddle dest-tile
    boundaries; straddled chunks get one matmul segment per tile.
    Buckets are node-block quarters interleaved across cores so each
    bucket's table rows come from one AllGather piece."""
    import ml_dtypes

    row = np.asarray(edge_index[0], dtype=np.int64)
    col = np.asarray(edge_index[1], dtype=np.int64)
    n, m, b, t = cfg.n, cfg.m, cfg.b, cfg.t
    nb = cfg.nbuck
    qstart = np.array(cfg.qstart + [cfg.bp], dtype=np.int64)
    qrows = np.array(cfg.qrows, dtype=np.int64)

    deg = np.bincount(row, minlength=n).astype(np.float32)
    gt0 = np.concatenate([[0], np.cumsum(cfg.qtiles)]).astype(np.int64)

    # --- balance permutation: per core, permute dests WITHIN their node
    # quarter so each (src-quarter, dest-tile) cell count flattens to the
    # mean; the cross-core max then carries far less padding.  A node's
    # quarter is invariant, so other cores' source-row quarters are
    # unaffected (no cross-core coupling).
    raw = []
    pos_of = []
    for c in range(m):
        sel = (row >= c * b) & (row < (c + 1) * b)
        d = (row[sel] - c * b).astype(np.int64)
        s = col[sel].astype(np.int64)
        raw.append((d, s))
        srcq = np.searchsorted(qstart, s % b, side="right") - 1
        v = np.zeros((cfg.bp, nb), dtype=np.int64)
        np.add.at(v, (d, srcq), 1)
        pos = np.empty(cfg.bp, dtype=np.int64)
        for g in range(nb):
            lo = int(qstart[g])
            hi = min(int(qstart[g + 1]), b)
            nt = cfg.qtiles[g]
            cap = 128
            dq = np.arange(lo, int(qstart[g + 1]))      # slots incl pad
            real = dq[dq < hi]
            vq = v[real]                                 # [nreal, nb]
            orderq = np.argsort(-vq.sum(1), kind="stable")
            load = np.zeros((nt, nb), dtype=np.int64)
            cnt = np.zeros(nt, dtype=np.int64)
            slot_in_tile = np.zeros(len(real), dtype=np.int64)
            tile_pick = np.zeros(len(real), dtype=np.int64)
            for di in orderq:
                sc_ = np.where(
                    cnt < cap, (load + vq[di]).max(1), 1 << 40
                )
                tt = int(np.argmin(sc_))
                tile_pick[di] = tt
                slot_in_tile[di] = cnt[tt]
                load[tt] += vq[di]
                cnt[tt] += 1
            # pad dests fill remaining slots (any order)
            pos[real] = lo + tile_pick * cap + slot_in_tile
        pos_of.append(pos)
    cfg.pos_of = pos_of

    per_core = []
    counts = np.zeros((m, nb, t), dtype=np.int64)
    for c in range(m):
        d_orig, s = raw[c]
        d = pos_of[c][d_orig]
        sc, lr = s // b, s % b
        # source permuted position within its owner core
        spos = np.stack(pos_of)[sc, lr]
        q = np.searchsorted(qstart, spos, side="right") - 1
        lidx = sc * qrows[q] + (spos - qstart[q])   # int16-safe bucket row
        tile_id = d >> 7
        key = q * t + tile_id
        order = np.argsort(key, kind="stable")
        d, lidx, key = d[order], lidx[order], key[order]
        np.add.at(counts[c], (q[order], tile_id[order]), 1)
        per_core.append((d, lidx, key))

    M = counts.max(axis=0)                        # [nb, t] cell sizes
    # stream layout: (tile-group, bucket)-major passes; cells back to
    # back within a pass, pass tail 128-padded.  Group g = the dest
    # tiles of node-block quarter g, so acc quarter g is final after
    # its 4 bucket passes (early blk writeout -> early next-hop AG).
    gt0 = np.concatenate([[0], np.cumsum(cfg.qtiles)]).astype(np.int64)
    cell_start = np.zeros((nb, t), dtype=np.int64)
    cfg.chb, cfg.chn, cfg.sched = {}, {}, {}
    pass_slot_base = {}
    base = 0
    for g in range(nb):
        for bk in range(nb):
            pos = 0
            sched_p = {}
            for ti in range(int(gt0[g]), int(gt0[g + 1])):
                cell_start[bk, ti] = pos
                if M[bk, ti] == 0:
                    sched_p[ti] = None
                    continue
                c0, c1 = pos // P, (pos + M[bk, ti] - 1) // P
                sched_p[ti] = (int(c0), int(c1))
                pos += M[bk, ti]
            nloc = (pos + P - 1) // P
            cfg.chb[(g, bk)] = base // P
            cfg.chn[(g, bk)] = int(nloc)
            cfg.sched[(g, bk)] = sched_p
            pass_slot_base[(g, bk)] = base
            base += nloc * P
    S = base
    cfg.nch = S // P
    # one-hot segment order must match kernel consumption: bucket-major
    nseg = 0
    seg_tile, seg_chunk = [], []                  # per segment (global chunk)
    for bk in range(nb):
        for g in range(nb):
            cb = cfg.chb[(g, bk)]
            for ti in range(int(gt0[g]), int(gt0[g + 1])):
                rng = cfg.sched[(g, bk)][ti]
                if rng is None:
                    continue
                c0, c1 = rng
                for cl in range(c0, c1 + 1):
                    seg_tile.append(ti)
                    seg_chunk.append(cb + cl)
                nseg += c1 - c0 + 1
    cfg.nseg = nseg
    seg_tile = np.array(seg_tile, dtype=np.int64)
    seg_chunk = np.array(seg_chunk, dtype=np.int64)

    pass_base = np.zeros((nb, t), dtype=np.int64)
    for g in range(nb):
        for bk in range(nb):
            pass_base[bk, int(gt0[g]) : int(gt0[g + 1])] = pass_slot_base[(g, bk)]

    idx_all, seg_all, degt_all = [], [], []
    for c in range(m):
        d, lidx, key = per_core[c]
        idx = np.zeros(S, dtype=np.int16)          # pad -> bucket row 0
        stile = np.full(S, -2, dtype=np.int64)     # slot -> dest tile
        sval = np.zeros(S, dtype=np.float32)       # slot -> dest & 127
        cnt_flat = counts[c].reshape(-1)
        pos_in_cell = np.arange(d.size) - np.concatenate(
            [[0], np.cumsum(cnt_flat)]
        )[key]
        slot = (
            pass_base.reshape(-1)[key]
            + cell_start.reshape(-1)[key]
            + pos_in_cell
        )
        idx[slot] = lidx.astype(np.int16)
        stile[slot] = key % t
        sval[slot] = (d & 127).astype(np.float32)
        # segment columns: chunk slots where the edge belongs to this
        # segment's tile; everything else -1
        ch_t = stile.reshape(cfg.nch, P)[seg_chunk]      # [nseg, P]
        ch_v = sval.reshape(cfg.nch, P)[seg_chunk]
        segm = np.where(ch_t == seg_tile[:, None], ch_v, -1.0)
        seg_all.append(
            np.ascontiguousarray(segm.T).astype(ml_dtypes.bfloat16)
        )
        # dma_gather idx layout: slot i -> partition i%16 col i//16,
        # replicated across the 8 groups of 16 partitions
        idx16 = idx.reshape(S // 16, 16).T       # [16, S/16]
        idx16 = np.tile(idx16, (8, 1))           # [128, S/16]
        idx_all.append(np.ascontiguousarray(idx16))
        degb = np.zeros(cfg.bp, dtype=np.float32)
        degb[pos_of[c][:b]] = deg[c * b : (c + 1) * b]
        degt_all.append(np.ascontiguousarray(degb.reshape(t, P).T))  # [P, t]
    return idx_all, seg_all, degt_all


def build_program(cfg: Cfg):
    import concourse.bass as bass
    import concourse.tile as tile
    from concourse import bacc, mybir

    f32 = mybir.dt.float32
    bf16 = mybir.dt.bfloat16
    n, m, b, T, bp = cfg.n, cfg.m, cfg.b, cfg.t, cfg.bp
    CIN, HID, OUT, CW = cfg.c_in, cfg.c_hid, cfg.c_out, cfg.cw
    GCH, OB, PB = cfg.gch, cfg.ob, cfg.pb
    NCH, NB = cfg.nch, cfg.nbuck
    NSEG, CHB, CHN, SCHED = cfg.nseg, cfg.chb, cfg.chn, cfg.sched
    QT, QR, QS = cfg.qtiles, cfg.qrows, cfg.qstart
    NTAB = m * bp

    nc = bacc.Bacc(
        "TRN2",
        target_bir_lowering=False,
        debug=False,
        num_devices=m,
        num_swdge_queues=4,
    )

    # ---- I/O ----
    x_t = nc.dram_tensor("x_blk", [bp, CIN], f32, kind="ExternalInput")
    w1_t = nc.dram_tensor("W1", [4, CIN, HID], f32, kind="ExternalInput")
    b1_t = nc.dram_tensor("b1", [HID], f32, kind="ExternalInput")
    w2_t = nc.dram_tensor("W2", [4, HID, OUT], f32, kind="ExternalInput")
    b2_t = nc.dram_tensor("b2", [OUT], f32, kind="ExternalInput")
    degt_t = nc.dram_tensor("deg_t", [P, T], f32, kind="ExternalInput")
    idx_t = nc.dram_tensor(
        "idx", [P, NCH * 8], mybir.dt.int16, kind="ExternalInput"
    )
    seg_t = nc.dram_tensor("seg", [P, NSEG], mybir.dt.bfloat16, kind="ExternalInput")
    y_t = nc.dram_tensor("y_blk", [bp, OUT], f32, kind="ExternalOutput")

    # ---- internal DRAM ----
    u1 = nc.dram_tensor("U1", [3, bp, HID], f32)
    u2 = nc.dram_tensor("U2", [3, bp, OUT], f32)
    # bf16 gather rows must be 256B -> 128 bf16 wide; only cols :CW are real.
    CWB = 128
    blks, tabs = {}, {}
    for l in (1, 2):
        for j in (3, 2, 1):
            blks[(l, j)] = nc.dram_tensor(f"blk_{l}_{j}", [bp, CWB], bf16)
            tabs[(l, j)] = nc.dram_tensor(f"tab_{l}_{j}", [NTAB, CWB], bf16)

    import ml_dtypes

    iota_np = np.broadcast_to(
        np.tile(np.arange(P, dtype=np.float32), OB), (P, OB * P)
    ).astype(ml_dtypes.bfloat16)
    iota_d = nc.inline_tensor(iota_np, name="iota_rep")
    ident_d = nc.inline_tensor(np.eye(P, dtype=np.float32), name="ident")

    with ExitStack() as ctx:
        tc = ctx.enter_context(tile.TileContext(nc, num_cores=m))
        const = ctx.enter_context(tc.tile_pool(name="const", bufs=1))
        xp = ctx.enter_context(tc.tile_pool(name="xp", bufs=3))
        wp = ctx.enter_context(tc.tile_pool(name="wp", bufs=3))
        up = ctx.enter_context(tc.tile_pool(name="up", bufs=3))
        gp = ctx.enter_context(tc.tile_pool(name="gp", bufs=28))
        op = ctx.enter_context(tc.tile_pool(name="op", bufs=6))
        accp = ctx.enter_context(tc.tile_pool(name="accp", bufs=2))
        ep = ctx.enter_context(tc.tile_pool(name="ep", bufs=1))
        pst = ctx.enter_context(tc.tile_pool(name="pst", bufs=2, space="PSUM"))
        psu = ctx.enter_context(tc.tile_pool(name="psu", bufs=2, space="PSUM"))
        psa = ctx.enter_context(tc.tile_pool(name="psa", bufs=4, space="PSUM"))

        # ---- constants ----
        iota_s = const.tile([P, OB * P], bf16)
        nc.sync.dma_start(iota_s[:], iota_d[:, :])
        ident_s = const.tile([P, P], f32)
        nc.sync.dma_start(ident_s[:], ident_d[:, :])

        seg_s = const.tile([P, NSEG], bf16)
        nc.sync.dma_start(seg_s[:], seg_t[:, :])

        # whole gather-index stream resident in SBUF (no per-call DMAs)
        idx_s = const.tile([P, NCH * 8], mybir.dt.int16)
        nc.sync.dma_start(idx_s[:], idx_t[:, :])

        # V1cat [CIN, 4, HID], V2cat [HID, 4, OUT]
        w1s = const.tile([CIN, 4, HID], f32)
        nc.sync.dma_start(w1s[:], w1_t[:, :, :].rearrange("k p c -> p k c"))
        v1 = const.tile([CIN, 4, HID], f32)
        nc.vector.tensor_sub(v1[:, 0, :], w1s[:, 0, :], w1s[:, 2, :])
        nc.vector.tensor_scalar(
            out=v1[:, 1, :], in0=w1s[:, 3, :], scalar1=-3.0, scalar2=None,
            op0=mybir.AluOpType.mult,
        )
        nc.vector.tensor_add(v1[:, 1, :], v1[:, 1, :], w1s[:, 1, :])
        nc.vector.tensor_scalar(
            out=v1[:, 2, :], in0=w1s[:, 2, :], scalar1=2.0, scalar2=None,
            op0=mybir.AluOpType.mult,
        )
        nc.vector.tensor_scalar(
            out=v1[:, 3, :], in0=w1s[:, 3, :], scalar1=4.0, scalar2=None,
            op0=mybir.AluOpType.mult,
        )

        w2s = const.tile([HID, 4, OUT], f32)
        nc.sync.dma_start(w2s[:], w2_t[:, :, :].rearrange("k p c -> p k c"))
        v2 = const.tile([HID, 4, OUT], f32)
        nc.vector.tensor_sub(v2[:, 0, :], w2s[:, 0, :], w2s[:, 2, :])
        nc.vector.tensor_scalar(
            out=v2[:, 1, :], in0=w2s[:, 3, :], scalar1=-3.0, scalar2=None,
            op0=mybir.AluOpType.mult,
        )
        nc.vector.tensor_add(v2[:, 1, :], v2[:, 1, :], w2s[:, 1, :])
        nc.vector.tensor_scalar(
            out=v2[:, 2, :], in0=w2s[:, 2, :], scalar1=2.0, scalar2=None,
            op0=mybir.AluOpType.mult,
        )
        nc.vector.tensor_scalar(
            out=v2[:, 3, :], in0=w2s[:, 3, :], scalar1=4.0, scalar2=None,
            op0=mybir.AluOpType.mult,
        )

        b1s = const.tile([P, HID], f32)
        nc.sync.dma_start(b1s[:1, :], b1_t[:].rearrange("(o c) -> o c", o=1))
        nc.gpsimd.partition_broadcast(b1s[:, :], b1s[:1, :])
        b2s = const.tile([P, OUT], f32)
        nc.sync.dma_start(b2s[:1, :], b2_t[:].rearrange("(o c) -> o c", o=1))
        nc.gpsimd.partition_broadcast(b2s[:, :], b2s[:1, :])

        # DMA-completion semaphores for prepare_only gathers (one per queue)
        dma_sems = [nc.alloc_semaphore(f"swdge_dma{q}") for q in range(4)]

        # ---- dis, -dis, -dis^2 in [P, T] (partition p, slot t) = dest 128t+p ----
        degs = const.tile([P, T], f32)
        nc.sync.dma_start(degs[:], degt_t[:, :])
        dis = const.tile([P, T], f32)
        ndis = const.tile([P, T], f32)
        ndis2 = const.tile([P, T], f32)
        tmp = const.tile([P, T], f32)
        nc.vector.tensor_scalar(
            out=tmp[:], in0=degs[:], scalar1=1.0, scalar2=None,
            op0=mybir.AluOpType.max,
        )
        nc.scalar.activation(tmp[:], tmp[:], mybir.ActivationFunctionType.Sqrt)
        nc.vector.reciprocal(dis[:], tmp[:])
        nc.vector.tensor_scalar(
            out=tmp[:], in0=degs[:], scalar1=0.0, scalar2=None,
            op0=mybir.AluOpType.is_gt,
        )
        nc.vector.tensor_mul(dis[:], dis[:], tmp[:])
        nc.vector.tensor_scalar(
            out=ndis[:], in0=dis[:], scalar1=-1.0, scalar2=None,
            op0=mybir.AluOpType.mult,
        )
        nc.vector.tensor_mul(ndis2[:], dis[:], ndis[:])

        # ---- prologue: U_j = src @ V_j ; U1/U2 to HBM (j=1,2 scaled by dis);
        #      j=3 scaled -> blk  ----
        def prologue(src, cl, w, vcat, u_hbm, blk_hbm, from_sbuf=False):
            for k in range(T):
                if from_sbuf:
                    xc = src[:, k, :]
                else:
                    xc = xp.tile([P, cl], f32, tag="xc")
                    nc.sync.dma_start(xc[:], src[k * P : (k + 1) * P, :])
                tp = pst.tile([cl, P], f32, space="PSUM", tag="tp")
                nc.tensor.transpose(out=tp[:, :], in_=xc[:, :], identity=ident_s[:])
                xT = wp.tile([cl, P], f32, tag="xT")
                nc.vector.tensor_copy(xT[:], tp[:, :])
                upsum = psu.tile([P, 4, w], f32, space="PSUM", tag="upsum")
                nc.tensor.matmul(
                    out=upsum[:].rearrange("p a c -> p (a c)"),
                    lhsT=xT[:, :],
                    rhs=vcat[:].rearrange("p a c -> p (a c)"),
                    start=True, stop=True,
                )
                ut = wp.tile([P, 3, w], f32, tag="ut")
                # fold bias into U_0 once (y = U0 + b - dis*(A S1))
                bb = b1s if w == HID else b2s
                nc.vector.tensor_add(ut[:, 0, :], upsum[:, 0, :], bb[:, :w])
                for j in (1, 2):
                    nc.vector.tensor_scalar(
                        out=ut[:, j, :], in0=upsum[:, j, :],
                        scalar1=dis[:, k : k + 1], scalar2=None,
                        op0=mybir.AluOpType.mult,
                    )
                nc.sync.dma_start(
                    u_hbm[:, k * P : (k + 1) * P, :].rearrange("a p c -> p a c"),
                    ut[:],
                )
                bt = wp.tile([P, w], bf16, tag="bt")
                nc.vector.tensor_scalar(
                    out=bt[:], in0=upsum[:, 3, :],
                    scalar1=dis[:, k : k + 1], scalar2=None,
                    op0=mybir.AluOpType.mult,
                )
                nc.sync.dma_start(blk_hbm[k * P : (k + 1) * P, :w], bt[:])

        # ---- one hop: table = AG(blk_in) split per bucket (overlaps the
        # gather passes); acc = scale*(A_gather) + U_j.  Edge stream is
        # (bucket, tile)-major with exact-max cells: chunks may straddle
        # tile boundaries, one matmul segment per (tile, chunk).  On the
        # last bucket pass, finished acc quarters are cast+written to
        # blk_out so the next hop's AllGather pieces can start early.
        def hop(l, j, blk_in, table, w, blk_out):
            for q in range(NB):
                nc.gpsimd.collective_compute(
                    "AllGather",
                    mybir.AluOpType.bypass,
                    replica_groups=[list(range(m))],
                    ins=[blk_in[QS[q] : QS[q] + QR[q], :].opt()],
                    outs=[table[8 * QS[q] : 8 * (QS[q] + QR[q]), :].opt()],
                )
            final = j == 0
            u_hbm = u1 if l == 1 else u2
            acc = accp.tile([P, T, w], f32, tag="acc")

            gath = None
            oneh = None
            o = 0           # segment ordinal (one-hot column)
            gcall = 0       # gather-call counter (round-robin SWDGE queues:
                            # each queue runs on its own Q7 cpu pair)
            # pass order: bucket-major (all of bucket bk across every tile
            # group) so CC AllGathers land in the gather lulls at hop
            # boundaries; quarter g of acc is written out as soon as its
            # bucket-3 batches finish.
            for bk in range(NB):
                gt0 = 0
                for g in range(NB):
                    gtn = QT[g]
                    tab_lo = 8 * QS[bk]
                    tab_hi = 8 * (QS[bk] + QR[bk])
                    cb = CHB[(g, bk)]
                    nloc = CHN[(g, bk)]
                    sched_p = SCHED[(g, bk)]
                    gnext = 0   # next ungathered pass-local chunk
                    gbase = 0   # current gather batch start (pass-local)
                    nbatch = (gtn + PB - 1) // PB
                    for bi in range(nbatch):
                        t0 = gt0 + bi * PB
                        tcnt = min(PB, gt0 + gtn - t0)
                        psum = psa.tile(
                            [P, PB, w], f32, space="PSUM", tag="apsum"
                        )
                        for tt in range(tcnt):
                            ti = t0 + tt
                            rng = sched_p[ti]
                            if rng is None:
                                nc.vector.memset(psum[:, tt, :], 0.0)
                                continue
                            c0l, c1l = rng
                            for k, cl in enumerate(range(c0l, c1l + 1)):
                                if cl >= gnext:
                                    glen = min(GCH, nloc - gnext)
                                    gbase = gnext
                                    cg = cb + gnext
                                    gath = gp.tile(
                                        [P, GCH, CWB], bf16, tag="gath"
                                    )
                                    nc.gpsimd.dma_gather(
                                        out_ap=gath[:, :glen, :],
                                        in_ap=table[tab_lo:tab_hi, :],
                                        idxs_ap=idx_s[
                                            :, cg * 8 : (cg + glen) * 8
                                        ],
                                        num_idxs=glen * P,
                                        num_idxs_reg=glen * P,
                                        elem_size=CWB,
                                        queue_num=gcall % 4,
                                    )
                                    gcall += 1
                                    gnext += glen
                                gpos = cl - gbase
                                oc, opos = divmod(o, OB)
                                if opos == 0:
                                    olen = min(OB, NSEG - oc * OB)
                                    oneh = op.tile(
                                        [P, OB, P], bf16, tag="oneh"
                                    )
                                    nc.vector.tensor_tensor(
                                        out=oneh[:, :olen, :],
                                        in0=iota_s[:].rearrange(
                                            "p (a q) -> p a q", q=P
                                        )[:, :olen, :],
                                        in1=seg_s[:, oc * OB : oc * OB + olen]
                                        .to_broadcast([P, olen, P]),
                                        op=mybir.AluOpType.is_equal,
                                    )
                                nc.tensor.matmul(
                                    out=psum[:, tt, :],
                                    lhsT=oneh[:, opos, :],
                                    rhs=gath[:, gpos, :w],
                                    start=(k == 0),
                                    stop=(k == c1l - c0l),
                                )
                                o += 1
                        # fold this pass's aggregation into acc
                        sl = slice(t0, t0 + tcnt)
                        if bk == 0:
                            nc.vector.tensor_copy(
                                acc[:, sl, :], psum[:, :tcnt, :]
                            )
                        else:
                            nc.vector.tensor_add(
                                acc[:, sl, :], acc[:, sl, :],
                                psum[:, :tcnt, :],
                            )
                        if bk == NB - 1:
                            scale = ndis if final else ndis2
                            nc.vector.tensor_tensor(
                                out=acc[:, sl, :],
                                in0=acc[:, sl, :],
                                in1=scale[:, sl].to_broadcast([P, tcnt, w]),
                                op=mybir.AluOpType.mult,
                            )
                            uti = up.tile([P, PB, w], f32, tag="uti")
                            nc.sync.dma_start(
                                uti[:, :tcnt, :],
                                u_hbm[
                                    j if not final else 0,
                                    t0 * P : (t0 + tcnt) * P, :,
                                ].rearrange("(a p) c -> p a c", p=P),
                            )
                            nc.vector.tensor_add(
                                acc[:, sl, :], acc[:, sl, :],
                                uti[:, :tcnt, :],
                            )
                            if final and l == 1:
                                nc.scalar.activation(
                                    acc[:, sl, :],
                                    acc[:, sl, :],
                                    mybir.ActivationFunctionType.Relu,
                                )
                    # acc quarter g final on the last bucket pass: cast +
                    # write to blk so the next hop's AG piece g starts early
                    if bk == NB - 1 and blk_out is not None:
                        ab = wp.tile([P, gtn, w], bf16, tag="accb")
                        nc.vector.tensor_copy(
                            ab[:], acc[:, gt0 : gt0 + gtn, :]
                        )
                        nc.sync.dma_start(
                            blk_out[QS[g] : QS[g] + QR[g], :w].rearrange(
                                "(a p) c -> p a c", p=P
                            ),
                            ab[:],
                        )
                    gt0 += gtn
            return acc

        def acc_to_hbm(acc, dst, w):
            nc.sync.dma_start(
                dst[:, :].rearrange("(a p) c -> p a c", p=P), acc[:, :, :w]
            )

        # ================= layer 1 =================
        prologue(x_t, CIN, HID, v1, u1, blks[(1, 3)])
        acc = hop(1, 2, blks[(1, 3)], tabs[(1, 3)], HID, blks[(1, 2)])
        acc = hop(1, 1, blks[(1, 2)], tabs[(1, 2)], HID, blks[(1, 1)])
        acc = hop(1, 0, blks[(1, 1)], tabs[(1, 1)], HID, None)

        # ================= layer 2 (prologue reads h=acc from SBUF) ======
        prologue(acc, HID, OUT, v2, u2, blks[(2, 3)], from_sbuf=True)
        acc = hop(2, 2, blks[(2, 3)], tabs[(2, 3)], OUT, blks[(2, 2)])
        acc = hop(2, 1, blks[(2, 2)], tabs[(2, 2)], OUT, blks[(2, 1)])
        acc = hop(2, 0, blks[(2, 1)], tabs[(2, 1)], OUT, None)

        # ---- log_softmax over acc [P, T, OUT] (in place) ----
        red = wp.tile([P, T], f32, tag="red")
        nc.vector.tensor_reduce(
            out=red[:], in_=acc[:, :, :], axis=mybir.AxisListType.X,
            op=mybir.AluOpType.max,
        )
        nc.vector.tensor_tensor(
            out=acc[:], in0=acc[:, :, :], in1=red[:].to_broadcast([P, T, OUT]),
            op=mybir.AluOpType.subtract,
        )
        ex = ep.tile([P, T, OUT], f32, tag="ex")
        nc.scalar.activation(ex[:], acc[:], mybir.ActivationFunctionType.Exp)
        nc.vector.tensor_reduce(
            out=red[:], in_=ex[:, :, :], axis=mybir.AxisListType.X,
            op=mybir.AluOpType.add,
        )
        nc.scalar.activation(red[:], red[:], mybir.ActivationFunctionType.Ln)
        nc.vector.tensor_tensor(
            out=acc[:], in0=acc[:, :, :], in1=red[:].to_broadcast([P, T, OUT]),
            op=mybir.AluOpType.subtract,
        )
        nc.sync.dma_start(
            y_t[:, :].rearrange("(a p) c -> p a c", p=P), acc[:, :, :]
        )

    nc.compile()
    return nc


def make_in_maps(cfg: Cfg, inputs: dict, idx_all, seg_all, degt_all):
    x = np.asarray(inputs["x"], dtype=np.float32)
    maps = []
    for c in range(cfg.m):
        xb = np.zeros((cfg.bp, cfg.c_in), dtype=np.float32)
        xb[cfg.pos_of[c][: cfg.b]] = x[c * cfg.b : (c + 1) * cfg.b]
        maps.append(
            {
                "x_blk": xb,
                "W1": np.asarray(inputs["W1"], dtype=np.float32),
                "b1": np.asarray(inputs["b1"], dtype=np.float32),
                "W2": np.asarray(inputs["W2"], dtype=np.float32),
                "b2": np.asarray(inputs["b2"], dtype=np.float32),
                "deg_t": degt_all[c],
                "idx": idx_all[c],
                "seg": seg_all[c],
            }
        )
    return maps


def kernel(**inputs) -> np.ndarray:
    from concourse import bass_utils

    cfg = Cfg(n=100000, m=8)
    cfg.finish()
    edge_index = np.asarray(inputs["edge_index"])
    idx_all, seg_all, degt_all = preprocess(edge_index, cfg)
    nc = build_program(cfg)
    in_maps = make_in_maps(cfg, inputs, idx_all, seg_all, degt_all)
    res = bass_utils.run_bass_kernel_spmd(nc, in_maps, core_ids=list(range(cfg.m)))
    out = np.concatenate(
        [res.results[c]["y_blk"][cfg.pos_of[c][: cfg.b]] for c in range(cfg.m)],
        axis=0,
    )
    return out.astype(np.float32)



# revision 57
# speedup vs baseline: 1.3701x; 1.0233x over previous
"""ChebNet (K=4, two layers, log_softmax) on 8 Trainium2 NeuronCores.

Self-contained: takes FULL inputs, shards by destination node internally,
runs a single SPMD Bass kernel on cores 0-7 (with ncfw AllGather between
propagation hops), returns the FULL [N, 32] output.

Math (Horner re-association so propagations happen at output width):
    y = sum_k T_k(L) x @ W[k] + b,  T_k Chebyshev,  L = -D^-1/2 A D^-1/2
      = U0 + L(U1 + L(U2 + L U3)),  U_j = x @ V_j
    V0 = W0 - W2, V1 = W1 - 3 W3, V2 = 2 W2, V3 = 4 W3
    L S = -dis * A (dis * S)   with dis = where(deg>0, 1/sqrt(max(deg,1)), 0)
Scaled-space recurrence (tables hold S_hat = dis * S):
    S_hat3 = dis*U3;  S_hat_{j} = dis*U_j - dis^2 * (A S_hat_{j+1})
    final: y = U_0 - dis * (A S_hat_1) + b
Per hop, A S_hat is an edge-gather + segment-sum done as:
    indirect-DMA row gather (dest-sorted edge stream) ->
    one-hot (seg == iota) built on VectorE ->
    TensorE matmul accumulation into PSUM per 128-dest tile.
"""

import sys

if "/opt/trn_rl_repo" not in sys.path:
    sys.path.insert(0, "/opt/trn_rl_repo")

import math
from contextlib import ExitStack
from dataclasses import dataclass, field

import numpy as np

P = 128
PADIDX = 1 << 20


@dataclass
class Cfg:
    n: int            # nodes
    m: int            # cores
    c_in: int = 128
    c_hid: int = 64
    c_out: int = 32
    cw: int = 64      # gather-table row width (f32) for both layers
    gch: int = 8      # chunks per gather call (>8 overflows the SWDGE
                      # descriptor ring and hangs the device)
    ob: int = 16      # one-hot columns per build batch
    pb: int = 8       # psum tiles per writeout batch
    # filled by finish/preprocess
    b: int = 0        # dests per core
    t: int = 0        # 128-dest tiles per core
    bp: int = 0       # padded block rows = t*128
    nbuck: int = 4    # source buckets = node-block quarters
    qtiles: list = field(default_factory=list)  # tiles per quarter
    qrows: list = field(default_factory=list)   # rows per quarter (tile-aligned)
    qstart: list = field(default_factory=list)  # local row start per quarter
    nch: int = 0      # total chunks in edge stream
    nseg: int = 0     # total (tile,chunk) matmul segments
    chb: list = field(default_factory=list)     # global chunk base per bucket
    chn: list = field(default_factory=list)     # chunks per bucket
    sched: list = field(default_factory=list)   # [nb][t] = (c0,c1) local or None

    def finish(self):
        assert self.n % self.m == 0
        self.b = self.n // self.m
        self.t = (self.b + P - 1) // P
        self.bp = self.t * P
        # tile-aligned quarters of the node block; bucket q = all cores'
        # quarter-q rows (int16 gather range: 8*qrows <= 32767)
        tq = (self.t + 3) // 4
        self.qtiles = [tq, tq, tq, self.t - 3 * tq]
        self.qrows = [q * P for q in self.qtiles]
        self.qstart = [0]
        for q in self.qrows[:-1]:
            self.qstart.append(self.qstart[-1] + q)
        assert all(8 * q <= 32767 for q in self.qrows)


def preprocess(edge_index: np.ndarray, cfg: Cfg):
    """Shard edges by destination row-block; per core build a
    (bucket, tile)-major edge stream.  Cells (bucket x dest-tile) are
    padded to the cross-core max count (NOT 128-aligned), so the chunk
    schedule is shared across cores while chunks may straddle dest-tile
    boundaries; straddled chunks get one matmul segment per tile.
    Buckets are node-block quarters interleaved across cores so each
    bucket's table rows come from one AllGather piece."""
    import ml_dtypes

    row = np.asarray(edge_index[0], dtype=np.int64)
    col = np.asarray(edge_index[1], dtype=np.int64)
    n, m, b, t = cfg.n, cfg.m, cfg.b, cfg.t
    nb = cfg.nbuck
    qstart = np.array(cfg.qstart + [cfg.bp], dtype=np.int64)
    qrows = np.array(cfg.qrows, dtype=np.int64)

    deg = np.bincount(row, minlength=n).astype(np.float32)
    gt0 = np.concatenate([[0], np.cumsum(cfg.qtiles)]).astype(np.int64)

    # --- balance permutation: per core, permute dests WITHIN their node
    # quarter so each (src-quarter, dest-tile) cell count flattens to the
    # mean; the cross-core max then carries far less padding.  A node's
    # quarter is invariant, so other cores' source-row quarters are
    # unaffected (no cross-core coupling).
    raw = []
    pos_of = []
    for c in range(m):
        sel = (row >= c * b) & (row < (c + 1) * b)
        d = (row[sel] - c * b).astype(np.int64)
        s = col[sel].astype(np.int64)
        raw.append((d, s))
        srcq = np.searchsorted(qstart, s % b, side="right") - 1
        v = np.zeros((cfg.bp, nb), dtype=np.int64)
        np.add.at(v, (d, srcq), 1)
        pos = np.empty(cfg.bp, dtype=np.int64)
        for g in range(nb):
            lo = int(qstart[g])
            hi = min(int(qstart[g + 1]), b)
            nt = cfg.qtiles[g]
            cap = 128
            dq = np.arange(lo, int(qstart[g + 1]))      # slots incl pad
            real = dq[dq < hi]
            vq = v[real]                                 # [nreal, nb]
            orderq = np.argsort(-vq.sum(1), kind="stable")
            load = np.zeros((nt, nb), dtype=np.int64)
            cnt = np.zeros(nt, dtype=np.int64)
            slot_in_tile = np.zeros(len(real), dtype=np.int64)
            tile_pick = np.zeros(len(real), dtype=np.int64)
            for di in orderq:
                sc_ = np.where(
                    cnt < cap, (load + vq[di]).max(1), 1 << 40
                )
                tt = int(np.argmin(sc_))
                tile_pick[di] = tt
                slot_in_tile[di] = cnt[tt]
                load[tt] += vq[di]
                cnt[tt] += 1
            # pad dests fill remaining slots (any order)
            pos[real] = lo + tile_pick * cap + slot_in_tile
        pos_of.append(pos)
    cfg.pos_of = pos_of

    per_core = []
    counts = np.zeros((m, nb, t), dtype=np.int64)
    for c in range(m):
        d_orig, s = raw[c]
        d = pos_of[c][d_orig]
        sc, lr = s // b, s % b
        # source permuted position within its owner core
        spos = np.stack(pos_of)[sc, lr]
        q = np.searchsorted(qstart, spos, side="right") - 1
        lidx = sc * qrows[q] + (spos - qstart[q])   # int16-safe bucket row
        tile_id = d >> 7
        key = q * t + tile_id
        order = np.argsort(key, kind="stable")
        d, lidx, key = d[order], lidx[order], key[order]
        np.add.at(counts[c], (q[order], tile_id[order]), 1)
        per_core.append((d, lidx, key))

    M = counts.max(axis=0)                        # [nb, t] cell sizes
    # stream layout: (tile-group, bucket)-major passes; cells back to
    # back within a pass, pass tail 128-padded.  Group g = the dest
    # tiles of node-block quarter g, so acc quarter g is final after
    # its 4 bucket passes (early blk writeout -> early next-hop AG).
    gt0 = np.concatenate([[0], np.cumsum(cfg.qtiles)]).astype(np.int64)
    cell_start = np.zeros((nb, t), dtype=np.int64)
    cfg.chb, cfg.chn, cfg.sched = {}, {}, {}
    pass_slot_base = {}
    base = 0
    for g in range(nb):
        for bk in range(nb):
            pos = 0
            sched_p = {}
            for ti in range(int(gt0[g]), int(gt0[g + 1])):
                cell_start[bk, ti] = pos
                if M[bk, ti] == 0:
                    sched_p[ti] = None
                    continue
                c0, c1 = pos // P, (pos + M[bk, ti] - 1) // P
                sched_p[ti] = (int(c0), int(c1))
                pos += M[bk, ti]
            nloc = (pos + P - 1) // P
            cfg.chb[(g, bk)] = base // P
            cfg.chn[(g, bk)] = int(nloc)
            cfg.sched[(g, bk)] = sched_p
            pass_slot_base[(g, bk)] = base
            base += nloc * P
    S = base
    cfg.nch = S // P
    # one-hot segment order must match kernel consumption: bucket-major
    nseg = 0
    seg_tile, seg_chunk = [], []                  # per segment (global chunk)
    for bk in range(nb):
        for g in range(nb):
            cb = cfg.chb[(g, bk)]
            for ti in range(int(gt0[g]), int(gt0[g + 1])):
                rng = cfg.sched[(g, bk)][ti]
                if rng is None:
                    continue
                c0, c1 = rng
                for cl in range(c0, c1 + 1):
                    seg_tile.append(ti)
                    seg_chunk.append(cb + cl)
                nseg += c1 - c0 + 1
    cfg.nseg = nseg
    seg_tile = np.array(seg_tile, dtype=np.int64)
    seg_chunk = np.array(seg_chunk, dtype=np.int64)

    pass_base = np.zeros((nb, t), dtype=np.int64)
    for g in range(nb):
        for bk in range(nb):
            pass_base[bk, int(gt0[g]) : int(gt0[g + 1])] = pass_slot_base[(g, bk)]

    idx_all, seg_all, degt_all = [], [], []
    for c in range(m):
        d, lidx, key = per_core[c]
        idx = np.zeros(S, dtype=np.int16)          # pad -> bucket row 0
        stile = np.full(S, -2, dtype=np.int64)     # slot -> dest tile
        sval = np.zeros(S, dtype=np.float32)       # slot -> dest & 127
        cnt_flat = counts[c].reshape(-1)
        pos_in_cell = np.arange(d.size) - np.concatenate(
            [[0], np.cumsum(cnt_flat)]
        )[key]
        slot = (
            pass_base.reshape(-1)[key]
            + cell_start.reshape(-1)[key]
            + pos_in_cell
        )
        idx[slot] = lidx.astype(np.int16)
        stile[slot] = key % t
        sval[slot] = (d & 127).astype(np.float32)
        # segment columns: chunk slots where the edge belongs to this
        # segment's tile; everything else -1
        ch_t = stile.reshape(cfg.nch, P)[seg_chunk]      # [nseg, P]
        ch_v = sval.reshape(cfg.nch, P)[seg_chunk]
        segm = np.where(ch_t == seg_tile[:, None], ch_v, -1.0)
        seg_all.append(
            np.ascontiguousarray(segm.T).astype(ml_dtypes.bfloat16)
        )
        # dma_gather idx layout: slot i -> partition i%16 col i//16,
        # replicated across the 8 groups of 16 partitions
        idx16 = idx.reshape(S // 16, 16).T       # [16, S/16]
        idx16 = np.tile(idx16, (8, 1))           # [128, S/16]
        idx_all.append(np.ascontiguousarray(idx16))
        degb = np.zeros(cfg.bp, dtype=np.float32)
        degb[pos_of[c][:b]] = deg[c * b : (c + 1) * b]
        degt_all.append(np.ascontiguousarray(degb.reshape(t, P).T))  # [P, t]
    return idx_all, seg_all, degt_all


def build_program(cfg: Cfg):
    import concourse.bass as bass
    import concourse.tile as tile
    from concourse import bacc, mybir

    f32 = mybir.dt.float32
    bf16 = mybir.dt.bfloat16
    n, m, b, T, bp = cfg.n, cfg.m, cfg.b, cfg.t, cfg.bp
    CIN, HID, OUT, CW = cfg.c_in, cfg.c_hid, cfg.c_out, cfg.cw
    GCH, OB, PB = cfg.gch, cfg.ob, cfg.pb
    NCH, NB = cfg.nch, cfg.nbuck
    NSEG, CHB, CHN, SCHED = cfg.nseg, cfg.chb, cfg.chn, cfg.sched
    QT, QR, QS = cfg.qtiles, cfg.qrows, cfg.qstart
    NTAB = m * bp

    nc = bacc.Bacc(
        "TRN2",
        target_bir_lowering=False,
        debug=False,
        num_devices=m,
        num_swdge_queues=4,
    )

    # ---- I/O ----
    x_t = nc.dram_tensor("x_blk", [bp, CIN], f32, kind="ExternalInput")
    w1_t = nc.dram_tensor("W1", [4, CIN, HID], f32, kind="ExternalInput")
    b1_t = nc.dram_tensor("b1", [HID], f32, kind="ExternalInput")
    w2_t = nc.dram_tensor("W2", [4, HID, OUT], f32, kind="ExternalInput")
    b2_t = nc.dram_tensor("b2", [OUT], f32, kind="ExternalInput")
    degt_t = nc.dram_tensor("deg_t", [P, T], f32, kind="ExternalInput")
    idx_t = nc.dram_tensor(
        "idx", [P, NCH * 8], mybir.dt.int16, kind="ExternalInput"
    )
    seg_t = nc.dram_tensor("seg", [P, NSEG], mybir.dt.bfloat16, kind="ExternalInput")
    y_t = nc.dram_tensor("y_blk", [bp, OUT], f32, kind="ExternalOutput")

    # ---- internal DRAM ----
    u1 = nc.dram_tensor("U1", [3, bp, HID], f32)
    u2 = nc.dram_tensor("U2", [3, bp, OUT], f32)
    # bf16 gather rows must be 256B -> 128 bf16 wide; only cols :CW are real.
    CWB = 128
    blks, tabs = {}, {}
    for l in (1, 2):
        for j in (3, 2, 1):
            blks[(l, j)] = nc.dram_tensor(f"blk_{l}_{j}", [bp, CWB], bf16)
            tabs[(l, j)] = nc.dram_tensor(f"tab_{l}_{j}", [NTAB, CWB], bf16)

    import ml_dtypes

    iota_np = np.broadcast_to(
        np.tile(np.arange(P, dtype=np.float32), OB), (P, OB * P)
    ).astype(ml_dtypes.bfloat16)
    iota_d = nc.inline_tensor(iota_np, name="iota_rep")
    ident_d = nc.inline_tensor(np.eye(P, dtype=np.float32), name="ident")

    with ExitStack() as ctx:
        tc = ctx.enter_context(tile.TileContext(nc, num_cores=m))
        const = ctx.enter_context(tc.tile_pool(name="const", bufs=1))
        xp = ctx.enter_context(tc.tile_pool(name="xp", bufs=3))
        wp = ctx.enter_context(tc.tile_pool(name="wp", bufs=3))
        up = ctx.enter_context(tc.tile_pool(name="up", bufs=3))
        gp = ctx.enter_context(tc.tile_pool(name="gp", bufs=28))
        op = ctx.enter_context(tc.tile_pool(name="op", bufs=6))
        accp = ctx.enter_context(tc.tile_pool(name="accp", bufs=2))
        ep = ctx.enter_context(tc.tile_pool(name="ep", bufs=1))
        pst = ctx.enter_context(tc.tile_pool(name="pst", bufs=2, space="PSUM"))
        psu = ctx.enter_context(tc.tile_pool(name="psu", bufs=2, space="PSUM"))
        psa = ctx.enter_context(tc.tile_pool(name="psa", bufs=4, space="PSUM"))

        # ---- constants ----
        iota_s = const.tile([P, OB * P], bf16)
        nc.sync.dma_start(iota_s[:], iota_d[:, :])
        ident_s = const.tile([P, P], f32)
        nc.sync.dma_start(ident_s[:], ident_d[:, :])

        seg_s = const.tile([P, NSEG], bf16)
        nc.sync.dma_start(seg_s[:], seg_t[:, :])

        # whole gather-index stream resident in SBUF (no per-call DMAs)
        idx_s = const.tile([P, NCH * 8], mybir.dt.int16)
        nc.sync.dma_start(idx_s[:], idx_t[:, :])

        # V1cat [CIN, 4, HID], V2cat [HID, 4, OUT]
        w1s = const.tile([CIN, 4, HID], f32)
        nc.sync.dma_start(w1s[:], w1_t[:, :, :].rearrange("k p c -> p k c"))
        v1 = const.tile([CIN, 4, HID], f32)
        nc.vector.tensor_sub(v1[:, 0, :], w1s[:, 0, :], w1s[:, 2, :])
        nc.vector.tensor_scalar(
            out=v1[:, 1, :], in0=w1s[:, 3, :], scalar1=-3.0, scalar2=None,
            op0=mybir.AluOpType.mult,
        )
        nc.vector.tensor_add(v1[:, 1, :], v1[:, 1, :], w1s[:, 1, :])
        nc.vector.tensor_scalar(
            out=v1[:, 2, :], in0=w1s[:, 2, :], scalar1=2.0, scalar2=None,
            op0=mybir.AluOpType.mult,
        )
        nc.vector.tensor_scalar(
            out=v1[:, 3, :], in0=w1s[:, 3, :], scalar1=4.0, scalar2=None,
            op0=mybir.AluOpType.mult,
        )

        w2s = const.tile([HID, 4, OUT], f32)
        nc.sync.dma_start(w2s[:], w2_t[:, :, :].rearrange("k p c -> p k c"))
        v2 = const.tile([HID, 4, OUT], f32)
        nc.vector.tensor_sub(v2[:, 0, :], w2s[:, 0, :], w2s[:, 2, :])
        nc.vector.tensor_scalar(
            out=v2[:, 1, :], in0=w2s[:, 3, :], scalar1=-3.0, scalar2=None,
            op0=mybir.AluOpType.mult,
        )
        nc.vector.tensor_add(v2[:, 1, :], v2[:, 1, :], w2s[:, 1, :])
        nc.vector.tensor_scalar(
            out=v2[:, 2, :], in0=w2s[:, 2, :], scalar1=2.0, scalar2=None,
            op0=mybir.AluOpType.mult,
        )
        nc.vector.tensor_scalar(
            out=v2[:, 3, :], in0=w2s[:, 3, :], scalar1=4.0, scalar2=None,
            op0=mybir.AluOpType.mult,
        )

        b1s = const.tile([P, HID], f32)
        nc.sync.dma_start(b1s[:1, :], b1_t[:].rearrange("(o c) -> o c", o=1))
        nc.gpsimd.partition_broadcast(b1s[:, :], b1s[:1, :])
        b2s = const.tile([P, OUT], f32)
        nc.sync.dma_start(b2s[:1, :], b2_t[:].rearrange("(o c) -> o c", o=1))
        nc.gpsimd.partition_broadcast(b2s[:, :], b2s[:1, :])

        # DMA-completion semaphores for prepare_only gathers (one per queue)
        dma_sems = [nc.alloc_semaphore(f"swdge_dma{q}") for q in range(4)]

        # ---- dis, -dis, -dis^2 in [P, T] (partition p, slot t) = dest 128t+p ----
        degs = const.tile([P, T], f32)
        nc.sync.dma_start(degs[:], degt_t[:, :])
        dis = const.tile([P, T], f32)
        ndis = const.tile([P, T], f32)
        ndis2 = const.tile([P, T], f32)
        tmp = const.tile([P, T], f32)
        nc.vector.tensor_scalar(
            out=tmp[:], in0=degs[:], scalar1=1.0, scalar2=None,
            op0=mybir.AluOpType.max,
        )
        nc.scalar.activation(tmp[:], tmp[:], mybir.ActivationFunctionType.Sqrt)
        nc.vector.reciprocal(dis[:], tmp[:])
        nc.vector.tensor_scalar(
            out=tmp[:], in0=degs[:], scalar1=0.0, scalar2=None,
            op0=mybir.AluOpType.is_gt,
        )
        nc.vector.tensor_mul(dis[:], dis[:], tmp[:])
        nc.vector.tensor_scalar(
            out=ndis[:], in0=dis[:], scalar1=-1.0, scalar2=None,
            op0=mybir.AluOpType.mult,
        )
        nc.vector.tensor_mul(ndis2[:], dis[:], ndis[:])

        # ---- prologue: U_j = src @ V_j ; U1/U2 to HBM (j=1,2 scaled by dis);
        #      j=3 scaled -> blk  ----
        def prologue(src, cl, w, vcat, u_hbm, blk_hbm, from_sbuf=False):
            for k in range(T):
                if from_sbuf:
                    xc = src[:, k, :]
                else:
                    xc = xp.tile([P, cl], f32, tag="xc")
                    nc.sync.dma_start(xc[:], src[k * P : (k + 1) * P, :])
                tp = pst.tile([cl, P], f32, space="PSUM", tag="tp")
                nc.tensor.transpose(out=tp[:, :], in_=xc[:, :], identity=ident_s[:])
                xT = wp.tile([cl, P], f32, tag="xT")
                nc.scalar.copy(xT[:], tp[:, :])
                upsum = psu.tile([P, 4, w], f32, space="PSUM", tag="upsum")
                nc.tensor.matmul(
                    out=upsum[:].rearrange("p a c -> p (a c)"),
                    lhsT=xT[:, :],
                    rhs=vcat[:].rearrange("p a c -> p (a c)"),
                    start=True, stop=True,
                )
                ut = wp.tile([P, 3, w], f32, tag="ut")
                # fold bias into U_0 once (y = U0 + b - dis*(A S1))
                bb = b1s if w == HID else b2s
                nc.vector.tensor_add(ut[:, 0, :], upsum[:, 0, :], bb[:, :w])
                for j in (1, 2):
                    nc.vector.tensor_scalar(
                        out=ut[:, j, :], in0=upsum[:, j, :],
                        scalar1=dis[:, k : k + 1], scalar2=None,
                        op0=mybir.AluOpType.mult,
                    )
                nc.sync.dma_start(
                    u_hbm[:, k * P : (k + 1) * P, :].rearrange("a p c -> p a c"),
                    ut[:],
                )
                bt = wp.tile([P, w], bf16, tag="bt")
                nc.vector.tensor_scalar(
                    out=bt[:], in0=upsum[:, 3, :],
                    scalar1=dis[:, k : k + 1], scalar2=None,
                    op0=mybir.AluOpType.mult,
                )
                nc.sync.dma_start(blk_hbm[k * P : (k + 1) * P, :w], bt[:])

        # ---- one hop: table = AG(blk_in) split per bucket (overlaps the
        # gather passes); acc = scale*(A_gather) + U_j.  Edge stream is
        # (bucket, tile)-major with exact-max cells: chunks may straddle
        # tile boundaries, one matmul segment per (tile, chunk).  On the
        # last bucket pass, finished acc quarters are cast+written to
        # blk_out so the next hop's AllGather pieces can start early.
        def hop(l, j, blk_in, table, w, blk_out):
            for q in range(NB):
                nc.gpsimd.collective_compute(
                    "AllGather",
                    mybir.AluOpType.bypass,
                    replica_groups=[list(range(m))],
                    ins=[blk_in[QS[q] : QS[q] + QR[q], :].opt()],
                    outs=[table[8 * QS[q] : 8 * (QS[q] + QR[q]), :].opt()],
                )
            final = j == 0
            u_hbm = u1 if l == 1 else u2
            acc = accp.tile([P, T, w], f32, tag="acc")

            gath = None
            oneh = None
            o = 0           # segment ordinal (one-hot column)
            gcall = 0       # gather-call counter (round-robin SWDGE queues:
                            # each queue runs on its own Q7 cpu pair)
            # pass order: bucket-major (all of bucket bk across every tile
            # group) so CC AllGathers land in the gather lulls at hop
            # boundaries; quarter g of acc is written out as soon as its
            # bucket-3 batches finish.
            for bk in range(NB):
                gt0 = 0
                for g in range(NB):
                    gtn = QT[g]
                    tab_lo = 8 * QS[bk]
                    tab_hi = 8 * (QS[bk] + QR[bk])
                    cb = CHB[(g, bk)]
                    nloc = CHN[(g, bk)]
                    sched_p = SCHED[(g, bk)]
                    gnext = 0   # next ungathered pass-local chunk
                    gbase = 0   # current gather batch start (pass-local)
                    nbatch = (gtn + PB - 1) // PB
                    for bi in range(nbatch):
                        t0 = gt0 + bi * PB
                        tcnt = min(PB, gt0 + gtn - t0)
                        psum = psa.tile(
                            [P, PB, w], f32, space="PSUM", tag="apsum"
                        )
                        for tt in range(tcnt):
                            ti = t0 + tt
                            rng = sched_p[ti]
                            if rng is None:
                                nc.vector.memset(psum[:, tt, :], 0.0)
                                continue
                            c0l, c1l = rng
                            for k, cl in enumerate(range(c0l, c1l + 1)):
                                if cl >= gnext:
                                    glen = min(GCH, nloc - gnext)
                                    gbase = gnext
                                    cg = cb + gnext
                                    gath = gp.tile(
                                        [P, GCH, CWB], bf16, tag="gath"
                                    )
                                    nc.gpsimd.dma_gather(
                                        out_ap=gath[:, :glen, :],
                                        in_ap=table[tab_lo:tab_hi, :],
                                        idxs_ap=idx_s[
                                            :, cg * 8 : (cg + glen) * 8
                                        ],
                                        num_idxs=glen * P,
                                        num_idxs_reg=glen * P,
                                        elem_size=CWB,
                                        queue_num=gcall % 4,
                                    )
                                    gcall += 1
                                    gnext += glen
                                gpos = cl - gbase
                                oc, opos = divmod(o, OB)
                                if opos == 0:
                                    olen = min(OB, NSEG - oc * OB)
                                    oneh = op.tile(
                                        [P, OB, P], bf16, tag="oneh"
                                    )
                                    nc.vector.tensor_tensor(
                                        out=oneh[:, :olen, :],
                                        in0=iota_s[:].rearrange(
                                            "p (a q) -> p a q", q=P
                                        )[:, :olen, :],
                                        in1=seg_s[:, oc * OB : oc * OB + olen]
                                        .to_broadcast([P, olen, P]),
                                        op=mybir.AluOpType.is_equal,
                                    )
                                nc.tensor.matmul(
                                    out=psum[:, tt, :],
                                    lhsT=oneh[:, opos, :],
                                    rhs=gath[:, gpos, :w],
                                    start=(k == 0),
                                    stop=(k == c1l - c0l),
                                )
                                o += 1
                        # fold this pass's aggregation into acc
                        sl = slice(t0, t0 + tcnt)
                        if bk == 0:
                            nc.vector.tensor_copy(
                                acc[:, sl, :], psum[:, :tcnt, :]
                            )
                        else:
                            nc.vector.tensor_add(
                                acc[:, sl, :], acc[:, sl, :],
                                psum[:, :tcnt, :],
                            )
                        if bk == NB - 1:
                            scale = ndis if final else ndis2
                            nc.vector.tensor_tensor(
                                out=acc[:, sl, :],
                                in0=acc[:, sl, :],
                                in1=scale[:, sl].to_broadcast([P, tcnt, w]),
                                op=mybir.AluOpType.mult,
                            )
                            uti = up.tile([P, PB, w], f32, tag="uti")
                            nc.sync.dma_start(
                                uti[:, :tcnt, :],
                                u_hbm[
                                    j if not final else 0,
                                    t0 * P : (t0 + tcnt) * P, :,
                                ].rearrange("(a p) c -> p a c", p=P),
                            )
                            nc.vector.tensor_add(
                                acc[:, sl, :], acc[:, sl, :],
                                uti[:, :tcnt, :],
                            )
                            if final and l == 1:
                                nc.scalar.activation(
                                    acc[:, sl, :],
                                    acc[:, sl, :],
                                    mybir.ActivationFunctionType.Relu,
                                )
                    # acc quarter g final on the last bucket pass: cast +
                    # write to blk so the next hop's AG piece g starts early
                    if bk == NB - 1 and blk_out is not None:
                        ab = wp.tile([P, gtn, w], bf16, tag="accb")
                        nc.scalar.copy(
                            ab[:], acc[:, gt0 : gt0 + gtn, :]
                        )
                        nc.sync.dma_start(
                            blk_out[QS[g] : QS[g] + QR[g], :w].rearrange(
                                "(a p) c -> p a c", p=P
                            ),
                            ab[:],
                        )
                    gt0 += gtn
            return acc

        def acc_to_hbm(acc, dst, w):
            nc.sync.dma_start(
                dst[:, :].rearrange("(a p) c -> p a c", p=P), acc[:, :, :w]
            )

        # ================= layer 1 =================
        prologue(x_t, CIN, HID, v1, u1, blks[(1, 3)])
        acc = hop(1, 2, blks[(1, 3)], tabs[(1, 3)], HID, blks[(1, 2)])
        acc = hop(1, 1, blks[(1, 2)], tabs[(1, 2)], HID, blks[(1, 1)])
        acc = hop(1, 0, blks[(1, 1)], tabs[(1, 1)], HID, None)

        # ================= layer 2 (prologue reads h=acc from SBUF) ======
        prologue(acc, HID, OUT, v2, u2, blks[(2, 3)], from_sbuf=True)
        acc = hop(2, 2, blks[(2, 3)], tabs[(2, 3)], OUT, blks[(2, 2)])
        acc = hop(2, 1, blks[(2, 2)], tabs[(2, 2)], OUT, blks[(2, 1)])
        acc = hop(2, 0, blks[(2, 1)], tabs[(2, 1)], OUT, None)

        # ---- log_softmax over acc [P, T, OUT] (in place) ----
        red = wp.tile([P, T], f32, tag="red")
        nc.vector.tensor_reduce(
            out=red[:], in_=acc[:, :, :], axis=mybir.AxisListType.X,
            op=mybir.AluOpType.max,
        )
        nc.vector.tensor_tensor(
            out=acc[:], in0=acc[:, :, :], in1=red[:].to_broadcast([P, T, OUT]),
            op=mybir.AluOpType.subtract,
        )
        ex = ep.tile([P, T, OUT], f32, tag="ex")
        nc.scalar.activation(ex[:], acc[:], mybir.ActivationFunctionType.Exp)
        nc.vector.tensor_reduce(
            out=red[:], in_=ex[:, :, :], axis=mybir.AxisListType.X,
            op=mybir.AluOpType.add,
        )
        nc.scalar.activation(red[:], red[:], mybir.ActivationFunctionType.Ln)
        nc.vector.tensor_tensor(
            out=acc[:], in0=acc[:, :, :], in1=red[:].to_broadcast([P, T, OUT]),
            op=mybir.AluOpType.subtract,
        )
        nc.sync.dma_start(
            y_t[:, :].rearrange("(a p) c -> p a c", p=P), acc[:, :, :]
        )

    nc.compile()
    return nc


def make_in_maps(cfg: Cfg, inputs: dict, idx_all, seg_all, degt_all):
    x = np.asarray(inputs["x"], dtype=np.float32)
    maps = []
    for c in range(cfg.m):
        xb = np.zeros((cfg.bp, cfg.c_in), dtype=np.float32)
        xb[cfg.pos_of[c][: cfg.b]] = x[c * cfg.b : (c + 1) * cfg.b]
        maps.append(
            {
                "x_blk": xb,
                "W1": np.asarray(inputs["W1"], dtype=np.float32),
                "b1": np.asarray(inputs["b1"], dtype=np.float32),
                "W2": np.asarray(inputs["W2"], dtype=np.float32),
                "b2": np.asarray(inputs["b2"], dtype=np.float32),
                "deg_t": degt_all[c],
                "idx": idx_all[c],
                "seg": seg_all[c],
            }
        )
    return maps


def kernel(**inputs) -> np.ndarray:
    from concourse import bass_utils

    cfg = Cfg(n=100000, m=8)
    cfg.finish()
    edge_index = np.asarray(inputs["edge_index"])
    idx_all, seg_all, degt_all = preprocess(edge_index, cfg)
    nc = build_program(cfg)
    in_maps = make_in_maps(cfg, inputs, idx_all, seg_all, degt_all)
    res = bass_utils.run_bass_kernel_spmd(nc, in_maps, core_ids=list(range(cfg.m)))
    out = np.concatenate(
        [res.results[c]["y_blk"][cfg.pos_of[c][: cfg.b]] for c in range(cfg.m)],
        axis=0,
    )
    return out.astype(np.float32)

